# revision 26
# baseline (speedup 1.0000x reference)
"""CharCNN token embedder (ELMo-style) on 8 Trainium2 NeuronCores.

Data-parallel over the 4096 = 16*256 tokens (512 per core). Weights replicated.

Per-core pipeline (v4):
  1. Char-PAIR gather: host packs each token's 50 chars into 25 pairs and
     builds a per-core table of unique pairs (~12k rows, 32 bf16 values =
     both chars' embeddings); dma_gather pulls 64B rows -> 4x fewer
     descriptors than per-char gathering. Shifted strided SBUF->SBUF copies
     build the K=112 im2col patch matrix (xsA: positions < 22 ready after 2
     gather chunks so conv starts ~6us in; xsB after chunk 4).
  2. Tokens processed in two halves of 256: phase1 = conv(A); phase2 =
     conv(B) interleaved with highway+proj(A); phase3 = highway+proj(B).
  3. Conv = bf16 matmuls, K=112, one position per matmul, rounds of 8
     (phase1) / 6 (phase2) positions in double-buffered PSUM.
     Max-pool drain per tile into a stack accumulator S[128, rpos, HALF]
     (bf16 SBUF):
       round 0: ACT copies all rpos positions into S (fold-free fill).
       A-led rounds: ACT copy PSUM->tmp stack; stack-max merge S|=tmp on
         DVE (bf16 2x) or the otherwise-idle Pool engine (debt allocator).
       D-led rounds: DVE reduce_max of the PSUM round + slot merge.
     Finale: S folded by a small TT tree (DVE/Pool), bias+relu on DVE.
  4. fp8 conversions batched in tile/j pairs; h8 layout [128, hi/lo, 16, n].
  5. Highway layers in fp8 DoubleRow at 2x bf16 throughput (hi+lo activation
     chains, scaled e4m3, descale via ACT sigmoid/relu scale).
  6. Projection fp8 DoubleRow (hi, lo, W-residual chains); PE-transpose;
     DMA out straight from PSUM.
"""

import numpy as np
import ml_dtypes

import concourse.bass as bass
import concourse.mybir as mybir
import concourse.tile as tile
from concourse import bacc
from concourse.bass_utils import run_bass_kernel_spmd
from concourse.vector_clock import ScopedClock

# ---------------------------------------------------------------- constants
B, S, L = 16, 256, 50
CHAR_DIM = 16
CHAR_VOCAB = 262
FILTERS = [(1, 32), (2, 32), (3, 64), (4, 128), (5, 256), (6, 512), (7, 1024)]
N_FILTERS = 2048
PROJ_DIM = 512
N_CORES = 8
NTOK = B * S
TOK = NTOK // N_CORES        # 512 tokens per core
NPOS = 50
NPAIR = 25                   # char pairs per token
NPPAD = 28                   # padded pair positions (tap reach 55 -> pair 27)
NI = TOK * NPAIR             # 12800 gather indices per core
TABLE_ROWS = 32768           # fixed per-core unique-pair table allocation
KDIM = 112                   # 7 taps * 16 dims
TSPLIT = 22                  # conv positions < TSPLIT read xsA

S_W = 512.0                  # fp8 storage scale for highway/proj weights
S_H = 32.0                   # fp8 storage scale for highway/proj activations
DESCALE = 1.0 / (S_W * S_H)

# per 128-channel tile: valid positions; tile 0 packs w=1,2,3 with tails
CH_TILES = []
CH_TILES.append({"t_main": 48, "tails": [(0, 32, 50), (32, 64, 49), (64, 128, 48)]})
CH_TILES.append({"t_main": 47, "tails": [(0, 128, 47)]})      # w4
for _ in range(2):
    CH_TILES.append({"t_main": 46, "tails": [(0, 128, 46)]})  # w5
for _ in range(4):
    CH_TILES.append({"t_main": 45, "tails": [(0, 128, 45)]})  # w6
for _ in range(8):
    CH_TILES.append({"t_main": 44, "tails": [(0, 128, 44)]})  # w7

BF16 = mybir.dt.bfloat16
FP32 = mybir.dt.float32
FP8 = mybir.dt.float8e4

# drain schedule fractions: (pA, pPoolMerge, pPoolFinale) per phase
PH1 = (0.82, 0.0, 0.0)
PH2 = (0.75, 0.0, 0.0)

_MAX_WAITS_PER_INST = 1


def _patched_drain_and_barrier(self, tick_clock, wait_clock):
    # The walrus build in this container rejects CTRL instructions carrying
    # more than one sem wait; spread the kernel-tail drain waits over NOPs.
    nc = self.nc
    carrier = nc.sync.nop()
    wait_clock.add_sem_waits(carrier.ins, ScopedClock({None: tick_clock.global_clock}))
    si = carrier.ins.sync_info
    waits = list(si.on_wait) if si is not None and si.on_wait else []
    if len(waits) > _MAX_WAITS_PER_INST:
        carrier.ins.sync_info = mybir.SyncInfo(
            on_wait=waits[:_MAX_WAITS_PER_INST],
            on_update=list(si.on_update) if si.on_update else [])
        for i in range(_MAX_WAITS_PER_INST, len(waits), _MAX_WAITS_PER_INST):
            extra = nc.sync.nop()
            extra.ins.sync_info = mybir.SyncInfo(
                on_wait=waits[i:i + _MAX_WAITS_PER_INST], on_update=[])
    nc.sync.drain()
    nc.all_engine_barrier()
    assert self.sems is not None
    popped = nc._tile_sem_poison_stack.pop()
    assert popped is self._sem_poison
    nc.clear_and_free_semaphores(list(self.sems.allocated().values()))
    nc.all_engine_barrier()


tile.TileContext._drain_and_barrier = _patched_drain_and_barrier


class DrainSched:
    """Debt-based allocator: round kind (A/D), fold + merge engines."""

    def __init__(self, nc, pA, unused0=0.0, unused1=0.0):
        self.nc = nc
        self.pA = pA
        self.da = 0.0

    def kind(self):
        self.da += self.pA
        if self.da >= 1.0:
            self.da -= 1.0
            return "A"
        return "D"

    def note_forced(self, k):
        pass

    def fold_eng(self):
        return self.nc.vector


# ---------------------------------------------------------------- device IR
def build_module():
    nc = bacc.Bacc()
    SIdx = NI // 16

    # 256-byte rows (dma_gather granularity); cols 0:32 hold the pair embs
    table = nc.dram_tensor("table", [TABLE_ROWS, 128], BF16,
                           kind="ExternalInput")
    idx = nc.dram_tensor("idx", [128, SIdx], mybir.dt.int16, kind="ExternalInput")
    wconv = nc.dram_tensor("wconv", [KDIM, N_FILTERS], BF16, kind="ExternalInput")
    bconv = nc.dram_tensor("bconv", [128, 16], FP32, kind="ExternalInput")
    # highway weights fp8, host-packed per (layer, j):
    #   [l, j, p(128), cc(8), g(2), half*128+oc] ; g = DoubleRow group
    whw = nc.dram_tensor("whw", [2, 16, 128, 8, 2, 256], FP8, kind="ExternalInput")
    bhw = nc.dram_tensor("bhw", [2, 128, 16, 2], FP32, kind="ExternalInput")
    wproj = nc.dram_tensor("wproj", [128, 8, 2, 512], FP8, kind="ExternalInput")
    wprojc = nc.dram_tensor("wprojc", [128, 8, 2, 512], FP8, kind="ExternalInput")
    bproj = nc.dram_tensor("bproj", [128, 4], FP32, kind="ExternalInput")
    ident = nc.dram_tensor("ident", [128, 128], FP32, kind="ExternalInput")
    out = nc.dram_tensor("out", [TOK, PROJ_DIM], FP32, kind="ExternalOutput")

    with tile.TileContext(nc) as tc:
        with (
            tc.tile_pool(name="xs", bufs=1) as xspool,
            tc.tile_pool(name="consts", bufs=1) as cpool,
        ):
            # ---- constants in (only wconv before the gather stream; the
            # rest is deferred so it does not delay xsA on the DMA device)
            wconv_t = cpool.tile([KDIM, N_FILTERS], BF16)
            nc.sync.dma_start(out=wconv_t[:], in_=wconv[:])
            bconv_t = cpool.tile([128, 16], FP32)
            bhw_t = cpool.tile([128, 2, 16, 2], FP32)
            bproj_t = cpool.tile([128, 4], FP32)
            ident_t = cpool.tile([128, 128], FP32)
            wproj_t = cpool.tile([128, 8, 2, 512], FP8)
            wprojc_t = cpool.tile([128, 8, 2, 512], FP8)

            # ---- 1. pair-gather char embeddings + build K=112 patch matrix.
            # xsA/xsB viewed [128, m, parity, TOK] so the strided parity
            # interleave is a plain AP (no step slicing).
            NA = TSPLIT // 2          # 11 position pairs in xsA
            NB = (NPOS - TSPLIT) // 2  # 14 in xsB
            xsA = xspool.tile([KDIM, NA, 2, TOK], BF16, name="xsA")
            xsB = xspool.tile([KDIM, NB, 2, TOK], BF16, name="xsB")
            with tc.tile_pool(name="gather", bufs=1) as gpool:
                idx_t = gpool.tile([128, SIdx], mybir.dt.int16)
                nc.sync.dma_start(out=idx_t[:], in_=idx[:])
                xg = gpool.tile([128, 1, TOK * NPPAD], BF16)
                nc.vector.memset(xg[0:32, 0, NI:TOK * NPPAD], 0.0)
                # pair-aligned chunks (idx counts): 7,7,7,4 pairs
                chunks = [(0, 3584), (3584, 3584), (7168, 3584), (10752, 2048)]
                for r, (o, cn) in enumerate(chunks):
                    nc.gpsimd.dma_gather(
                        out_ap=xg[:, :, o:o + cn],
                        in_ap=table[:],
                        idxs_ap=idx_t[:, o // 16:(o + cn) // 16],
                        num_idxs=cn,
                        num_idxs_reg=cn,
                        elem_size=128,
                        transpose=True,
                        single_packet=False,
                    )
                    if r == 1:
                        # xsA copies, fused over (k, k+1) pairs where the
                        # source partition blocks are contiguous (halves the
                        # HWDGE descriptor-generation serial chain)
                        for s in range(2):
                            k = 0
                            while k < 7:
                                p0 = (s + k) // 2
                                par = (s + k) % 2
                                if par == 0 and k + 1 < 7:
                                    nc.sync.dma_start(
                                        out=xsA[16 * k:16 * (k + 2), :, s, :],
                                        in_=xg[0:32, 0,
                                               TOK * p0:TOK * (p0 + NA)],
                                    )
                                    k += 2
                                else:
                                    nc.sync.dma_start(
                                        out=xsA[16 * k:16 * (k + 1), :, s, :],
                                        in_=xg[16 * par:16 * par + 16, 0,
                                               TOK * p0:TOK * (p0 + NA)],
                                    )
                                    k += 1
                for s in range(2):
                    k = 0
                    while k < 7:
                        t0 = TSPLIT + s
                        p0 = (t0 + k) // 2
                        par = (t0 + k) % 2
                        if par == 0 and k + 1 < 7:
                            nc.sync.dma_start(
                                out=xsB[16 * k:16 * (k + 2), :, s, :],
                                in_=xg[0:32, 0, TOK * p0:TOK * (p0 + NB)],
                            )
                            k += 2
                        else:
                            nc.sync.dma_start(
                                out=xsB[16 * k:16 * (k + 1), :, s, :],
                                in_=xg[16 * par:16 * par + 16, 0,
                                       TOK * p0:TOK * (p0 + NB)],
                            )
                            k += 1
            nc.sync.dma_start(out=bconv_t[:], in_=bconv[:])
            nc.sync.dma_start(out=bhw_t[:], in_=bhw[:].rearrange("l p j h -> p l j h"))
            nc.sync.dma_start(out=bproj_t[:], in_=bproj[:])
            nc.sync.dma_start(out=ident_t[:], in_=ident[:])

            HALF = TOK // 2

            def conv_rhs(t, hlo):
                if t < TSPLIT:
                    return xsA[:, t // 2, t % 2, hlo:hlo + HALF]
                tl = t - TSPLIT
                return xsB[:, tl // 2, tl % 2, hlo:hlo + HALF]

            stack = tc.tile_pool(name="hbuf", bufs=1)
            hpool = stack.__enter__()
            stack2 = tc.tile_pool(name="h8buf", bufs=1)
            h8pool = stack2.__enter__()
            stack3 = tc.tile_pool(name="wstream", bufs=3)
            wpool = stack3.__enter__()
            stack4 = tc.tile_pool(name="small", bufs=2)
            spool = stack4.__enter__()
            stack6 = tc.tile_pool(name="accpool", bufs=4)
            accpool = stack6.__enter__()
            convp = None  # phase-2/3 PSUM pool, opened after phase 1

            # per-half persistent tensors (separate tiles avoid false deps)
            h1 = [hpool.tile([128, 16, HALF], BF16, tag=f"h1{s_}", name=f"h1{s_}")
                  for s_ in range(2)]
            hmid = [hpool.tile([128, 16, HALF], BF16, tag=f"hm{s_}", name=f"hm{s_}")
                    for s_ in range(2)]
            # h8 layout: [128, hi/lo, j, n] so (j, j+1) casts batch into one op
            h8c = [h8pool.tile([128, 2, 16, HALF], FP8, tag=f"h8c{s_}", name=f"h8c{s_}")
                   for s_ in range(2)]
            h8m = [h8pool.tile([128, 2, 16, HALF], FP8, tag=f"h8m{s_}", name=f"h8m{s_}")
                   for s_ in range(2)]
            # h8f aliases h8c: layer-0 chains fully consume h8c before
            # layer-1 writes the final activations (WAR handled by deps)
            h8f = h8c

            def cast_pair(h_bf, j0, nj, h8, lo_eng=None):
                # hi = fp8(h * S_H) on ACT; lo = fp8(h*S_H - hi) on DVE/Pool
                nc.scalar.activation(
                    out=h8[:, 0, j0:j0 + nj, :], in_=h_bf[:, j0:j0 + nj, :],
                    func=mybir.ActivationFunctionType.Copy, scale=S_H)
                (lo_eng or nc.vector).scalar_tensor_tensor(
                    out=h8[:, 1, j0:j0 + nj, :], in0=h_bf[:, j0:j0 + nj, :],
                    scalar=S_H, in1=h8[:, 0, j0:j0 + nj, :],
                    op0=mybir.AluOpType.mult, op1=mybir.AluOpType.subtract)

            def conv_half(hf, rpos, pool, sched):
                """Generator: conv + max-pool drain for token half hf.

                Touches (PSUM reads) emit immediately; folds/merges/finales
                emit one round late via `pending` so no engine head-of-line
                blocks on a cross-engine dependency that is not ready yet.
                """
                from collections import deque
                hlo = HALF * hf
                hw_ = rpos // 2  # acc width
                pending = deque()
                pcast = deque()   # fp8 casts lag one extra round so the ACT
                                  # hi-cast never parks waiting the DVE finale

                def flush(keep):
                    while len(pending) > keep:
                        pending.popleft()()
                    while len(pcast) > max(keep, 1) + 1:
                        pcast.popleft()()

                DX = 3  # spare direct-write slots for D-round reduces
                for i, spec in enumerate(CH_TILES):
                    lhsT = wconv_t[:, 128 * i:128 * (i + 1)]
                    t_main = spec["t_main"]
                    acc = accpool.tile([128, hw_ + DX, HALF], BF16, tag="acc")
                    first = True
                    dstate = {"next": hw_}
                    t0 = 0
                    while t0 < t_main:
                        nt = min(rpos, t_main - t0)
                        P = pool.tile([128, rpos, HALF], FP32, tag=f"ps{rpos}")
                        for r in range(nt):
                            nc.tensor.matmul(
                                out=P[:, r, :], lhsT=lhsT,
                                rhs=conv_rhs(t0 + r, hlo),
                                start=True, stop=True)
                        flush(1)
                        if nt == rpos and (first or sched.kind() == "A"):
                            if first:
                                sched.note_forced("A")
                            tmp = spool.tile([128, rpos, HALF], BF16,
                                             tag="astk", bufs=3)
                            nc.scalar.activation(
                                out=tmp[:], in_=P[:],
                                func=mybir.ActivationFunctionType.Copy, scale=1.0)
                            eng = sched.fold_eng()
                            if first:
                                def op(eng=eng, tmp=tmp, acc=acc):
                                    eng.tensor_tensor(
                                        out=acc[:, 0:hw_, :],
                                        in0=tmp[:, 0:hw_, :],
                                        in1=tmp[:, hw_:rpos, :],
                                        op=mybir.AluOpType.max)
                                first = False
                            else:
                                def op(eng=eng, tmp=tmp, acc=acc):
                                    fh = spool.tile([128, hw_, HALF], BF16,
                                                    tag="fh", bufs=3)
                                    eng.tensor_tensor(
                                        out=fh[:], in0=tmp[:, 0:hw_, :],
                                        in1=tmp[:, hw_:rpos, :],
                                        op=mybir.AluOpType.max)
                                    eng.tensor_tensor(
                                        out=acc[:, 0:hw_, :],
                                        in0=acc[:, 0:hw_, :], in1=fh[:],
                                        op=mybir.AluOpType.max)
                            pending.append(op)
                        elif nt == 1:
                            nc.vector.tensor_tensor(
                                out=acc[:, 0, :], in0=acc[:, 0, :],
                                in1=P[:, 0, :], op=mybir.AluOpType.max)
                        elif dstate["next"] < hw_ + DX:
                            # D-led: reduce straight into a spare acc slot
                            nc.vector.reduce_max(
                                out=acc[:, dstate["next"], :],
                                in_=P[:, 0:nt, :].rearrange("p t n -> p n t"),
                                axis=mybir.AxisListType.X)
                            dstate["next"] += 1
                        else:
                            part = spool.tile([128, HALF], BF16, tag="part",
                                              bufs=3)
                            nc.vector.reduce_max(
                                out=part[:],
                                in_=P[:, 0:nt, :].rearrange("p t n -> p n t"),
                                axis=mybir.AxisListType.X)

                            def op(part=part, acc=acc):
                                nc.vector.tensor_tensor(
                                    out=acc[:, 0, :], in0=acc[:, 0, :],
                                    in1=part[:], op=mybir.AluOpType.max)
                            pending.append(op)
                        t0 += nt
                    # ragged tails (tile 0): positions t_main..50 on partition
                    # subranges; reduces touch PSUM now, merges deferred
                    if spec["tails"][0][2] > t_main:
                        nt = spec["tails"][0][2] - t_main
                        P = pool.tile([128, rpos, HALF], FP32, tag=f"ps{rpos}")
                        for r in range(nt):
                            nc.tensor.matmul(
                                out=P[:, r, :], lhsT=lhsT,
                                rhs=conv_rhs(t_main + r, hlo),
                                start=True, stop=True)
                        for (lo, hi, g_cnt) in spec["tails"]:
                            g_nt = g_cnt - t_main
                            if g_nt <= 0:
                                continue
                            if g_nt == 1:
                                nc.vector.tensor_tensor(
                                    out=acc[lo:hi, 0, :], in0=acc[lo:hi, 0, :],
                                    in1=P[lo:hi, 0, :], op=mybir.AluOpType.max)
                            else:
                                part = spool.tile([128, HALF], BF16, tag="part",
                                                  bufs=3)
                                nc.vector.reduce_max(
                                    out=part[lo:hi, :],
                                    in_=P[lo:hi, 0:g_nt, :].rearrange(
                                        "p t n -> p n t"),
                                    axis=mybir.AxisListType.X)

                                def op(part=part, acc=acc, lo=lo, hi=hi):
                                    nc.vector.tensor_tensor(
                                        out=acc[lo:hi, 1, :],
                                        in0=acc[lo:hi, 1, :],
                                        in1=part[lo:hi, :],
                                        op=mybir.AluOpType.max)
                                pending.append(op)

                    used = dstate["next"]

                    def finale(i=i, acc=acc, used=used):
                        pre = spool.tile([128, HALF], BF16, tag="pre")
                        cw = used
                        while cw > 2:
                            if cw % 2:
                                nc.vector.tensor_tensor(
                                    out=acc[:, 0, :], in0=acc[:, 0, :],
                                    in1=acc[:, cw - 1, :],
                                    op=mybir.AluOpType.max)
                                cw -= 1
                            h = cw // 2
                            nc.vector.tensor_tensor(
                                out=acc[:, 0:h, :], in0=acc[:, 0:h, :],
                                in1=acc[:, h:cw, :], op=mybir.AluOpType.max)
                            cw = h
                        nc.vector.tensor_tensor(
                            out=pre[:], in0=acc[:, 0, :], in1=acc[:, 1, :],
                            op=mybir.AluOpType.max)
                        nc.vector.tensor_scalar(
                            out=h1[hf][:, i, :], in0=pre[:],
                            scalar1=bconv_t[:, i:i + 1], scalar2=0.0,
                            op0=mybir.AluOpType.add, op1=mybir.AluOpType.max)
                    pending.append(finale)
                    if i % 2 == 1:
                        def cst(i=i):
                            cast_pair(h1[hf], i - 1, 2, h8c[hf])
                        pcast.append(cst)
                    yield
                flush(0)
                while pcast:
                    pcast.popleft()()

            def hw_mm_chain(p_out, wslab, h8, ofs):
                # W8 x (h_hi + h_lo); Wl correction skipped for the highway
                for hl in range(2):
                    for cc in range(8):
                        nc.tensor.matmul(
                            out=p_out, lhsT=wslab[:, cc, :, ofs:ofs + 128],
                            rhs=h8[:, hl, 2 * cc:2 * cc + 2, :],
                            start=(hl == 0 and cc == 0), stop=(hl == 1 and cc == 7),
                            perf_mode=mybir.MatmulPerfMode.DoubleRow)

            def hw_mm_chain_proj(p_out, h8, ofs):
                for hl in range(2):
                    for cc in range(8):
                        nc.tensor.matmul(
                            out=p_out, lhsT=wproj_t[:, cc, :, ofs:ofs + 128],
                            rhs=h8[:, hl, 2 * cc:2 * cc + 2, :],
                            start=(hl == 0 and cc == 0), stop=False,
                            perf_mode=mybir.MatmulPerfMode.DoubleRow)
                for cc in range(8):
                    nc.tensor.matmul(
                        out=p_out, lhsT=wprojc_t[:, cc, :, ofs:ofs + 128],
                        rhs=h8[:, 0, 2 * cc:2 * cc + 2, :],
                        start=False, stop=(cc == 7),
                        perf_mode=mybir.MatmulPerfMode.DoubleRow)

            def hw_half(hf):
                """Generator: highway l0+l1 + proj for token half hf.

                PE chains emit immediately; ACT/DVE epilogues lag one j so
                neither engine parks at its queue head waiting on a chain.
                """
                from collections import deque
                pending = deque()

                def flush(keep):
                    while len(pending) > keep:
                        pending.popleft()()

                state = {}
                for layer in range(2):
                    h_in = h1[hf] if layer == 0 else hmid[hf]
                    h8_in = h8c[hf] if layer == 0 else h8m[hf]
                    h8_out = h8m[hf] if layer == 0 else h8f[hf]
                    for j in range(16):
                        wslab = wpool.tile([128, 8, 2, 256], FP8, tag="wslab")
                        nc.sync.dma_start(out=wslab[:], in_=whw[layer, j])
                        hp = convp.tile([128, 2, HALF], FP32, tag="hwps",
                                        name="hp", bufs=2)
                        p_nl = hp[:, 0, :]
                        p_g = hp[:, 1, :]
                        hw_mm_chain(p_nl, wslab, h8_in, 0)
                        hw_mm_chain(p_g, wslab, h8_in, 128)
                        flush(1)

                        def epi(layer=layer, j=j, p_nl=p_nl, p_g=p_g,
                                h_in=h_in, h8_out=h8_out):
                            nl = spool.tile([128, HALF], BF16, tag="nl")
                            gt = spool.tile([128, HALF], BF16, tag="gt")
                            nc.scalar.activation(
                                out=nl[:], in_=p_nl,
                                func=mybir.ActivationFunctionType.Relu,
                                bias=bhw_t[:, layer, j, 0:1], scale=DESCALE)
                            nc.scalar.activation(
                                out=gt[:], in_=p_g,
                                func=mybir.ActivationFunctionType.Sigmoid,
                                bias=bhw_t[:, layer, j, 1:2], scale=DESCALE)
                            d = spool.tile([128, HALF], BF16, tag="d")
                            nc.vector.tensor_tensor(
                                out=d[:], in0=h_in[:, j, :], in1=nl[:],
                                op=mybir.AluOpType.subtract)
                            m = spool.tile([128, HALF], BF16, tag="m")
                            nc.vector.tensor_mul(out=m[:], in0=gt[:], in1=d[:])
                            if layer == 0:
                                nc.vector.tensor_add(
                                    out=hmid[hf][:, j, :], in0=nl[:], in1=m[:])
                                if j % 2 == 1:
                                    cast_pair(hmid[hf], j - 1, 2, h8_out)
                            else:
                                if j % 2 == 0:
                                    state["htp"] = spool.tile(
                                        [128, 2, HALF], BF16, tag="htp",
                                        name="htp")
                                htp = state["htp"]
                                nc.vector.tensor_add(
                                    out=htp[:, j % 2, :], in0=nl[:], in1=m[:])
                                if j % 2 == 1:
                                    nc.scalar.activation(
                                        out=h8_out[:, 0, j - 1:j + 1, :],
                                        in_=htp[:],
                                        func=mybir.ActivationFunctionType.Copy,
                                        scale=S_H)
                                    nc.vector.scalar_tensor_tensor(
                                        out=h8_out[:, 1, j - 1:j + 1, :],
                                        in0=htp[:], scalar=S_H,
                                        in1=h8_out[:, 0, j - 1:j + 1, :],
                                        op0=mybir.AluOpType.mult,
                                        op1=mybir.AluOpType.subtract)
                        pending.append(epi)
                        yield
                    # layer barrier: next layer's chains read every h8 column
                    flush(0)
                # projection + transpose + out for this half
                hlo = HALF * hf
                for j2 in range(4):
                    hp = convp.tile([128, 2, HALF], FP32, tag="hwps",
                                    name="hp", bufs=2)
                    p_o = hp[:, 0, :]
                    hw_mm_chain_proj(p_o, h8f[hf], 128 * j2)
                    flush(1)

                    def proj_epi(j2=j2, hp=hp, p_o=p_o):
                        ot = spool.tile([128, HALF], FP32, tag="ot")
                        nc.scalar.activation(
                            out=ot[:], in_=p_o,
                            func=mybir.ActivationFunctionType.Identity,
                            bias=bproj_t[:, j2:j2 + 1], scale=DESCALE)
                        for m4 in range(2):
                            p_t = hp[:, 1, 128 * m4:128 * (m4 + 1)]
                            nc.tensor.transpose(
                                out=p_t, in_=ot[:, 128 * m4:128 * (m4 + 1)],
                                identity=ident_t[:])
                            ob = spool.tile([128, 128], FP32, tag="ob")
                            nc.scalar.copy(out=ob[:], in_=p_t)
                            row0 = hlo + 128 * m4
                            nc.sync.dma_start(
                                out=out[row0:row0 + 128,
                                        128 * j2:128 * (j2 + 1)],
                                in_=ob[:])
                    pending.append(proj_epi)
                    yield
                flush(0)

            # ---- phase 1: conv half A, 8-position rounds, all 8 PSUM banks
            sched1 = DrainSched(nc, *PH1)
            with tc.tile_pool(name="convp8", bufs=2, space="PSUM") as p8pool:
                for _ in conv_half(0, 8, p8pool, sched1):
                    pass
            stack5 = tc.tile_pool(name="convp", bufs=2, space="PSUM")
            convp = stack5.__enter__()
            # WAW-gate the projection-weight loads behind a DVE op that sits
            # late in DVE program order, so they cannot steal the DMA device
            # from the gather at t=0 (the sim schedules by readiness)
            nc.vector.memset(wproj_t[0:1, 0:1, 0:1, 0:1], 0.0)
            nc.vector.memset(wprojc_t[0:1, 0:1, 0:1, 0:1], 0.0)
            nc.sync.dma_start(out=wproj_t[:], in_=wproj[:])
            nc.sync.dma_start(out=wprojc_t[:], in_=wprojc[:])
            # ---- phase 2: conv half B interleaved with highway+proj half A
            sched2 = DrainSched(nc, *PH2)
            genB = conv_half(1, 6, convp, sched2)
            genA = hw_half(0)
            unitsB, unitsA = 16, 36
            credit = 0.0
            doneB = doneA = False
            while not (doneB and doneA):
                credit += unitsA / unitsB
                if not doneB:
                    doneB = next(genB, "end") == "end"
                while credit >= 1.0 and not doneA:
                    doneA = next(genA, "end") == "end"
                    credit -= 1.0
                if doneB:
                    while not doneA:
                        doneA = next(genA, "end") == "end"
            # ---- phase 3: highway+proj half B
            for _ in hw_half(1):
                pass

            for st in (stack5, stack6, stack4, stack3, stack2, stack):
                st.__exit__(None, None, None)

    nc.compile()
    return nc


_CACHED = {}


def _prep(inputs):
    """Host-side layout prep: sharding, pair tables, weight packing."""
    chars = np.asarray(inputs["chars"]).astype(np.int64).reshape(NTOK, L)
    pairs = chars[:, 0::2] * CHAR_VOCAB + chars[:, 1::2]   # [NTOK, 25]

    emb = np.asarray(inputs["char_emb"], np.float32)

    wc = np.zeros((7, CHAR_DIM, N_FILTERS), np.float32)
    off = 0
    for fi, (w, n) in enumerate(FILTERS):
        cw = np.asarray(inputs[f"conv_w_{fi}"], np.float32)
        wc[:w, :, off:off + n] = cw.transpose(2, 1, 0)
        off += n
    wconv = wc.reshape(KDIM, N_FILTERS).astype(ml_dtypes.bfloat16)
    bconv = np.concatenate([np.asarray(inputs[f"conv_b_{i}"], np.float32)
                            for i in range(7)])
    bconv_dev = bconv.reshape(16, 128).T.copy()

    # highway weights: fp8 W8 packed for DoubleRow streaming.
    whw8 = np.zeros((2, 16, 128, 8, 2, 256), np.float32)
    bhw = np.zeros((2, 128, 16, 2), np.float32)
    for l in range(2):
        W = np.asarray(inputs[f"hw_w_{l}"], np.float32)   # (4096, 2048)
        bb = np.asarray(inputs[f"hw_b_{l}"], np.float32)
        Ws = W * S_W
        W8 = Ws.astype(ml_dtypes.float8_e4m3).astype(np.float32)
        W8T = W8.T  # (2048 ic, 4096 oc)
        for j in range(16):
            for hf in range(2):
                oc0 = 2048 * hf + 128 * j
                for cc in range(8):
                    for g in range(2):
                        cb = 2 * cc + g
                        blk8 = W8T[128 * cb:128 * (cb + 1), oc0:oc0 + 128]
                        whw8[l, j, :, cc, g, 128 * hf:128 * hf + 128] = blk8
            bhw[l, :, j, 0] = bb[128 * j:128 * (j + 1)]
            bhw[l, :, j, 1] = bb[2048 + 128 * j:2048 + 128 * (j + 1)]
    whw8 = whw8.astype(ml_dtypes.float8_e4m3)

    Wp = np.asarray(inputs["proj_w"], np.float32) * S_W  # (512, 2048)
    Wp8 = Wp.astype(ml_dtypes.float8_e4m3).astype(np.float32)
    Wpl = (Wp - Wp8).astype(ml_dtypes.float8_e4m3).astype(np.float32)
    Wp8T = Wp8.T  # (2048, 512)
    WplT = Wpl.T
    wproj8 = np.zeros((128, 8, 2, 512), np.float32)
    wprojc8 = np.zeros((128, 8, 2, 512), np.float32)
    for cc in range(8):
        for g in range(2):
            cb = 2 * cc + g
            wproj8[:, cc, g, :] = Wp8T[128 * cb:128 * (cb + 1), :]
            wprojc8[:, cc, g, :] = WplT[128 * cb:128 * (cb + 1), :]
    wproj8 = wproj8.astype(ml_dtypes.float8_e4m3)
    wprojc8 = wprojc8.astype(ml_dtypes.float8_e4m3)
    bproj = np.zeros((128, 4), np.float32)
    bp = np.asarray(inputs["proj_b"], np.float32)
    for j2 in range(4):
        bproj[:, j2] = bp[128 * j2:128 * (j2 + 1)]

    ident = np.eye(128, dtype=np.float32)

    shared = dict(wconv=wconv, bconv=bconv_dev, whw=whw8,
                  bhw=bhw, wproj=wproj8, wprojc=wprojc8,
                  bproj=bproj, ident=ident)

    in_maps = []
    for core in range(N_CORES):
        cp = pairs[core * TOK:(core + 1) * TOK]            # [512, 25]
        uniq, inv = np.unique(cp, return_inverse=True)
        assert len(uniq) <= TABLE_ROWS, len(uniq)
        tbl = np.zeros((TABLE_ROWS, 128), np.float32)
        tbl[:len(uniq), 0:CHAR_DIM] = emb[uniq // CHAR_VOCAB]
        tbl[:len(uniq), CHAR_DIM:2 * CHAR_DIM] = emb[uniq % CHAR_VOCAB]
        idx_flat = inv.reshape(TOK, NPAIR).T.reshape(-1).astype(np.int16)
        idx16 = idx_flat.reshape(NI // 16, 16).T.copy()
        idx16 = np.tile(idx16, (8, 1))
        m = dict(shared)
        m["table"] = tbl.astype(ml_dtypes.bfloat16)
        m["idx"] = idx16
        in_maps.append(m)
    return in_maps


def kernel(**inputs) -> np.ndarray:
    if "nc" not in _CACHED:
        _CACHED["nc"] = build_module()
    nc = _CACHED["nc"]
    in_maps = _prep(inputs)
    res = run_bass_kernel_spmd(nc, in_maps, core_ids=list(range(N_CORES)))
    full = np.concatenate([r["out"] for r in res.results], axis=0)
    return full.reshape(B, S, PROJ_DIM)


# revision 27
# speedup vs baseline: 1.0002x; 1.0002x over previous
"""CharCNN token embedder (ELMo-style) on 8 Trainium2 NeuronCores.

Data-parallel over the 4096 = 16*256 tokens (512 per core). Weights replicated.

Per-core pipeline (v4):
  1. Char-PAIR gather: host packs each token's 50 chars into 25 pairs and
     builds a per-core table of unique pairs (~12k rows, 32 bf16 values =
     both chars' embeddings); dma_gather pulls 64B rows -> 4x fewer
     descriptors than per-char gathering. Shifted strided SBUF->SBUF copies
     build the K=112 im2col patch matrix (xsA: positions < 22 ready after 2
     gather chunks so conv starts ~6us in; xsB after chunk 4).
  2. Tokens processed in two halves of 256: phase1 = conv(A); phase2 =
     conv(B) interleaved with highway+proj(A); phase3 = highway+proj(B).
  3. Conv = bf16 matmuls, K=112, one position per matmul, rounds of 8
     (phase1) / 6 (phase2) positions in double-buffered PSUM.
     Max-pool drain per tile into a stack accumulator S[128, rpos, HALF]
     (bf16 SBUF):
       round 0: ACT copies all rpos positions into S (fold-free fill).
       A-led rounds: ACT copy PSUM->tmp stack; stack-max merge S|=tmp on
         DVE (bf16 2x) or the otherwise-idle Pool engine (debt allocator).
       D-led rounds: DVE reduce_max of the PSUM round + slot merge.
     Finale: S folded by a small TT tree (DVE/Pool), bias+relu on DVE.
  4. fp8 conversions batched in tile/j pairs; h8 layout [128, hi/lo, 16, n].
  5. Highway layers in fp8 DoubleRow at 2x bf16 throughput (hi+lo activation
     chains, scaled e4m3, descale via ACT sigmoid/relu scale).
  6. Projection fp8 DoubleRow (hi, lo, W-residual chains); PE-transpose;
     DMA out straight from PSUM.
"""

import numpy as np
import ml_dtypes

import concourse.bass as bass
import concourse.mybir as mybir
import concourse.tile as tile
from concourse import bacc
from concourse.bass_utils import run_bass_kernel_spmd
from concourse.vector_clock import ScopedClock

# ---------------------------------------------------------------- constants
B, S, L = 16, 256, 50
CHAR_DIM = 16
CHAR_VOCAB = 262
FILTERS = [(1, 32), (2, 32), (3, 64), (4, 128), (5, 256), (6, 512), (7, 1024)]
N_FILTERS = 2048
PROJ_DIM = 512
N_CORES = 8
NTOK = B * S
TOK = NTOK // N_CORES        # 512 tokens per core
NPOS = 50
NPAIR = 25                   # char pairs per token
NPPAD = 28                   # padded pair positions (tap reach 55 -> pair 27)
NI = TOK * NPAIR             # 12800 gather indices per core
TABLE_ROWS = 32768           # fixed per-core unique-pair table allocation
KDIM = 112                   # 7 taps * 16 dims
TSPLIT = 22                  # conv positions < TSPLIT read xsA

S_W = 512.0                  # fp8 storage scale for highway/proj weights
S_H = 32.0                   # fp8 storage scale for highway/proj activations
DESCALE = 1.0 / (S_W * S_H)

# per 128-channel tile: valid positions; tile 0 packs w=1,2,3 with tails
CH_TILES = []
CH_TILES.append({"t_main": 48, "tails": [(0, 32, 50), (32, 64, 49), (64, 128, 48)]})
CH_TILES.append({"t_main": 47, "tails": [(0, 128, 47)]})      # w4
for _ in range(2):
    CH_TILES.append({"t_main": 46, "tails": [(0, 128, 46)]})  # w5
for _ in range(4):
    CH_TILES.append({"t_main": 45, "tails": [(0, 128, 45)]})  # w6
for _ in range(8):
    CH_TILES.append({"t_main": 44, "tails": [(0, 128, 44)]})  # w7

BF16 = mybir.dt.bfloat16
FP32 = mybir.dt.float32
FP8 = mybir.dt.float8e4

# drain schedule fractions: (pA, pPoolMerge, pPoolFinale) per phase
PH1 = (0.82, 0.0, 0.0)
PH2 = (0.75, 0.0, 0.0)

_MAX_WAITS_PER_INST = 1


def _patched_drain_and_barrier(self, tick_clock, wait_clock):
    # The walrus build in this container rejects CTRL instructions carrying
    # more than one sem wait; spread the kernel-tail drain waits over NOPs.
    nc = self.nc
    carrier = nc.sync.nop()
    wait_clock.add_sem_waits(carrier.ins, ScopedClock({None: tick_clock.global_clock}))
    si = carrier.ins.sync_info
    waits = list(si.on_wait) if si is not None and si.on_wait else []
    if len(waits) > _MAX_WAITS_PER_INST:
        carrier.ins.sync_info = mybir.SyncInfo(
            on_wait=waits[:_MAX_WAITS_PER_INST],
            on_update=list(si.on_update) if si.on_update else [])
        for i in range(_MAX_WAITS_PER_INST, len(waits), _MAX_WAITS_PER_INST):
            extra = nc.sync.nop()
            extra.ins.sync_info = mybir.SyncInfo(
                on_wait=waits[i:i + _MAX_WAITS_PER_INST], on_update=[])
    nc.sync.drain()
    nc.all_engine_barrier()
    assert self.sems is not None
    popped = nc._tile_sem_poison_stack.pop()
    assert popped is self._sem_poison
    nc.clear_and_free_semaphores(list(self.sems.allocated().values()))
    nc.all_engine_barrier()


tile.TileContext._drain_and_barrier = _patched_drain_and_barrier


class DrainSched:
    """Debt-based allocator: round kind (A/D), fold + merge engines."""

    def __init__(self, nc, pA, unused0=0.0, unused1=0.0):
        self.nc = nc
        self.pA = pA
        self.da = 0.0

    def kind(self):
        self.da += self.pA
        if self.da >= 1.0:
            self.da -= 1.0
            return "A"
        return "D"

    def note_forced(self, k):
        pass

    def fold_eng(self):
        return self.nc.vector


# ---------------------------------------------------------------- device IR
def build_module():
    nc = bacc.Bacc()
    SIdx = NI // 16

    # 256-byte rows (dma_gather granularity); cols 0:32 hold the pair embs
    table = nc.dram_tensor("table", [TABLE_ROWS, 128], BF16,
                           kind="ExternalInput")
    idx = nc.dram_tensor("idx", [128, SIdx], mybir.dt.int16, kind="ExternalInput")
    wconv = nc.dram_tensor("wconv", [KDIM, N_FILTERS], BF16, kind="ExternalInput")
    bconv = nc.dram_tensor("bconv", [128, 16], FP32, kind="ExternalInput")
    # highway weights fp8, host-packed per (layer, j):
    #   [l, j, p(128), cc(8), g(2), half*128+oc] ; g = DoubleRow group
    whw = nc.dram_tensor("whw", [2, 16, 128, 8, 2, 256], FP8, kind="ExternalInput")
    bhw = nc.dram_tensor("bhw", [2, 128, 16, 2], FP32, kind="ExternalInput")
    wproj = nc.dram_tensor("wproj", [128, 8, 2, 512], FP8, kind="ExternalInput")
    wprojc = nc.dram_tensor("wprojc", [128, 8, 2, 512], FP8, kind="ExternalInput")
    bproj = nc.dram_tensor("bproj", [128, 4], FP32, kind="ExternalInput")
    ident = nc.dram_tensor("ident", [128, 128], FP32, kind="ExternalInput")
    out = nc.dram_tensor("out", [TOK, PROJ_DIM], FP32, kind="ExternalOutput")

    with tile.TileContext(nc) as tc:
        with (
            tc.tile_pool(name="xs", bufs=1) as xspool,
            tc.tile_pool(name="consts", bufs=1) as cpool,
        ):
            # ---- constants in (only wconv before the gather stream; the
            # rest is deferred so it does not delay xsA on the DMA device)
            wconv_t = cpool.tile([KDIM, N_FILTERS], BF16)
            nc.sync.dma_start(out=wconv_t[:], in_=wconv[:])
            bconv_t = cpool.tile([128, 16], FP32)
            bhw_t = cpool.tile([128, 2, 16, 2], FP32)
            bproj_t = cpool.tile([128, 4], FP32)
            ident_t = cpool.tile([128, 128], FP32)
            wproj_t = cpool.tile([128, 8, 2, 512], FP8)
            wprojc_t = cpool.tile([128, 8, 2, 512], FP8)

            # ---- 1. pair-gather char embeddings + build K=112 patch matrix.
            # xsA/xsB viewed [128, m, parity, TOK] so the strided parity
            # interleave is a plain AP (no step slicing).
            NA = TSPLIT // 2          # 11 position pairs in xsA
            NB = (NPOS - TSPLIT) // 2  # 14 in xsB
            xsA = xspool.tile([KDIM, NA, 2, TOK], BF16, name="xsA")
            xsB = xspool.tile([KDIM, NB, 2, TOK], BF16, name="xsB")
            with tc.tile_pool(name="gather", bufs=1) as gpool:
                idx_t = gpool.tile([128, SIdx], mybir.dt.int16)
                nc.sync.dma_start(out=idx_t[:], in_=idx[:])
                xg = gpool.tile([128, 1, TOK * NPPAD], BF16)
                nc.vector.memset(xg[0:32, 0, NI:TOK * NPPAD], 0.0)
                # pair-aligned chunks (idx counts): 7,7,7,4 pairs
                chunks = [(0, 3584), (3584, 3584), (7168, 3584), (10752, 2048)]
                for r, (o, cn) in enumerate(chunks):
                    nc.gpsimd.dma_gather(
                        out_ap=xg[:, :, o:o + cn],
                        in_ap=table[:],
                        idxs_ap=idx_t[:, o // 16:(o + cn) // 16],
                        num_idxs=cn,
                        num_idxs_reg=cn,
                        elem_size=128,
                        transpose=True,
                        single_packet=False,
                    )
                    if r == 1:
                        # xsA copies, fused over (k, k+1) pairs where the
                        # source partition blocks are contiguous (halves the
                        # HWDGE descriptor-generation serial chain)
                        for s in range(2):
                            k = 0
                            while k < 7:
                                p0 = (s + k) // 2
                                par = (s + k) % 2
                                if par == 0 and k + 1 < 7:
                                    nc.sync.dma_start(
                                        out=xsA[16 * k:16 * (k + 2), :, s, :],
                                        in_=xg[0:32, 0,
                                               TOK * p0:TOK * (p0 + NA)],
                                    )
                                    k += 2
                                else:
                                    nc.sync.dma_start(
                                        out=xsA[16 * k:16 * (k + 1), :, s, :],
                                        in_=xg[16 * par:16 * par + 16, 0,
                                               TOK * p0:TOK * (p0 + NA)],
                                    )
                                    k += 1
                for s in range(2):
                    k = 0
                    while k < 7:
                        t0 = TSPLIT + s
                        p0 = (t0 + k) // 2
                        par = (t0 + k) % 2
                        if par == 0 and k + 1 < 7:
                            nc.sync.dma_start(
                                out=xsB[16 * k:16 * (k + 2), :, s, :],
                                in_=xg[0:32, 0, TOK * p0:TOK * (p0 + NB)],
                            )
                            k += 2
                        else:
                            nc.sync.dma_start(
                                out=xsB[16 * k:16 * (k + 1), :, s, :],
                                in_=xg[16 * par:16 * par + 16, 0,
                                       TOK * p0:TOK * (p0 + NB)],
                            )
                            k += 1
            nc.sync.dma_start(out=bconv_t[:], in_=bconv[:])
            nc.sync.dma_start(out=bhw_t[:], in_=bhw[:].rearrange("l p j h -> p l j h"))
            nc.sync.dma_start(out=bproj_t[:], in_=bproj[:])
            nc.sync.dma_start(out=ident_t[:], in_=ident[:])

            HALF = TOK // 2

            def conv_rhs(t, hlo):
                if t < TSPLIT:
                    return xsA[:, t // 2, t % 2, hlo:hlo + HALF]
                tl = t - TSPLIT
                return xsB[:, tl // 2, tl % 2, hlo:hlo + HALF]

            stack = tc.tile_pool(name="hbuf", bufs=1)
            hpool = stack.__enter__()
            stack2 = tc.tile_pool(name="h8buf", bufs=1)
            h8pool = stack2.__enter__()
            stack3 = tc.tile_pool(name="wstream", bufs=3)
            wpool = stack3.__enter__()
            stack4 = tc.tile_pool(name="small", bufs=2)
            spool = stack4.__enter__()
            stack6 = tc.tile_pool(name="accpool", bufs=6)
            accpool = stack6.__enter__()
            convp = None  # phase-2/3 PSUM pool, opened after phase 1

            # per-half persistent tensors (separate tiles avoid false deps)
            h1 = [hpool.tile([128, 16, HALF], BF16, tag=f"h1{s_}", name=f"h1{s_}")
                  for s_ in range(2)]
            hmid = [hpool.tile([128, 16, HALF], BF16, tag=f"hm{s_}", name=f"hm{s_}")
                    for s_ in range(2)]
            # h8 layout: [128, hi/lo, j, n] so (j, j+1) casts batch into one op
            h8c = [h8pool.tile([128, 2, 16, HALF], FP8, tag=f"h8c{s_}", name=f"h8c{s_}")
                   for s_ in range(2)]
            h8m = [h8pool.tile([128, 2, 16, HALF], FP8, tag=f"h8m{s_}", name=f"h8m{s_}")
                   for s_ in range(2)]
            # h8f aliases h8c: layer-0 chains fully consume h8c before
            # layer-1 writes the final activations (WAR handled by deps)
            h8f = h8c

            def cast_pair(h_bf, j0, nj, h8, lo_eng=None):
                # hi = fp8(h * S_H) on ACT; lo = fp8(h*S_H - hi) on DVE/Pool
                nc.scalar.activation(
                    out=h8[:, 0, j0:j0 + nj, :], in_=h_bf[:, j0:j0 + nj, :],
                    func=mybir.ActivationFunctionType.Copy, scale=S_H)
                (lo_eng or nc.vector).scalar_tensor_tensor(
                    out=h8[:, 1, j0:j0 + nj, :], in0=h_bf[:, j0:j0 + nj, :],
                    scalar=S_H, in1=h8[:, 0, j0:j0 + nj, :],
                    op0=mybir.AluOpType.mult, op1=mybir.AluOpType.subtract)

            def conv_half(hf, rpos, pool, sched):
                """Generator: conv + max-pool drain for token half hf.

                Touches (PSUM reads) emit immediately; folds/merges/finales
                emit one round late via `pending` so no engine head-of-line
                blocks on a cross-engine dependency that is not ready yet.
                """
                from collections import deque
                hlo = HALF * hf
                hw_ = rpos // 2  # acc width
                pending = deque()
                pcast = deque()   # fp8 casts lag one extra round so the ACT
                                  # hi-cast never parks waiting the DVE finale

                def flush(keep):
                    while len(pending) > keep:
                        pending.popleft()()
                    while len(pcast) > max(keep, 1) + 1:
                        pcast.popleft()()

                DX = 3  # spare direct-write slots for D-round reduces
                for i, spec in enumerate(CH_TILES):
                    lhsT = wconv_t[:, 128 * i:128 * (i + 1)]
                    t_main = spec["t_main"]
                    acc = accpool.tile([128, hw_ + DX, HALF], BF16, tag="acc")
                    first = True
                    dstate = {"next": hw_}
                    t0 = 0
                    while t0 < t_main:
                        nt = min(rpos, t_main - t0)
                        P = pool.tile([128, rpos, HALF], FP32, tag=f"ps{rpos}")
                        for r in range(nt):
                            nc.tensor.matmul(
                                out=P[:, r, :], lhsT=lhsT,
                                rhs=conv_rhs(t0 + r, hlo),
                                start=True, stop=True)
                        flush(1)
                        if nt == rpos and (first or sched.kind() == "A"):
                            if first:
                                sched.note_forced("A")
                            tmp = spool.tile([128, rpos, HALF], BF16,
                                             tag="astk", bufs=4)
                            nc.scalar.activation(
                                out=tmp[:], in_=P[:],
                                func=mybir.ActivationFunctionType.Copy, scale=1.0)
                            eng = sched.fold_eng()
                            if first:
                                def op(eng=eng, tmp=tmp, acc=acc):
                                    eng.tensor_tensor(
                                        out=acc[:, 0:hw_, :],
                                        in0=tmp[:, 0:hw_, :],
                                        in1=tmp[:, hw_:rpos, :],
                                        op=mybir.AluOpType.max)
                                first = False
                            else:
                                def op(eng=eng, tmp=tmp, acc=acc):
                                    fh = spool.tile([128, hw_, HALF], BF16,
                                                    tag="fh", bufs=3)
                                    eng.tensor_tensor(
                                        out=fh[:], in0=tmp[:, 0:hw_, :],
                                        in1=tmp[:, hw_:rpos, :],
                                        op=mybir.AluOpType.max)
                                    eng.tensor_tensor(
                                        out=acc[:, 0:hw_, :],
                                        in0=acc[:, 0:hw_, :], in1=fh[:],
                                        op=mybir.AluOpType.max)
                            pending.append(op)
                        elif nt == 1:
                            nc.vector.tensor_tensor(
                                out=acc[:, 0, :], in0=acc[:, 0, :],
                                in1=P[:, 0, :], op=mybir.AluOpType.max)
                        elif dstate["next"] < hw_ + DX:
                            # D-led: reduce straight into a spare acc slot
                            nc.vector.reduce_max(
                                out=acc[:, dstate["next"], :],
                                in_=P[:, 0:nt, :].rearrange("p t n -> p n t"),
                                axis=mybir.AxisListType.X)
                            dstate["next"] += 1
                        else:
                            part = spool.tile([128, HALF], BF16, tag="part",
                                              bufs=3)
                            nc.vector.reduce_max(
                                out=part[:],
                                in_=P[:, 0:nt, :].rearrange("p t n -> p n t"),
                                axis=mybir.AxisListType.X)

                            def op(part=part, acc=acc):
                                nc.vector.tensor_tensor(
                                    out=acc[:, 0, :], in0=acc[:, 0, :],
                                    in1=part[:], op=mybir.AluOpType.max)
                            pending.append(op)
                        t0 += nt
                    # ragged tails (tile 0): positions t_main..50 on partition
                    # subranges; reduces touch PSUM now, merges deferred
                    if spec["tails"][0][2] > t_main:
                        nt = spec["tails"][0][2] - t_main
                        P = pool.tile([128, rpos, HALF], FP32, tag=f"ps{rpos}")
                        for r in range(nt):
                            nc.tensor.matmul(
                                out=P[:, r, :], lhsT=lhsT,
                                rhs=conv_rhs(t_main + r, hlo),
                                start=True, stop=True)
                        for (lo, hi, g_cnt) in spec["tails"]:
                            g_nt = g_cnt - t_main
                            if g_nt <= 0:
                                continue
                            if g_nt == 1:
                                nc.vector.tensor_tensor(
                                    out=acc[lo:hi, 0, :], in0=acc[lo:hi, 0, :],
                                    in1=P[lo:hi, 0, :], op=mybir.AluOpType.max)
                            else:
                                part = spool.tile([128, HALF], BF16, tag="part",
                                                  bufs=3)
                                nc.vector.reduce_max(
                                    out=part[lo:hi, :],
                                    in_=P[lo:hi, 0:g_nt, :].rearrange(
                                        "p t n -> p n t"),
                                    axis=mybir.AxisListType.X)

                                def op(part=part, acc=acc, lo=lo, hi=hi):
                                    nc.vector.tensor_tensor(
                                        out=acc[lo:hi, 1, :],
                                        in0=acc[lo:hi, 1, :],
                                        in1=part[lo:hi, :],
                                        op=mybir.AluOpType.max)
                                pending.append(op)

                    used = dstate["next"]

                    def finale(i=i, acc=acc, used=used):
                        pre = spool.tile([128, HALF], BF16, tag="pre")
                        cw = used
                        while cw > 2:
                            if cw % 2:
                                nc.vector.tensor_tensor(
                                    out=acc[:, 0, :], in0=acc[:, 0, :],
                                    in1=acc[:, cw - 1, :],
                                    op=mybir.AluOpType.max)
                                cw -= 1
                            h = cw // 2
                            nc.vector.tensor_tensor(
                                out=acc[:, 0:h, :], in0=acc[:, 0:h, :],
                                in1=acc[:, h:cw, :], op=mybir.AluOpType.max)
                            cw = h
                        nc.vector.tensor_tensor(
                            out=pre[:], in0=acc[:, 0, :], in1=acc[:, 1, :],
                            op=mybir.AluOpType.max)
                        nc.vector.tensor_scalar(
                            out=h1[hf][:, i, :], in0=pre[:],
                            scalar1=bconv_t[:, i:i + 1], scalar2=0.0,
                            op0=mybir.AluOpType.add, op1=mybir.AluOpType.max)
                    pending.append(finale)
                    if i % 2 == 1:
                        def cst(i=i):
                            cast_pair(h1[hf], i - 1, 2, h8c[hf])
                        pcast.append(cst)
                    yield
                flush(0)
                while pcast:
                    pcast.popleft()()

            def hw_mm_chain(p_out, wslab, h8, ofs):
                # W8 x (h_hi + h_lo); Wl correction skipped for the highway
                for hl in range(2):
                    for cc in range(8):
                        nc.tensor.matmul(
                            out=p_out, lhsT=wslab[:, cc, :, ofs:ofs + 128],
                            rhs=h8[:, hl, 2 * cc:2 * cc + 2, :],
                            start=(hl == 0 and cc == 0), stop=(hl == 1 and cc == 7),
                            perf_mode=mybir.MatmulPerfMode.DoubleRow)

            def hw_mm_chain_proj(p_out, h8, ofs):
                for hl in range(2):
                    for cc in range(8):
                        nc.tensor.matmul(
                            out=p_out, lhsT=wproj_t[:, cc, :, ofs:ofs + 128],
                            rhs=h8[:, hl, 2 * cc:2 * cc + 2, :],
                            start=(hl == 0 and cc == 0), stop=False,
                            perf_mode=mybir.MatmulPerfMode.DoubleRow)
                for cc in range(8):
                    nc.tensor.matmul(
                        out=p_out, lhsT=wprojc_t[:, cc, :, ofs:ofs + 128],
                        rhs=h8[:, 0, 2 * cc:2 * cc + 2, :],
                        start=False, stop=(cc == 7),
                        perf_mode=mybir.MatmulPerfMode.DoubleRow)

            def hw_half(hf):
                """Generator: highway l0+l1 + proj for token half hf.

                PE chains emit immediately; ACT/DVE epilogues lag one j so
                neither engine parks at its queue head waiting on a chain.
                """
                from collections import deque
                pending = deque()

                def flush(keep):
                    while len(pending) > keep:
                        pending.popleft()()

                state = {}
                for layer in range(2):
                    h_in = h1[hf] if layer == 0 else hmid[hf]
                    h8_in = h8c[hf] if layer == 0 else h8m[hf]
                    h8_out = h8m[hf] if layer == 0 else h8f[hf]
                    for j in range(16):
                        wslab = wpool.tile([128, 8, 2, 256], FP8, tag="wslab")
                        nc.sync.dma_start(out=wslab[:], in_=whw[layer, j])
                        hp = convp.tile([128, 2, HALF], FP32, tag="hwps",
                                        name="hp", bufs=2)
                        p_nl = hp[:, 0, :]
                        p_g = hp[:, 1, :]
                        hw_mm_chain(p_nl, wslab, h8_in, 0)
                        hw_mm_chain(p_g, wslab, h8_in, 128)
                        flush(1)

                        def epi(layer=layer, j=j, p_nl=p_nl, p_g=p_g,
                                h_in=h_in, h8_out=h8_out):
                            nl = spool.tile([128, HALF], BF16, tag="nl")
                            gt = spool.tile([128, HALF], BF16, tag="gt")
                            nc.scalar.activation(
                                out=nl[:], in_=p_nl,
                                func=mybir.ActivationFunctionType.Relu,
                                bias=bhw_t[:, layer, j, 0:1], scale=DESCALE)
                            nc.scalar.activation(
                                out=gt[:], in_=p_g,
                                func=mybir.ActivationFunctionType.Sigmoid,
                                bias=bhw_t[:, layer, j, 1:2], scale=DESCALE)
                            d = spool.tile([128, HALF], BF16, tag="d")
                            nc.vector.tensor_tensor(
                                out=d[:], in0=h_in[:, j, :], in1=nl[:],
                                op=mybir.AluOpType.subtract)
                            m = spool.tile([128, HALF], BF16, tag="m")
                            nc.vector.tensor_mul(out=m[:], in0=gt[:], in1=d[:])
                            if layer == 0:
                                nc.vector.tensor_add(
                                    out=hmid[hf][:, j, :], in0=nl[:], in1=m[:])
                                if j % 2 == 1:
                                    cast_pair(hmid[hf], j - 1, 2, h8_out)
                            else:
                                if j % 2 == 0:
                                    state["htp"] = spool.tile(
                                        [128, 2, HALF], BF16, tag="htp",
                                        name="htp")
                                htp = state["htp"]
                                nc.vector.tensor_add(
                                    out=htp[:, j % 2, :], in0=nl[:], in1=m[:])
                                if j % 2 == 1:
                                    nc.scalar.activation(
                                        out=h8_out[:, 0, j - 1:j + 1, :],
                                        in_=htp[:],
                                        func=mybir.ActivationFunctionType.Copy,
                                        scale=S_H)
                                    nc.vector.scalar_tensor_tensor(
                                        out=h8_out[:, 1, j - 1:j + 1, :],
                                        in0=htp[:], scalar=S_H,
                                        in1=h8_out[:, 0, j - 1:j + 1, :],
                                        op0=mybir.AluOpType.mult,
                                        op1=mybir.AluOpType.subtract)
                        pending.append(epi)
                        yield
                    # layer barrier: next layer's chains read every h8 column
                    flush(0)
                # projection + transpose + out for this half
                hlo = HALF * hf
                for j2 in range(4):
                    hp = convp.tile([128, 2, HALF], FP32, tag="hwps",
                                    name="hp", bufs=2)
                    p_o = hp[:, 0, :]
                    hw_mm_chain_proj(p_o, h8f[hf], 128 * j2)
                    flush(1)

                    def proj_epi(j2=j2, hp=hp, p_o=p_o):
                        ot = spool.tile([128, HALF], FP32, tag="ot")
                        nc.scalar.activation(
                            out=ot[:], in_=p_o,
                            func=mybir.ActivationFunctionType.Identity,
                            bias=bproj_t[:, j2:j2 + 1], scale=DESCALE)
                        for m4 in range(2):
                            p_t = hp[:, 1, 128 * m4:128 * (m4 + 1)]
                            nc.tensor.transpose(
                                out=p_t, in_=ot[:, 128 * m4:128 * (m4 + 1)],
                                identity=ident_t[:])
                            ob = spool.tile([128, 128], FP32, tag="ob")
                            nc.scalar.copy(out=ob[:], in_=p_t)
                            row0 = hlo + 128 * m4
                            nc.sync.dma_start(
                                out=out[row0:row0 + 128,
                                        128 * j2:128 * (j2 + 1)],
                                in_=ob[:])
                    pending.append(proj_epi)
                    yield
                flush(0)

            # ---- phase 1: conv half A, 8-position rounds, all 8 PSUM banks
            sched1 = DrainSched(nc, *PH1)
            with tc.tile_pool(name="convp8", bufs=2, space="PSUM") as p8pool:
                for _ in conv_half(0, 8, p8pool, sched1):
                    pass
            stack5 = tc.tile_pool(name="convp", bufs=2, space="PSUM")
            convp = stack5.__enter__()
            # WAW-gate the projection-weight loads behind a DVE op that sits
            # late in DVE program order, so they cannot steal the DMA device
            # from the gather at t=0 (the sim schedules by readiness)
            nc.vector.memset(wproj_t[0:1, 0:1, 0:1, 0:1], 0.0)
            nc.vector.memset(wprojc_t[0:1, 0:1, 0:1, 0:1], 0.0)
            nc.sync.dma_start(out=wproj_t[:], in_=wproj[:])
            nc.sync.dma_start(out=wprojc_t[:], in_=wprojc[:])
            # ---- phase 2: conv half B interleaved with highway+proj half A
            sched2 = DrainSched(nc, *PH2)
            genB = conv_half(1, 6, convp, sched2)
            genA = hw_half(0)
            unitsB, unitsA = 16, 36
            credit = 0.0
            doneB = doneA = False
            while not (doneB and doneA):
                credit += unitsA / unitsB
                if not doneB:
                    doneB = next(genB, "end") == "end"
                while credit >= 1.0 and not doneA:
                    doneA = next(genA, "end") == "end"
                    credit -= 1.0
                if doneB:
                    while not doneA:
                        doneA = next(genA, "end") == "end"
            # ---- phase 3: highway+proj half B
            for _ in hw_half(1):
                pass

            for st in (stack5, stack6, stack4, stack3, stack2, stack):
                st.__exit__(None, None, None)

    nc.compile()
    return nc


_CACHED = {}


def _prep(inputs):
    """Host-side layout prep: sharding, pair tables, weight packing."""
    chars = np.asarray(inputs["chars"]).astype(np.int64).reshape(NTOK, L)
    pairs = chars[:, 0::2] * CHAR_VOCAB + chars[:, 1::2]   # [NTOK, 25]

    emb = np.asarray(inputs["char_emb"], np.float32)

    wc = np.zeros((7, CHAR_DIM, N_FILTERS), np.float32)
    off = 0
    for fi, (w, n) in enumerate(FILTERS):
        cw = np.asarray(inputs[f"conv_w_{fi}"], np.float32)
        wc[:w, :, off:off + n] = cw.transpose(2, 1, 0)
        off += n
    wconv = wc.reshape(KDIM, N_FILTERS).astype(ml_dtypes.bfloat16)
    bconv = np.concatenate([np.asarray(inputs[f"conv_b_{i}"], np.float32)
                            for i in range(7)])
    bconv_dev = bconv.reshape(16, 128).T.copy()

    # highway weights: fp8 W8 packed for DoubleRow streaming.
    whw8 = np.zeros((2, 16, 128, 8, 2, 256), np.float32)
    bhw = np.zeros((2, 128, 16, 2), np.float32)
    for l in range(2):
        W = np.asarray(inputs[f"hw_w_{l}"], np.float32)   # (4096, 2048)
        bb = np.asarray(inputs[f"hw_b_{l}"], np.float32)
        Ws = W * S_W
        W8 = Ws.astype(ml_dtypes.float8_e4m3).astype(np.float32)
        W8T = W8.T  # (2048 ic, 4096 oc)
        for j in range(16):
            for hf in range(2):
                oc0 = 2048 * hf + 128 * j
                for cc in range(8):
                    for g in range(2):
                        cb = 2 * cc + g
                        blk8 = W8T[128 * cb:128 * (cb + 1), oc0:oc0 + 128]
                        whw8[l, j, :, cc, g, 128 * hf:128 * hf + 128] = blk8
            bhw[l, :, j, 0] = bb[128 * j:128 * (j + 1)]
            bhw[l, :, j, 1] = bb[2048 + 128 * j:2048 + 128 * (j + 1)]
    whw8 = whw8.astype(ml_dtypes.float8_e4m3)

    Wp = np.asarray(inputs["proj_w"], np.float32) * S_W  # (512, 2048)
    Wp8 = Wp.astype(ml_dtypes.float8_e4m3).astype(np.float32)
    Wpl = (Wp - Wp8).astype(ml_dtypes.float8_e4m3).astype(np.float32)
    Wp8T = Wp8.T  # (2048, 512)
    WplT = Wpl.T
    wproj8 = np.zeros((128, 8, 2, 512), np.float32)
    wprojc8 = np.zeros((128, 8, 2, 512), np.float32)
    for cc in range(8):
        for g in range(2):
            cb = 2 * cc + g
            wproj8[:, cc, g, :] = Wp8T[128 * cb:128 * (cb + 1), :]
            wprojc8[:, cc, g, :] = WplT[128 * cb:128 * (cb + 1), :]
    wproj8 = wproj8.astype(ml_dtypes.float8_e4m3)
    wprojc8 = wprojc8.astype(ml_dtypes.float8_e4m3)
    bproj = np.zeros((128, 4), np.float32)
    bp = np.asarray(inputs["proj_b"], np.float32)
    for j2 in range(4):
        bproj[:, j2] = bp[128 * j2:128 * (j2 + 1)]

    ident = np.eye(128, dtype=np.float32)

    shared = dict(wconv=wconv, bconv=bconv_dev, whw=whw8,
                  bhw=bhw, wproj=wproj8, wprojc=wprojc8,
                  bproj=bproj, ident=ident)

    in_maps = []
    for core in range(N_CORES):
        cp = pairs[core * TOK:(core + 1) * TOK]            # [512, 25]
        uniq, inv = np.unique(cp, return_inverse=True)
        assert len(uniq) <= TABLE_ROWS, len(uniq)
        tbl = np.zeros((TABLE_ROWS, 128), np.float32)
        tbl[:len(uniq), 0:CHAR_DIM] = emb[uniq // CHAR_VOCAB]
        tbl[:len(uniq), CHAR_DIM:2 * CHAR_DIM] = emb[uniq % CHAR_VOCAB]
        idx_flat = inv.reshape(TOK, NPAIR).T.reshape(-1).astype(np.int16)
        idx16 = idx_flat.reshape(NI // 16, 16).T.copy()
        idx16 = np.tile(idx16, (8, 1))
        m = dict(shared)
        m["table"] = tbl.astype(ml_dtypes.bfloat16)
        m["idx"] = idx16
        in_maps.append(m)
    return in_maps


def kernel(**inputs) -> np.ndarray:
    if "nc" not in _CACHED:
        _CACHED["nc"] = build_module()
    nc = _CACHED["nc"]
    in_maps = _prep(inputs)
    res = run_bass_kernel_spmd(nc, in_maps, core_ids=list(range(N_CORES)))
    full = np.concatenate([r["out"] for r in res.results], axis=0)
    return full.reshape(B, S, PROJ_DIM)


# revision 31
# speedup vs baseline: 1.0193x; 1.0191x over previous
"""CharCNN token embedder (ELMo-style) on 8 Trainium2 NeuronCores.

Data-parallel over the 4096 = 16*256 tokens (512 per core). Weights replicated.

Per-core pipeline (v5):
  1. Char-PAIR gather: host packs each token's 50 chars into 25 pairs and
     builds a per-core table of unique pairs (~12k rows of 256B; cols 0:32
     hold both chars' embeddings) -> 12800 descriptors instead of 28672.
     Four pair-aligned gather chunks; shifted parity-strided SBUF copies
     (fused over contiguous tap pairs) build the K=112 im2col patch matrix;
     conv starts after two chunks.
  2. Tokens processed in two halves of 256: phase1 = conv(A); phase2 =
     conv(B) interleaved with highway+proj(A); phase3 = highway+proj(B).
  3. Conv = bf16 matmuls, K=112, one position per matmul, rounds of 8
     (phase1) / 6 (phase2) positions in double-buffered PSUM. Max-pool
     drain per tile into acc[128, rpos//2 + 3, HALF] bf16:
       ~80% A-led rounds: ACT copies the PSUM round to a tmp stack; DVE
         folds it to half width and merges into acc (emitted one round
         late via a deferred-op queue so no in-order engine parks on a
         cross-engine dependency).
       ~20% D-led rounds: DVE reduce_max writes a spare acc slot directly
         (no merge op). Partial rounds are always D-led.
     Finale: in-place pairwise fold of acc + bias+relu on DVE; fp8 hi/lo
     casts batched per tile pair and deferred two rounds (ACT never waits
     the DVE finale chain). Only ACT and DVE can touch PSUM and the Pool
     engine cannot run TensorTensor on TRN2, so Pool only runs the gather.
  4. Highway layers in fp8 DoubleRow at 2x bf16 throughput: per (layer, j),
     psum = W8(h_hi) + W8(h_lo), scaled e4m3 (S_W=512, S_H=32), descale via
     the ACT sigmoid/relu scale; gating on DVE bf16. PE chains emit
     immediately, ACT/DVE epilogues lag one j. h8 layout [128, hi/lo, j, n]
     batches (j, j+1) casts; the final-layer fp8 buffer aliases the conv
     one (layer-0 chains consume it before layer-1 writes).
  5. Projection fp8 DoubleRow (hi, lo, W-residual chains); PE-transpose;
     ACT bounce to SBUF; DMA out. Projection weight loads are WAW-gated
     behind a late DVE memset so they cannot steal the DMA device from the
     gather at t=0.
"""

import numpy as np
import ml_dtypes

import concourse.bass as bass
import concourse.mybir as mybir
import concourse.tile as tile
from concourse import bacc
from concourse.bass_utils import run_bass_kernel_spmd
from concourse.vector_clock import ScopedClock

# ---------------------------------------------------------------- constants
B, S, L = 16, 256, 50
CHAR_DIM = 16
CHAR_VOCAB = 262
FILTERS = [(1, 32), (2, 32), (3, 64), (4, 128), (5, 256), (6, 512), (7, 1024)]
N_FILTERS = 2048
PROJ_DIM = 512
N_CORES = 8
NTOK = B * S
TOK = NTOK // N_CORES        # 512 tokens per core
NPOS = 50
NPAIR = 25                   # char pairs per token
NPPAD = 28                   # padded pair positions (tap reach 55 -> pair 27)
NI = TOK * NPAIR             # 12800 gather indices per core
TABLE_ROWS = 32768           # fixed per-core unique-pair table allocation
KDIM = 112                   # 7 taps * 16 dims
TSPLIT = 22                  # conv positions < TSPLIT read xsA

S_W = 512.0                  # fp8 storage scale for highway/proj weights
S_H = 32.0                   # fp8 storage scale for highway/proj activations
DESCALE = 1.0 / (S_W * S_H)

# per 128-channel tile: valid positions; tile 0 packs w=1,2,3 with tails
CH_TILES = []
CH_TILES.append({"t_main": 48, "tails": [(0, 32, 50), (32, 64, 49), (64, 128, 48)]})
CH_TILES.append({"t_main": 47, "tails": [(0, 128, 47)]})      # w4
for _ in range(2):
    CH_TILES.append({"t_main": 46, "tails": [(0, 128, 46)]})  # w5
for _ in range(4):
    CH_TILES.append({"t_main": 45, "tails": [(0, 128, 45)]})  # w6
for _ in range(8):
    CH_TILES.append({"t_main": 44, "tails": [(0, 128, 44)]})  # w7

BF16 = mybir.dt.bfloat16
FP32 = mybir.dt.float32
FP8 = mybir.dt.float8e4

# drain schedule fractions: (pA, pPoolMerge, pPoolFinale) per phase
PH1 = (0.82, 0.0, 0.0)
PH2 = (0.75, 0.0, 0.0)

_MAX_WAITS_PER_INST = 1


def _patched_drain_and_barrier(self, tick_clock, wait_clock):
    # The walrus build in this container rejects CTRL instructions carrying
    # more than one sem wait; spread the kernel-tail drain waits over NOPs.
    nc = self.nc
    carrier = nc.sync.nop()
    wait_clock.add_sem_waits(carrier.ins, ScopedClock({None: tick_clock.global_clock}))
    si = carrier.ins.sync_info
    waits = list(si.on_wait) if si is not None and si.on_wait else []
    if len(waits) > _MAX_WAITS_PER_INST:
        carrier.ins.sync_info = mybir.SyncInfo(
            on_wait=waits[:_MAX_WAITS_PER_INST],
            on_update=list(si.on_update) if si.on_update else [])
        for i in range(_MAX_WAITS_PER_INST, len(waits), _MAX_WAITS_PER_INST):
            extra = nc.sync.nop()
            extra.ins.sync_info = mybir.SyncInfo(
                on_wait=waits[i:i + _MAX_WAITS_PER_INST], on_update=[])
    nc.sync.drain()
    nc.all_engine_barrier()
    assert self.sems is not None
    popped = nc._tile_sem_poison_stack.pop()
    assert popped is self._sem_poison
    nc.clear_and_free_semaphores(list(self.sems.allocated().values()))
    nc.all_engine_barrier()


tile.TileContext._drain_and_barrier = _patched_drain_and_barrier


class DrainSched:
    """Debt-based allocator: round kind (A/D), fold + merge engines."""

    def __init__(self, nc, pA, unused0=0.0, unused1=0.0):
        self.nc = nc
        self.pA = pA
        self.da = 0.0

    def kind(self):
        self.da += self.pA
        if self.da >= 1.0:
            self.da -= 1.0
            return "A"
        return "D"

    def note_forced(self, k):
        pass

    def fold_eng(self):
        return self.nc.vector


# ---------------------------------------------------------------- device IR
def build_module():
    nc = bacc.Bacc()
    SIdx = NI // 16

    # 256-byte rows (dma_gather granularity); cols 0:32 hold the pair embs
    table = nc.dram_tensor("table", [TABLE_ROWS, 128], BF16,
                           kind="ExternalInput")
    idx = nc.dram_tensor("idx", [128, SIdx], mybir.dt.int16, kind="ExternalInput")
    wconv = nc.dram_tensor("wconv", [KDIM, N_FILTERS], BF16, kind="ExternalInput")
    bconv = nc.dram_tensor("bconv", [128, 16], FP32, kind="ExternalInput")
    # highway weights fp8, host-packed per (layer, j):
    #   [l, j, p(128), cc(8), g(2), half*128+oc] ; g = DoubleRow group
    whw = nc.dram_tensor("whw", [2, 16, 128, 8, 2, 256], FP8, kind="ExternalInput")
    bhw = nc.dram_tensor("bhw", [2, 128, 16, 2], FP32, kind="ExternalInput")
    wproj = nc.dram_tensor("wproj", [128, 8, 2, 512], FP8, kind="ExternalInput")
    wprojc = nc.dram_tensor("wprojc", [128, 8, 2, 512], FP8, kind="ExternalInput")
    bproj = nc.dram_tensor("bproj", [128, 4], FP32, kind="ExternalInput")
    ident = nc.dram_tensor("ident", [128, 128], FP32, kind="ExternalInput")
    out = nc.dram_tensor("out", [TOK, PROJ_DIM], FP32, kind="ExternalOutput")

    with tile.TileContext(nc) as tc:
        with (
            tc.tile_pool(name="xs", bufs=1) as xspool,
            tc.tile_pool(name="consts", bufs=1) as cpool,
        ):
            # ---- constants in (wconv loads after idx; the rest is deferred
            # so nothing delays the gather + xsA stream on the DMA device)
            wconv_t = cpool.tile([KDIM, N_FILTERS], BF16)
            bconv_t = cpool.tile([128, 16], FP32)
            bhw_t = cpool.tile([128, 2, 16, 2], FP32)
            bproj_t = cpool.tile([128, 4], FP32)
            ident_t = cpool.tile([128, 128], FP32)

            # ---- 1. pair-gather char embeddings + build K=112 patch matrix.
            # xsA/xsB viewed [128, m, parity, TOK] so the strided parity
            # interleave is a plain AP (no step slicing).
            NA = TSPLIT // 2          # 11 position pairs in xsA
            NB = (NPOS - TSPLIT) // 2  # 14 in xsB
            xsA = xspool.tile([KDIM, NA, 2, TOK], BF16, name="xsA")
            xsB = xspool.tile([KDIM, NB, 2, TOK], BF16, name="xsB")
            with tc.tile_pool(name="gather", bufs=1) as gpool:
                idx_t = gpool.tile([128, SIdx], mybir.dt.int16)
                nc.sync.dma_start(out=idx_t[:], in_=idx[:])
                nc.sync.dma_start(out=wconv_t[:], in_=wconv[:])
                xg = gpool.tile([128, 1, TOK * NPPAD], BF16)
                nc.vector.memset(xg[0:32, 0, NI:TOK * NPPAD], 0.0)
                # pair-aligned chunks (idx counts): 7,7,7,4 pairs
                chunks = [(0, 3584), (3584, 3584), (7168, 3584), (10752, 2048)]
                for r, (o, cn) in enumerate(chunks):
                    nc.gpsimd.dma_gather(
                        out_ap=xg[:, :, o:o + cn],
                        in_ap=table[:],
                        idxs_ap=idx_t[:, o // 16:(o + cn) // 16],
                        num_idxs=cn,
                        num_idxs_reg=cn,
                        elem_size=128,
                        transpose=True,
                        single_packet=False,
                    )
                    if r == 1:
                        # xsA copies, fused over (k, k+1) pairs where the
                        # source partition blocks are contiguous (halves the
                        # HWDGE descriptor-generation serial chain)
                        for s in range(2):
                            k = 0
                            while k < 7:
                                p0 = (s + k) // 2
                                par = (s + k) % 2
                                if par == 0 and k + 1 < 7:
                                    nc.sync.dma_start(
                                        out=xsA[16 * k:16 * (k + 2), :, s, :],
                                        in_=xg[0:32, 0,
                                               TOK * p0:TOK * (p0 + NA)],
                                    )
                                    k += 2
                                else:
                                    nc.sync.dma_start(
                                        out=xsA[16 * k:16 * (k + 1), :, s, :],
                                        in_=xg[16 * par:16 * par + 16, 0,
                                               TOK * p0:TOK * (p0 + NA)],
                                    )
                                    k += 1
                for s in range(2):
                    k = 0
                    while k < 7:
                        t0 = TSPLIT + s
                        p0 = (t0 + k) // 2
                        par = (t0 + k) % 2
                        if par == 0 and k + 1 < 7:
                            nc.sync.dma_start(
                                out=xsB[16 * k:16 * (k + 2), :, s, :],
                                in_=xg[0:32, 0, TOK * p0:TOK * (p0 + NB)],
                            )
                            k += 2
                        else:
                            nc.sync.dma_start(
                                out=xsB[16 * k:16 * (k + 1), :, s, :],
                                in_=xg[16 * par:16 * par + 16, 0,
                                       TOK * p0:TOK * (p0 + NB)],
                            )
                            k += 1
            nc.sync.dma_start(out=bconv_t[:], in_=bconv[:])
            nc.sync.dma_start(out=bhw_t[:], in_=bhw[:].rearrange("l p j h -> p l j h"))
            nc.sync.dma_start(out=bproj_t[:], in_=bproj[:])
            nc.sync.dma_start(out=ident_t[:], in_=ident[:])
            stackw = tc.tile_pool(name="wppool", bufs=1)
            wppool = stackw.__enter__()
            wproj_t = wppool.tile([128, 8, 2, 512], FP8)
            wprojc_t = wppool.tile([128, 8, 2, 512], FP8)

            HALF = TOK // 2

            def conv_rhs(t, hlo):
                if t < TSPLIT:
                    return xsA[:, t // 2, t % 2, hlo:hlo + HALF]
                tl = t - TSPLIT
                return xsB[:, tl // 2, tl % 2, hlo:hlo + HALF]

            stack = tc.tile_pool(name="hbuf", bufs=1)
            hpool = stack.__enter__()
            stack2 = tc.tile_pool(name="h8buf", bufs=1)
            h8pool = stack2.__enter__()
            stack3 = tc.tile_pool(name="wstream", bufs=3)
            wpool = stack3.__enter__()
            stack4 = tc.tile_pool(name="small", bufs=2)
            spool = stack4.__enter__()
            stack6 = tc.tile_pool(name="accpool", bufs=6)
            accpool = stack6.__enter__()
            convp = None  # phase-2/3 PSUM pool, opened after phase 1

            # per-half persistent tensors (separate tiles avoid false deps)
            h1 = [hpool.tile([128, 16, HALF], BF16, tag=f"h1{s_}", name=f"h1{s_}")
                  for s_ in range(2)]
            hmid = [hpool.tile([128, 16, HALF], BF16, tag=f"hm{s_}", name=f"hm{s_}")
                    for s_ in range(2)]
            # h8 layout: [128, hi/lo, j, n] so (j, j+1) casts batch into one op
            h8c = [h8pool.tile([128, 2, 16, HALF], FP8, tag=f"h8c{s_}", name=f"h8c{s_}")
                   for s_ in range(2)]
            h8m = [h8pool.tile([128, 2, 16, HALF], FP8, tag=f"h8m{s_}", name=f"h8m{s_}")
                   for s_ in range(2)]
            # h8f aliases h8c: layer-0 chains fully consume h8c before
            # layer-1 writes the final activations (WAR handled by deps)
            h8f = h8c

            def cast_pair(h_bf, j0, nj, h8, lo_eng=None):
                # hi = fp8(h * S_H) on ACT; lo = fp8(h*S_H - hi) on DVE/Pool
                nc.scalar.activation(
                    out=h8[:, 0, j0:j0 + nj, :], in_=h_bf[:, j0:j0 + nj, :],
                    func=mybir.ActivationFunctionType.Copy, scale=S_H)
                (lo_eng or nc.vector).scalar_tensor_tensor(
                    out=h8[:, 1, j0:j0 + nj, :], in0=h_bf[:, j0:j0 + nj, :],
                    scalar=S_H, in1=h8[:, 0, j0:j0 + nj, :],
                    op0=mybir.AluOpType.mult, op1=mybir.AluOpType.subtract)

            def conv_half(hf, rpos, pool, sched):
                """Generator: conv + max-pool drain for token half hf.

                Touches (PSUM reads) emit immediately; folds/merges/finales
                emit one round late via `pending` so no engine head-of-line
                blocks on a cross-engine dependency that is not ready yet.
                """
                from collections import deque
                hlo = HALF * hf
                hw_ = rpos // 2  # acc width
                pending = deque()
                pcast = deque()   # fp8 casts lag one extra round so the ACT
                                  # hi-cast never parks waiting the DVE finale

                def flush(keep):
                    while len(pending) > keep:
                        pending.popleft()()
                    while len(pcast) > max(keep, 1) + 1:
                        pcast.popleft()()

                DX = 3  # spare direct-write slots for D-round reduces
                for i, spec in enumerate(CH_TILES):
                    lhsT = wconv_t[:, 128 * i:128 * (i + 1)]
                    t_main = spec["t_main"]
                    acc = accpool.tile([128, hw_ + DX, HALF], BF16, tag="acc")
                    first = True
                    dstate = {"next": hw_}
                    t0 = 0
                    while t0 < t_main:
                        nt = min(rpos, t_main - t0)
                        P = pool.tile([128, rpos, HALF], FP32, tag=f"ps{rpos}")
                        for r in range(nt):
                            nc.tensor.matmul(
                                out=P[:, r, :], lhsT=lhsT,
                                rhs=conv_rhs(t0 + r, hlo),
                                start=True, stop=True)
                        flush(1)
                        if nt == rpos and (first or sched.kind() == "A"):
                            if first:
                                sched.note_forced("A")
                            tmp = spool.tile([128, rpos, HALF], BF16,
                                             tag="astk", bufs=4)
                            nc.scalar.activation(
                                out=tmp[:], in_=P[:],
                                func=mybir.ActivationFunctionType.Copy, scale=1.0)
                            eng = sched.fold_eng()
                            if first:
                                def op(eng=eng, tmp=tmp, acc=acc):
                                    eng.tensor_tensor(
                                        out=acc[:, 0:hw_, :],
                                        in0=tmp[:, 0:hw_, :],
                                        in1=tmp[:, hw_:rpos, :],
                                        op=mybir.AluOpType.max)
                                first = False
                            else:
                                def op(eng=eng, tmp=tmp, acc=acc):
                                    fh = spool.tile([128, hw_, HALF], BF16,
                                                    tag="fh", bufs=3)
                                    eng.tensor_tensor(
                                        out=fh[:], in0=tmp[:, 0:hw_, :],
                                        in1=tmp[:, hw_:rpos, :],
                                        op=mybir.AluOpType.max)
                                    eng.tensor_tensor(
                                        out=acc[:, 0:hw_, :],
                                        in0=acc[:, 0:hw_, :], in1=fh[:],
                                        op=mybir.AluOpType.max)
                            pending.append(op)
                        elif nt == 1:
                            nc.vector.tensor_tensor(
                                out=acc[:, 0, :], in0=acc[:, 0, :],
                                in1=P[:, 0, :], op=mybir.AluOpType.max)
                        elif dstate["next"] < hw_ + DX:
                            # D-led: reduce straight into a spare acc slot
                            nc.vector.reduce_max(
                                out=acc[:, dstate["next"], :],
                                in_=P[:, 0:nt, :].rearrange("p t n -> p n t"),
                                axis=mybir.AxisListType.X)
                            dstate["next"] += 1
                        else:
                            part = spool.tile([128, HALF], BF16, tag="part",
                                              bufs=3)
                            nc.vector.reduce_max(
                                out=part[:],
                                in_=P[:, 0:nt, :].rearrange("p t n -> p n t"),
                                axis=mybir.AxisListType.X)

                            def op(part=part, acc=acc):
                                nc.vector.tensor_tensor(
                                    out=acc[:, 0, :], in0=acc[:, 0, :],
                                    in1=part[:], op=mybir.AluOpType.max)
                            pending.append(op)
                        t0 += nt
                    # ragged tails (tile 0): positions t_main..50 on partition
                    # subranges; reduces touch PSUM now, merges deferred
                    if spec["tails"][0][2] > t_main:
                        nt = spec["tails"][0][2] - t_main
                        P = pool.tile([128, rpos, HALF], FP32, tag=f"ps{rpos}")
                        for r in range(nt):
                            nc.tensor.matmul(
                                out=P[:, r, :], lhsT=lhsT,
                                rhs=conv_rhs(t_main + r, hlo),
                                start=True, stop=True)
                        for (lo, hi, g_cnt) in spec["tails"]:
                            g_nt = g_cnt - t_main
                            if g_nt <= 0:
                                continue
                            if g_nt == 1:
                                nc.vector.tensor_tensor(
                                    out=acc[lo:hi, 0, :], in0=acc[lo:hi, 0, :],
                                    in1=P[lo:hi, 0, :], op=mybir.AluOpType.max)
                            else:
                                part = spool.tile([128, HALF], BF16, tag="part",
                                                  bufs=3)
                                nc.vector.reduce_max(
                                    out=part[lo:hi, :],
                                    in_=P[lo:hi, 0:g_nt, :].rearrange(
                                        "p t n -> p n t"),
                                    axis=mybir.AxisListType.X)

                                def op(part=part, acc=acc, lo=lo, hi=hi):
                                    nc.vector.tensor_tensor(
                                        out=acc[lo:hi, 1, :],
                                        in0=acc[lo:hi, 1, :],
                                        in1=part[lo:hi, :],
                                        op=mybir.AluOpType.max)
                                pending.append(op)

                    used = dstate["next"]

                    def finale(i=i, acc=acc, used=used):
                        pre = spool.tile([128, HALF], BF16, tag="pre")
                        cw = used
                        while cw > 2:
                            if cw % 2:
                                nc.vector.tensor_tensor(
                                    out=acc[:, 0, :], in0=acc[:, 0, :],
                                    in1=acc[:, cw - 1, :],
                                    op=mybir.AluOpType.max)
                                cw -= 1
                            h = cw // 2
                            nc.vector.tensor_tensor(
                                out=acc[:, 0:h, :], in0=acc[:, 0:h, :],
                                in1=acc[:, h:cw, :], op=mybir.AluOpType.max)
                            cw = h
                        nc.vector.tensor_tensor(
                            out=pre[:], in0=acc[:, 0, :], in1=acc[:, 1, :],
                            op=mybir.AluOpType.max)
                        nc.vector.tensor_scalar(
                            out=h1[hf][:, i, :], in0=pre[:],
                            scalar1=bconv_t[:, i:i + 1], scalar2=0.0,
                            op0=mybir.AluOpType.add, op1=mybir.AluOpType.max)
                    pending.append(finale)
                    if i % 2 == 1:
                        def cst(i=i):
                            cast_pair(h1[hf], i - 1, 2, h8c[hf])
                        pcast.append(cst)
                    yield
                flush(0)
                while pcast:
                    pcast.popleft()()

            def hw_mm_chain(p_out, wslab, h8, ofs, lo=True):
                # W8 x (h_hi [+ h_lo]); Wl correction skipped for the highway.
                # The sigmoid gate path also skips the h_lo chain (the gate
                # damps the quantization error; verified within tolerance).
                nhl = 2 if lo else 1
                for hl in range(nhl):
                    for cc in range(8):
                        nc.tensor.matmul(
                            out=p_out, lhsT=wslab[:, cc, :, ofs:ofs + 128],
                            rhs=h8[:, hl, 2 * cc:2 * cc + 2, :],
                            start=(hl == 0 and cc == 0),
                            stop=(hl == nhl - 1 and cc == 7),
                            perf_mode=mybir.MatmulPerfMode.DoubleRow)

            def hw_mm_chain_proj(p_out, h8, ofs):
                for hl in range(2):
                    for cc in range(8):
                        nc.tensor.matmul(
                            out=p_out, lhsT=wproj_t[:, cc, :, ofs:ofs + 128],
                            rhs=h8[:, hl, 2 * cc:2 * cc + 2, :],
                            start=(hl == 0 and cc == 0), stop=False,
                            perf_mode=mybir.MatmulPerfMode.DoubleRow)
                for cc in range(8):
                    nc.tensor.matmul(
                        out=p_out, lhsT=wprojc_t[:, cc, :, ofs:ofs + 128],
                        rhs=h8[:, 0, 2 * cc:2 * cc + 2, :],
                        start=False, stop=(cc == 7),
                        perf_mode=mybir.MatmulPerfMode.DoubleRow)

            def hw_half(hf):
                """Generator: highway l0+l1 + proj for token half hf.

                PE chains emit immediately; ACT/DVE epilogues lag one j so
                neither engine parks at its queue head waiting on a chain.
                """
                from collections import deque
                pending = deque()

                def flush(keep):
                    while len(pending) > keep:
                        pending.popleft()()

                state = {}
                for layer in range(2):
                    h_in = h1[hf] if layer == 0 else hmid[hf]
                    h8_in = h8c[hf] if layer == 0 else h8m[hf]
                    h8_out = h8m[hf] if layer == 0 else h8f[hf]
                    for j in range(16):
                        wslab = wpool.tile([128, 8, 2, 256], FP8, tag="wslab")
                        nc.sync.dma_start(out=wslab[:], in_=whw[layer, j])
                        hp = convp.tile([128, 2, HALF], FP32, tag="hwps",
                                        name="hp", bufs=2)
                        p_nl = hp[:, 0, :]
                        p_g = hp[:, 1, :]
                        hw_mm_chain(p_nl, wslab, h8_in, 0)
                        hw_mm_chain(p_g, wslab, h8_in, 128, lo=False)
                        flush(1)

                        def epi(layer=layer, j=j, p_nl=p_nl, p_g=p_g,
                                h_in=h_in, h8_out=h8_out):
                            nl = spool.tile([128, HALF], BF16, tag="nl")
                            gt = spool.tile([128, HALF], BF16, tag="gt")
                            nc.scalar.activation(
                                out=nl[:], in_=p_nl,
                                func=mybir.ActivationFunctionType.Relu,
                                bias=bhw_t[:, layer, j, 0:1], scale=DESCALE)
                            nc.scalar.activation(
                                out=gt[:], in_=p_g,
                                func=mybir.ActivationFunctionType.Sigmoid,
                                bias=bhw_t[:, layer, j, 1:2], scale=DESCALE)
                            d = spool.tile([128, HALF], BF16, tag="d")
                            nc.vector.tensor_tensor(
                                out=d[:], in0=h_in[:, j, :], in1=nl[:],
                                op=mybir.AluOpType.subtract)
                            m = spool.tile([128, HALF], BF16, tag="m")
                            nc.vector.tensor_mul(out=m[:], in0=gt[:], in1=d[:])
                            if layer == 0:
                                nc.vector.tensor_add(
                                    out=hmid[hf][:, j, :], in0=nl[:], in1=m[:])
                                if j % 2 == 1:
                                    cast_pair(hmid[hf], j - 1, 2, h8_out)
                            else:
                                if j % 2 == 0:
                                    state["htp"] = spool.tile(
                                        [128, 2, HALF], BF16, tag="htp",
                                        name="htp")
                                htp = state["htp"]
                                nc.vector.tensor_add(
                                    out=htp[:, j % 2, :], in0=nl[:], in1=m[:])
                                if j % 2 == 1:
                                    nc.scalar.activation(
                                        out=h8_out[:, 0, j - 1:j + 1, :],
                                        in_=htp[:],
                                        func=mybir.ActivationFunctionType.Copy,
                                        scale=S_H)
                                    nc.vector.scalar_tensor_tensor(
                                        out=h8_out[:, 1, j - 1:j + 1, :],
                                        in0=htp[:], scalar=S_H,
                                        in1=h8_out[:, 0, j - 1:j + 1, :],
                                        op0=mybir.AluOpType.mult,
                                        op1=mybir.AluOpType.subtract)
                        pending.append(epi)
                        yield
                    # layer barrier: next layer's chains read every h8 column
                    flush(0)
                # projection + transpose + out for this half
                hlo = HALF * hf
                for j2 in range(4):
                    hp = convp.tile([128, 2, HALF], FP32, tag="hwps",
                                    name="hp", bufs=2)
                    p_o = hp[:, 0, :]
                    hw_mm_chain_proj(p_o, h8f[hf], 128 * j2)
                    flush(1)

                    def proj_epi(j2=j2, hp=hp, p_o=p_o):
                        ot = spool.tile([128, HALF], FP32, tag="ot")
                        nc.scalar.activation(
                            out=ot[:], in_=p_o,
                            func=mybir.ActivationFunctionType.Identity,
                            bias=bproj_t[:, j2:j2 + 1], scale=DESCALE)
                        for m4 in range(2):
                            p_t = hp[:, 1, 128 * m4:128 * (m4 + 1)]
                            nc.tensor.transpose(
                                out=p_t, in_=ot[:, 128 * m4:128 * (m4 + 1)],
                                identity=ident_t[:])
                            ob = spool.tile([128, 128], FP32, tag="ob")
                            nc.scalar.copy(out=ob[:], in_=p_t)
                            row0 = hlo + 128 * m4
                            nc.sync.dma_start(
                                out=out[row0:row0 + 128,
                                        128 * j2:128 * (j2 + 1)],
                                in_=ob[:])
                    pending.append(proj_epi)
                    yield
                flush(0)

            # ---- phase 1: conv half A, 8-position rounds, all 8 PSUM banks
            sched1 = DrainSched(nc, *PH1)
            with tc.tile_pool(name="convp8", bufs=2, space="PSUM") as p8pool:
                for _ in conv_half(0, 8, p8pool, sched1):
                    pass
            stack5 = tc.tile_pool(name="convp", bufs=2, space="PSUM")
            convp = stack5.__enter__()
            # WAW-gate the projection-weight loads behind a DVE op that sits
            # late in DVE program order, so they cannot steal the DMA device
            # from the gather at t=0 (the sim schedules by readiness)
            nc.vector.memset(wproj_t[0:1, 0:1, 0:1, 0:1], 0.0)
            nc.vector.memset(wprojc_t[0:1, 0:1, 0:1, 0:1], 0.0)
            nc.sync.dma_start(out=wproj_t[:], in_=wproj[:])
            nc.sync.dma_start(out=wprojc_t[:], in_=wprojc[:])
            # ---- phase 2: conv half B interleaved with highway+proj half A
            sched2 = DrainSched(nc, *PH2)
            genB = conv_half(1, 6, convp, sched2)
            genA = hw_half(0)
            unitsB, unitsA = 16, 36
            credit = 0.0
            doneB = doneA = False
            while not (doneB and doneA):
                credit += unitsA / unitsB
                if not doneB:
                    doneB = next(genB, "end") == "end"
                while credit >= 1.0 and not doneA:
                    doneA = next(genA, "end") == "end"
                    credit -= 1.0
                if doneB:
                    while not doneA:
                        doneA = next(genA, "end") == "end"
            # ---- phase 3: highway+proj half B
            for _ in hw_half(1):
                pass

            for st in (stack5, stack6, stack4, stack3, stack2, stack, stackw):
                st.__exit__(None, None, None)

    nc.compile()
    return nc


_CACHED = {}


def _prep(inputs):
    """Host-side layout prep: sharding, pair tables, weight packing."""
    chars = np.asarray(inputs["chars"]).astype(np.int64).reshape(NTOK, L)
    pairs = chars[:, 0::2] * CHAR_VOCAB + chars[:, 1::2]   # [NTOK, 25]

    emb = np.asarray(inputs["char_emb"], np.float32)

    wc = np.zeros((7, CHAR_DIM, N_FILTERS), np.float32)
    off = 0
    for fi, (w, n) in enumerate(FILTERS):
        cw = np.asarray(inputs[f"conv_w_{fi}"], np.float32)
        wc[:w, :, off:off + n] = cw.transpose(2, 1, 0)
        off += n
    wconv = wc.reshape(KDIM, N_FILTERS).astype(ml_dtypes.bfloat16)
    bconv = np.concatenate([np.asarray(inputs[f"conv_b_{i}"], np.float32)
                            for i in range(7)])
    bconv_dev = bconv.reshape(16, 128).T.copy()

    # highway weights: fp8 W8 packed for DoubleRow streaming.
    whw8 = np.zeros((2, 16, 128, 8, 2, 256), np.float32)
    bhw = np.zeros((2, 128, 16, 2), np.float32)
    for l in range(2):
        W = np.asarray(inputs[f"hw_w_{l}"], np.float32)   # (4096, 2048)
        bb = np.asarray(inputs[f"hw_b_{l}"], np.float32)
        Ws = W * S_W
        W8 = Ws.astype(ml_dtypes.float8_e4m3).astype(np.float32)
        W8T = W8.T  # (2048 ic, 4096 oc)
        for j in range(16):
            for hf in range(2):
                oc0 = 2048 * hf + 128 * j
                for cc in range(8):
                    for g in range(2):
                        cb = 2 * cc + g
                        blk8 = W8T[128 * cb:128 * (cb + 1), oc0:oc0 + 128]
                        whw8[l, j, :, cc, g, 128 * hf:128 * hf + 128] = blk8
            bhw[l, :, j, 0] = bb[128 * j:128 * (j + 1)]
            bhw[l, :, j, 1] = bb[2048 + 128 * j:2048 + 128 * (j + 1)]
    whw8 = whw8.astype(ml_dtypes.float8_e4m3)

    Wp = np.asarray(inputs["proj_w"], np.float32) * S_W  # (512, 2048)
    Wp8 = Wp.astype(ml_dtypes.float8_e4m3).astype(np.float32)
    Wpl = (Wp - Wp8).astype(ml_dtypes.float8_e4m3).astype(np.float32)
    Wp8T = Wp8.T  # (2048, 512)
    WplT = Wpl.T
    wproj8 = np.zeros((128, 8, 2, 512), np.float32)
    wprojc8 = np.zeros((128, 8, 2, 512), np.float32)
    for cc in range(8):
        for g in range(2):
            cb = 2 * cc + g
            wproj8[:, cc, g, :] = Wp8T[128 * cb:128 * (cb + 1), :]
            wprojc8[:, cc, g, :] = WplT[128 * cb:128 * (cb + 1), :]
    wproj8 = wproj8.astype(ml_dtypes.float8_e4m3)
    wprojc8 = wprojc8.astype(ml_dtypes.float8_e4m3)
    bproj = np.zeros((128, 4), np.float32)
    bp = np.asarray(inputs["proj_b"], np.float32)
    for j2 in range(4):
        bproj[:, j2] = bp[128 * j2:128 * (j2 + 1)]

    ident = np.eye(128, dtype=np.float32)

    shared = dict(wconv=wconv, bconv=bconv_dev, whw=whw8,
                  bhw=bhw, wproj=wproj8, wprojc=wprojc8,
                  bproj=bproj, ident=ident)

    in_maps = []
    for core in range(N_CORES):
        cp = pairs[core * TOK:(core + 1) * TOK]            # [512, 25]
        uniq, inv = np.unique(cp, return_inverse=True)
        assert len(uniq) <= TABLE_ROWS, len(uniq)
        tbl = np.zeros((TABLE_ROWS, 128), np.float32)
        tbl[:len(uniq), 0:CHAR_DIM] = emb[uniq // CHAR_VOCAB]
        tbl[:len(uniq), CHAR_DIM:2 * CHAR_DIM] = emb[uniq % CHAR_VOCAB]
        idx_flat = inv.reshape(TOK, NPAIR).T.reshape(-1).astype(np.int16)
        idx16 = idx_flat.reshape(NI // 16, 16).T.copy()
        idx16 = np.tile(idx16, (8, 1))
        m = dict(shared)
        m["table"] = tbl.astype(ml_dtypes.bfloat16)
        m["idx"] = idx16
        in_maps.append(m)
    return in_maps


def kernel(**inputs) -> np.ndarray:
    if "nc" not in _CACHED:
        _CACHED["nc"] = build_module()
    nc = _CACHED["nc"]
    in_maps = _prep(inputs)
    res = run_bass_kernel_spmd(nc, in_maps, core_ids=list(range(N_CORES)))
    full = np.concatenate([r["out"] for r in res.results], axis=0)
    return full.reshape(B, S, PROJ_DIM)


# revision 36
# speedup vs baseline: 1.0586x; 1.0386x over previous
"""CharCNN token embedder (ELMo-style) on 8 Trainium2 NeuronCores.

Data-parallel over the 4096 = 16*256 tokens (512 per core). Weights replicated.

Per-core pipeline (v5):
  1. Char-PAIR gather: host packs each token's 50 chars into 25 pairs and
     builds a per-core table of unique pairs (~12k rows of 256B; cols 0:32
     hold both chars' embeddings) -> 12800 descriptors instead of 28672.
     Four pair-aligned gather chunks; shifted parity-strided SBUF copies
     (fused over contiguous tap pairs) build the K=112 im2col patch matrix;
     conv starts after two chunks.
  2. Tokens processed in two halves of 256: phase1 = conv(A); phase2 =
     conv(B) interleaved with highway+proj(A); phase3 = highway+proj(B).
  3. Conv = bf16 matmuls, K=112, one position per matmul, rounds of 8
     (phase1) / 6 (phase2) positions in double-buffered PSUM. Max-pool
     drain per tile into acc[128, rpos//2 + 3, HALF] bf16:
       ~80% A-led rounds: ACT copies the PSUM round to a tmp stack; DVE
         folds it to half width and merges into acc (emitted one round
         late via a deferred-op queue so no in-order engine parks on a
         cross-engine dependency).
       ~20% D-led rounds: DVE reduce_max writes a spare acc slot directly
         (no merge op). Partial rounds are always D-led.
     Finale: in-place pairwise fold of acc + bias+relu on DVE; fp8 hi/lo
     casts batched per tile pair and deferred two rounds (ACT never waits
     the DVE finale chain). Only ACT and DVE can touch PSUM and the Pool
     engine cannot run TensorTensor on TRN2, so Pool only runs the gather.
  4. Highway layers in fp8 DoubleRow at 2x bf16 throughput: per (layer, j),
     psum = W8(h_hi) + W8(h_lo), scaled e4m3 (S_W=512, S_H=32), descale via
     the ACT sigmoid/relu scale; gating on DVE bf16. PE chains emit
     immediately, ACT/DVE epilogues lag one j. h8 layout [128, hi/lo, j, n]
     batches (j, j+1) casts; the final-layer fp8 buffer aliases the conv
     one (layer-0 chains consume it before layer-1 writes).
  5. Projection fp8 DoubleRow (hi, lo, W-residual chains); PE-transpose;
     ACT bounce to SBUF; DMA out. Projection weight loads are WAW-gated
     behind a late DVE memset so they cannot steal the DMA device from the
     gather at t=0.
"""

import numpy as np
import ml_dtypes

import concourse.bass as bass
import concourse.mybir as mybir
import concourse.tile as tile
from concourse import bacc
from concourse.bass_utils import run_bass_kernel_spmd
from concourse.vector_clock import ScopedClock

# ---------------------------------------------------------------- constants
B, S, L = 16, 256, 50
CHAR_DIM = 16
CHAR_VOCAB = 262
FILTERS = [(1, 32), (2, 32), (3, 64), (4, 128), (5, 256), (6, 512), (7, 1024)]
N_FILTERS = 2048
PROJ_DIM = 512
N_CORES = 8
NTOK = B * S
TOK = NTOK // N_CORES        # 512 tokens per core
NPOS = 50
NPAIR = 25                   # char pairs per token
NPPAD = 28                   # padded pair positions (tap reach 55 -> pair 27)
NI = TOK * NPAIR             # 12800 gather indices per core
TABLE_ROWS = 32768           # fixed per-core unique-pair table allocation
KDIM = 112                   # 7 taps * 16 dims
TSPLIT = 22                  # conv positions < TSPLIT read xsA

S_W = 512.0                  # fp8 storage scale for highway/proj weights
S_H = 32.0                   # fp8 storage scale for highway/proj activations
DESCALE = 1.0 / (S_W * S_H)

# per 128-channel tile: valid positions; tile 0 packs w=1,2,3 with tails
CH_TILES = []
CH_TILES.append({"t_main": 48, "tails": [(0, 32, 50), (32, 64, 49), (64, 128, 48)]})
CH_TILES.append({"t_main": 47, "tails": [(0, 128, 47)]})      # w4
for _ in range(2):
    CH_TILES.append({"t_main": 46, "tails": [(0, 128, 46)]})  # w5
for _ in range(4):
    CH_TILES.append({"t_main": 45, "tails": [(0, 128, 45)]})  # w6
for _ in range(8):
    CH_TILES.append({"t_main": 44, "tails": [(0, 128, 44)]})  # w7

BF16 = mybir.dt.bfloat16
FP32 = mybir.dt.float32
FP8 = mybir.dt.float8e4

# drain schedule fractions: (pA, pPoolMerge, pPoolFinale) per phase
PH1 = (0.78, 0.0, 0.0)
PH2 = (0.75, 0.0, 0.0)

_MAX_WAITS_PER_INST = 1


def _patched_drain_and_barrier(self, tick_clock, wait_clock):
    # The walrus build in this container rejects CTRL instructions carrying
    # more than one sem wait; spread the kernel-tail drain waits over NOPs.
    nc = self.nc
    carrier = nc.sync.nop()
    wait_clock.add_sem_waits(carrier.ins, ScopedClock({None: tick_clock.global_clock}))
    si = carrier.ins.sync_info
    waits = list(si.on_wait) if si is not None and si.on_wait else []
    if len(waits) > _MAX_WAITS_PER_INST:
        carrier.ins.sync_info = mybir.SyncInfo(
            on_wait=waits[:_MAX_WAITS_PER_INST],
            on_update=list(si.on_update) if si.on_update else [])
        for i in range(_MAX_WAITS_PER_INST, len(waits), _MAX_WAITS_PER_INST):
            extra = nc.sync.nop()
            extra.ins.sync_info = mybir.SyncInfo(
                on_wait=waits[i:i + _MAX_WAITS_PER_INST], on_update=[])
    nc.sync.drain()
    nc.all_engine_barrier()
    assert self.sems is not None
    popped = nc._tile_sem_poison_stack.pop()
    assert popped is self._sem_poison
    nc.clear_and_free_semaphores(list(self.sems.allocated().values()))
    nc.all_engine_barrier()


tile.TileContext._drain_and_barrier = _patched_drain_and_barrier


class DrainSched:
    """Debt-based allocator: round kind (A/D), fold + merge engines."""

    def __init__(self, nc, pA, unused0=0.0, unused1=0.0):
        self.nc = nc
        self.pA = pA
        self.da = 0.0

    def kind(self):
        self.da += self.pA
        if self.da >= 1.0:
            self.da -= 1.0
            return "A"
        return "D"

    def note_forced(self, k):
        pass

    def fold_eng(self):
        return self.nc.vector


# ---------------------------------------------------------------- device IR
def build_module():
    nc = bacc.Bacc()
    SIdx = NI // 16

    # 256-byte rows (dma_gather granularity); cols 0:32 hold the pair embs
    table = nc.dram_tensor("table", [TABLE_ROWS, 128], BF16,
                           kind="ExternalInput")
    idx = nc.dram_tensor("idx", [128, SIdx], mybir.dt.int16, kind="ExternalInput")
    wconv = nc.dram_tensor("wconv", [KDIM, N_FILTERS], BF16, kind="ExternalInput")
    bconv = nc.dram_tensor("bconv", [128, 16], FP32, kind="ExternalInput")
    # highway weights fp8, host-packed per (layer, j):
    #   [l, j, p(128), cc(8), g(2), half*128+oc] ; g = DoubleRow group
    whw = nc.dram_tensor("whw", [2, 16, 128, 8, 2, 256], FP8, kind="ExternalInput")
    bhw = nc.dram_tensor("bhw", [2, 128, 16, 2], FP32, kind="ExternalInput")
    wproj = nc.dram_tensor("wproj", [128, 8, 2, 512], FP8, kind="ExternalInput")
    wprojc = nc.dram_tensor("wprojc", [128, 8, 2, 512], FP8, kind="ExternalInput")
    bproj = nc.dram_tensor("bproj", [128, 4], FP32, kind="ExternalInput")
    ident = nc.dram_tensor("ident", [128, 128], FP32, kind="ExternalInput")
    out = nc.dram_tensor("out", [TOK, PROJ_DIM], FP32, kind="ExternalOutput")

    with tile.TileContext(nc) as tc:
        with (
            tc.tile_pool(name="xs", bufs=1) as xspool,
            tc.tile_pool(name="consts", bufs=1) as cpool,
        ):
            # ---- constants in (wconv loads after idx; the rest is deferred
            # so nothing delays the gather + xsA stream on the DMA device)
            wconv_t = cpool.tile([KDIM, N_FILTERS], BF16)
            bconv_t = cpool.tile([128, 16], FP32)
            bhw_t = cpool.tile([128, 2, 16, 2], FP32)
            bproj_t = cpool.tile([128, 4], FP32)
            ident_t = cpool.tile([128, 128], FP32)

            # ---- 1. pair-gather char embeddings + build K=112 patch matrix.
            # xsA/xsB viewed [128, m, parity, TOK] so the strided parity
            # interleave is a plain AP (no step slicing).
            NA = TSPLIT // 2          # 11 position pairs in xsA
            NB = (NPOS - TSPLIT) // 2  # 14 in xsB
            xsA = xspool.tile([KDIM, NA, 2, TOK], BF16, name="xsA")
            xsB = xspool.tile([KDIM, NB, 2, TOK], BF16, name="xsB")
            with tc.tile_pool(name="gather", bufs=1) as gpool:
                idx_t = gpool.tile([128, SIdx], mybir.dt.int16)
                nc.sync.dma_start(out=idx_t[:], in_=idx[:])
                nc.sync.dma_start(out=wconv_t[:], in_=wconv[:])
                xg = gpool.tile([128, 1, TOK * NPPAD], BF16)
                nc.vector.memset(xg[0:32, 0, NI:TOK * NPPAD], 0.0)
                # pair-aligned chunks (idx counts): 7,7,7,4 pairs
                chunks = [(0, 3584), (3584, 3584), (7168, 3584), (10752, 2048)]
                for r, (o, cn) in enumerate(chunks):
                    nc.gpsimd.dma_gather(
                        out_ap=xg[:, :, o:o + cn],
                        in_ap=table[:],
                        idxs_ap=idx_t[:, o // 16:(o + cn) // 16],
                        num_idxs=cn,
                        num_idxs_reg=cn,
                        elem_size=128,
                        transpose=True,
                        single_packet=False,
                    )
                    if r == 1:
                        # xsA copies, fused over (k, k+1) pairs where the
                        # source partition blocks are contiguous (halves the
                        # HWDGE descriptor-generation serial chain)
                        for s in range(2):
                            k = 0
                            while k < 7:
                                p0 = (s + k) // 2
                                par = (s + k) % 2
                                if par == 0 and k + 1 < 7:
                                    nc.sync.dma_start(
                                        out=xsA[16 * k:16 * (k + 2), :, s, :],
                                        in_=xg[0:32, 0,
                                               TOK * p0:TOK * (p0 + NA)],
                                    )
                                    k += 2
                                else:
                                    nc.sync.dma_start(
                                        out=xsA[16 * k:16 * (k + 1), :, s, :],
                                        in_=xg[16 * par:16 * par + 16, 0,
                                               TOK * p0:TOK * (p0 + NA)],
                                    )
                                    k += 1
                for s in range(2):
                    k = 0
                    while k < 7:
                        t0 = TSPLIT + s
                        p0 = (t0 + k) // 2
                        par = (t0 + k) % 2
                        if par == 0 and k + 1 < 7:
                            nc.sync.dma_start(
                                out=xsB[16 * k:16 * (k + 2), :, s, :],
                                in_=xg[0:32, 0, TOK * p0:TOK * (p0 + NB)],
                            )
                            k += 2
                        else:
                            nc.sync.dma_start(
                                out=xsB[16 * k:16 * (k + 1), :, s, :],
                                in_=xg[16 * par:16 * par + 16, 0,
                                       TOK * p0:TOK * (p0 + NB)],
                            )
                            k += 1
            nc.sync.dma_start(out=bconv_t[:], in_=bconv[:])
            nc.sync.dma_start(out=bhw_t[:], in_=bhw[:].rearrange("l p j h -> p l j h"))
            nc.sync.dma_start(out=bproj_t[:], in_=bproj[:])
            nc.sync.dma_start(out=ident_t[:], in_=ident[:])
            stackw = tc.tile_pool(name="wppool", bufs=1)
            wppool = stackw.__enter__()
            wproj_t = wppool.tile([128, 8, 2, 512], FP8)
            wprojc_t = wppool.tile([128, 8, 2, 512], FP8)

            HALF = TOK // 2

            def conv_rhs(t, hlo):
                if t < TSPLIT:
                    return xsA[:, t // 2, t % 2, hlo:hlo + HALF]
                tl = t - TSPLIT
                return xsB[:, tl // 2, tl % 2, hlo:hlo + HALF]

            stack = tc.tile_pool(name="hbuf", bufs=1)
            hpool = stack.__enter__()
            stack2 = tc.tile_pool(name="h8buf", bufs=1)
            h8pool = stack2.__enter__()
            stack3 = tc.tile_pool(name="wstream", bufs=3)
            wpool = stack3.__enter__()
            stack4 = tc.tile_pool(name="small", bufs=2)
            spool = stack4.__enter__()
            stack6 = tc.tile_pool(name="accpool", bufs=6)
            accpool = stack6.__enter__()
            convp = None  # phase-2/3 PSUM pool, opened after phase 1

            # per-half persistent tensors (separate tiles avoid false deps)
            h1 = [hpool.tile([128, 16, HALF], BF16, tag=f"h1{s_}", name=f"h1{s_}")
                  for s_ in range(2)]
            hmid = [hpool.tile([128, 16, HALF], BF16, tag=f"hm{s_}", name=f"hm{s_}")
                    for s_ in range(2)]
            # h8 layout: [128, hi/lo, j, n] so (j, j+1) casts batch into one op
            h8c = [h8pool.tile([128, 2, 16, HALF], FP8, tag=f"h8c{s_}", name=f"h8c{s_}")
                   for s_ in range(2)]
            h8m = [h8pool.tile([128, 2, 16, HALF], FP8, tag=f"h8m{s_}", name=f"h8m{s_}")
                   for s_ in range(2)]
            # h8f aliases h8c: layer-0 chains fully consume h8c before
            # layer-1 writes the final activations (WAR handled by deps)
            h8f = h8c

            def cast_pair(h_bf, j0, nj, h8, lo_eng=None):
                # hi = fp8(h * S_H) on ACT; lo = fp8(h*S_H - hi) on DVE/Pool
                nc.scalar.activation(
                    out=h8[:, 0, j0:j0 + nj, :], in_=h_bf[:, j0:j0 + nj, :],
                    func=mybir.ActivationFunctionType.Copy, scale=S_H)
                (lo_eng or nc.vector).scalar_tensor_tensor(
                    out=h8[:, 1, j0:j0 + nj, :], in0=h_bf[:, j0:j0 + nj, :],
                    scalar=S_H, in1=h8[:, 0, j0:j0 + nj, :],
                    op0=mybir.AluOpType.mult, op1=mybir.AluOpType.subtract)

            def conv_half(hf, rpos, pool, sched):
                """Generator: conv + max-pool drain for token half hf.

                Touches (PSUM reads) emit immediately; folds/merges/finales
                emit one round late via `pending` so no engine head-of-line
                blocks on a cross-engine dependency that is not ready yet.
                """
                from collections import deque
                hlo = HALF * hf
                hw_ = rpos // 2  # acc width
                pending = deque()
                pcast = deque()   # fp8 casts lag one extra round so the ACT
                                  # hi-cast never parks waiting the DVE finale

                def flush(keep):
                    while len(pending) > keep:
                        pending.popleft()()
                    while len(pcast) > max(keep, 1) + 1:
                        pcast.popleft()()

                DX = 4  # spare direct-write slots for D-round reduces
                for i, spec in enumerate(CH_TILES):
                    lhsT = wconv_t[:, 128 * i:128 * (i + 1)]
                    t_main = spec["t_main"]
                    acc = accpool.tile([128, hw_ + DX, HALF], BF16, tag="acc")
                    first = True
                    dstate = {"next": hw_}
                    t0 = 0
                    while t0 < t_main:
                        nt = min(rpos, t_main - t0)
                        P = pool.tile([128, rpos, HALF], FP32, tag=f"ps{rpos}")
                        for r in range(nt):
                            nc.tensor.matmul(
                                out=P[:, r, :], lhsT=lhsT,
                                rhs=conv_rhs(t0 + r, hlo),
                                start=True, stop=True)
                        flush(1)
                        if nt == rpos and (first or sched.kind() == "A"):
                            if first:
                                sched.note_forced("A")
                            tmp = spool.tile([128, rpos, HALF], BF16,
                                             tag="astk", bufs=4)
                            nc.scalar.activation(
                                out=tmp[:], in_=P[:],
                                func=mybir.ActivationFunctionType.Copy, scale=1.0)
                            eng = sched.fold_eng()
                            if first:
                                def op(eng=eng, tmp=tmp, acc=acc):
                                    eng.tensor_tensor(
                                        out=acc[:, 0:hw_, :],
                                        in0=tmp[:, 0:hw_, :],
                                        in1=tmp[:, hw_:rpos, :],
                                        op=mybir.AluOpType.max)
                                first = False
                            else:
                                def op(eng=eng, tmp=tmp, acc=acc):
                                    fh = spool.tile([128, hw_, HALF], BF16,
                                                    tag="fh", bufs=3)
                                    eng.tensor_tensor(
                                        out=fh[:], in0=tmp[:, 0:hw_, :],
                                        in1=tmp[:, hw_:rpos, :],
                                        op=mybir.AluOpType.max)
                                    eng.tensor_tensor(
                                        out=acc[:, 0:hw_, :],
                                        in0=acc[:, 0:hw_, :], in1=fh[:],
                                        op=mybir.AluOpType.max)
                            pending.append(op)
                        elif nt == 1:
                            nc.vector.tensor_tensor(
                                out=acc[:, 0, :], in0=acc[:, 0, :],
                                in1=P[:, 0, :], op=mybir.AluOpType.max)
                        elif dstate["next"] < hw_ + DX:
                            # D-led: reduce straight into a spare acc slot
                            nc.vector.reduce_max(
                                out=acc[:, dstate["next"], :],
                                in_=P[:, 0:nt, :].rearrange("p t n -> p n t"),
                                axis=mybir.AxisListType.X)
                            dstate["next"] += 1
                        else:
                            part = spool.tile([128, HALF], BF16, tag="part",
                                              bufs=3)
                            nc.vector.reduce_max(
                                out=part[:],
                                in_=P[:, 0:nt, :].rearrange("p t n -> p n t"),
                                axis=mybir.AxisListType.X)

                            def op(part=part, acc=acc):
                                nc.vector.tensor_tensor(
                                    out=acc[:, 0, :], in0=acc[:, 0, :],
                                    in1=part[:], op=mybir.AluOpType.max)
                            pending.append(op)
                        t0 += nt
                    # ragged tails (tile 0): positions t_main..50 on partition
                    # subranges; reduces touch PSUM now, merges deferred
                    if spec["tails"][0][2] > t_main:
                        nt = spec["tails"][0][2] - t_main
                        P = pool.tile([128, rpos, HALF], FP32, tag=f"ps{rpos}")
                        for r in range(nt):
                            nc.tensor.matmul(
                                out=P[:, r, :], lhsT=lhsT,
                                rhs=conv_rhs(t_main + r, hlo),
                                start=True, stop=True)
                        for (lo, hi, g_cnt) in spec["tails"]:
                            g_nt = g_cnt - t_main
                            if g_nt <= 0:
                                continue
                            if g_nt == 1:
                                nc.vector.tensor_tensor(
                                    out=acc[lo:hi, 0, :], in0=acc[lo:hi, 0, :],
                                    in1=P[lo:hi, 0, :], op=mybir.AluOpType.max)
                            else:
                                part = spool.tile([128, HALF], BF16, tag="part",
                                                  bufs=3)
                                nc.vector.reduce_max(
                                    out=part[lo:hi, :],
                                    in_=P[lo:hi, 0:g_nt, :].rearrange(
                                        "p t n -> p n t"),
                                    axis=mybir.AxisListType.X)

                                def op(part=part, acc=acc, lo=lo, hi=hi):
                                    nc.vector.tensor_tensor(
                                        out=acc[lo:hi, 1, :],
                                        in0=acc[lo:hi, 1, :],
                                        in1=part[lo:hi, :],
                                        op=mybir.AluOpType.max)
                                pending.append(op)

                    used = dstate["next"]

                    def finale(i=i, acc=acc, used=used):
                        pre = spool.tile([128, HALF], BF16, tag="pre")
                        cw = used
                        while cw > 2:
                            if cw % 2:
                                nc.vector.tensor_tensor(
                                    out=acc[:, 0, :], in0=acc[:, 0, :],
                                    in1=acc[:, cw - 1, :],
                                    op=mybir.AluOpType.max)
                                cw -= 1
                            h = cw // 2
                            nc.vector.tensor_tensor(
                                out=acc[:, 0:h, :], in0=acc[:, 0:h, :],
                                in1=acc[:, h:cw, :], op=mybir.AluOpType.max)
                            cw = h
                        nc.vector.tensor_tensor(
                            out=pre[:], in0=acc[:, 0, :], in1=acc[:, 1, :],
                            op=mybir.AluOpType.max)
                        nc.vector.tensor_scalar(
                            out=h1[hf][:, i, :], in0=pre[:],
                            scalar1=bconv_t[:, i:i + 1], scalar2=0.0,
                            op0=mybir.AluOpType.add, op1=mybir.AluOpType.max)
                    pending.append(finale)
                    if i % 2 == 1:
                        def cst(i=i):
                            cast_pair(h1[hf], i - 1, 2, h8c[hf])
                        pcast.append(cst)
                    yield
                flush(0)
                while pcast:
                    pcast.popleft()()

            def hw_mm_chain(p_out, wslab, h8, ofs, lo=True):
                # W8 x (h_hi [+ h_lo]); Wl correction skipped for the highway.
                # The sigmoid gate path also skips the h_lo chain (the gate
                # damps the quantization error; verified within tolerance).
                nhl = 2 if lo else 1
                for hl in range(nhl):
                    for cc in range(8):
                        nc.tensor.matmul(
                            out=p_out, lhsT=wslab[:, cc, :, ofs:ofs + 128],
                            rhs=h8[:, hl, 2 * cc:2 * cc + 2, :],
                            start=(hl == 0 and cc == 0),
                            stop=(hl == nhl - 1 and cc == 7),
                            perf_mode=mybir.MatmulPerfMode.DoubleRow)

            def hw_mm_chain_proj(p_out, h8, ofs):
                for hl in range(2):
                    for cc in range(8):
                        nc.tensor.matmul(
                            out=p_out, lhsT=wproj_t[:, cc, :, ofs:ofs + 128],
                            rhs=h8[:, hl, 2 * cc:2 * cc + 2, :],
                            start=(hl == 0 and cc == 0), stop=False,
                            perf_mode=mybir.MatmulPerfMode.DoubleRow)
                for cc in range(8):
                    nc.tensor.matmul(
                        out=p_out, lhsT=wprojc_t[:, cc, :, ofs:ofs + 128],
                        rhs=h8[:, 0, 2 * cc:2 * cc + 2, :],
                        start=False, stop=(cc == 7),
                        perf_mode=mybir.MatmulPerfMode.DoubleRow)

            def hw_half(hf):
                """Generator: highway l0+l1 + proj for token half hf.

                PE chains emit immediately; ACT/DVE epilogues lag one j so
                neither engine parks at its queue head waiting on a chain.
                """
                from collections import deque
                pending = deque()

                def flush(keep):
                    while len(pending) > keep:
                        pending.popleft()()

                state = {}
                for layer in range(2):
                    h_in = h1[hf] if layer == 0 else hmid[hf]
                    h8_in = h8c[hf] if layer == 0 else h8m[hf]
                    h8_out = h8m[hf] if layer == 0 else h8f[hf]
                    for j in range(16):
                        wslab = wpool.tile([128, 8, 2, 256], FP8, tag="wslab")
                        nc.sync.dma_start(out=wslab[:], in_=whw[layer, j])
                        hp = convp.tile([128, 2, HALF], FP32, tag="hwps",
                                        name="hp", bufs=2)
                        p_nl = hp[:, 0, :]
                        p_g = hp[:, 1, :]
                        hw_mm_chain(p_nl, wslab, h8_in, 0)
                        hw_mm_chain(p_g, wslab, h8_in, 128, lo=False)
                        flush(1)

                        def epi(layer=layer, j=j, p_nl=p_nl, p_g=p_g,
                                h_in=h_in, h8_out=h8_out):
                            nl = spool.tile([128, HALF], BF16, tag="nl")
                            gt = spool.tile([128, HALF], BF16, tag="gt")
                            nc.scalar.activation(
                                out=nl[:], in_=p_nl,
                                func=mybir.ActivationFunctionType.Relu,
                                bias=bhw_t[:, layer, j, 0:1], scale=DESCALE)
                            nc.scalar.activation(
                                out=gt[:], in_=p_g,
                                func=mybir.ActivationFunctionType.Sigmoid,
                                bias=bhw_t[:, layer, j, 1:2], scale=DESCALE)
                            d = spool.tile([128, HALF], BF16, tag="d")
                            nc.vector.tensor_tensor(
                                out=d[:], in0=h_in[:, j, :], in1=nl[:],
                                op=mybir.AluOpType.subtract)
                            m = spool.tile([128, HALF], BF16, tag="m")
                            nc.vector.tensor_mul(out=m[:], in0=gt[:], in1=d[:])
                            if layer == 0:
                                nc.vector.tensor_add(
                                    out=hmid[hf][:, j, :], in0=nl[:], in1=m[:])
                                if j % 2 == 1:
                                    cast_pair(hmid[hf], j - 1, 2, h8_out)
                            else:
                                if j % 2 == 0:
                                    state["htp"] = spool.tile(
                                        [128, 2, HALF], BF16, tag="htp",
                                        name="htp")
                                htp = state["htp"]
                                nc.vector.tensor_add(
                                    out=htp[:, j % 2, :], in0=nl[:], in1=m[:])
                                if j % 2 == 1:
                                    nc.scalar.activation(
                                        out=h8_out[:, 0, j - 1:j + 1, :],
                                        in_=htp[:],
                                        func=mybir.ActivationFunctionType.Copy,
                                        scale=S_H)
                                    nc.vector.scalar_tensor_tensor(
                                        out=h8_out[:, 1, j - 1:j + 1, :],
                                        in0=htp[:], scalar=S_H,
                                        in1=h8_out[:, 0, j - 1:j + 1, :],
                                        op0=mybir.AluOpType.mult,
                                        op1=mybir.AluOpType.subtract)
                        pending.append(epi)
                        yield
                    # layer barrier: next layer's chains read every h8 column
                    flush(0)
                # projection + transpose + out for this half
                hlo = HALF * hf
                for j2 in range(4):
                    hp = convp.tile([128, 2, HALF], FP32, tag="hwps",
                                    name="hp", bufs=2)
                    p_o = hp[:, 0, :]
                    hw_mm_chain_proj(p_o, h8f[hf], 128 * j2)
                    flush(1)

                    def proj_epi(j2=j2, hp=hp, p_o=p_o):
                        ot = spool.tile([128, HALF], FP32, tag="ot")
                        nc.scalar.activation(
                            out=ot[:], in_=p_o,
                            func=mybir.ActivationFunctionType.Identity,
                            bias=bproj_t[:, j2:j2 + 1], scale=DESCALE)
                        for m4 in range(2):
                            p_t = hp[:, 1, 128 * m4:128 * (m4 + 1)]
                            nc.tensor.transpose(
                                out=p_t, in_=ot[:, 128 * m4:128 * (m4 + 1)],
                                identity=ident_t[:])
                            ob = spool.tile([128, 128], FP32, tag="ob")
                            nc.scalar.copy(out=ob[:], in_=p_t)
                            row0 = hlo + 128 * m4
                            nc.sync.dma_start(
                                out=out[row0:row0 + 128,
                                        128 * j2:128 * (j2 + 1)],
                                in_=ob[:])
                    pending.append(proj_epi)
                    yield
                flush(0)

            # ---- phase 1: conv half A, 8-position rounds, all 8 PSUM banks
            sched1 = DrainSched(nc, *PH1)
            with tc.tile_pool(name="convp8", bufs=4, space="PSUM") as p8pool:
                for _ in conv_half(0, 4, p8pool, sched1):
                    pass
            stack5 = tc.tile_pool(name="convp", bufs=2, space="PSUM")
            convp = stack5.__enter__()
            # WAW-gate the projection-weight loads behind a DVE op that sits
            # late in DVE program order, so they cannot steal the DMA device
            # from the gather at t=0 (the sim schedules by readiness)
            nc.vector.memset(wproj_t[0:1, 0:1, 0:1, 0:1], 0.0)
            nc.vector.memset(wprojc_t[0:1, 0:1, 0:1, 0:1], 0.0)
            nc.sync.dma_start(out=wproj_t[:], in_=wproj[:])
            nc.sync.dma_start(out=wprojc_t[:], in_=wprojc[:])
            # ---- phase 2: conv half B interleaved with highway+proj half A
            sched2 = DrainSched(nc, *PH2)
            genB = conv_half(1, 6, convp, sched2)
            genA = hw_half(0)
            unitsB, unitsA = 16, 36
            credit = 0.0
            doneB = doneA = False
            while not (doneB and doneA):
                credit += unitsA / unitsB
                if not doneB:
                    doneB = next(genB, "end") == "end"
                while credit >= 1.0 and not doneA:
                    doneA = next(genA, "end") == "end"
                    credit -= 1.0
                if doneB:
                    while not doneA:
                        doneA = next(genA, "end") == "end"
            # ---- phase 3: highway+proj half B
            for _ in hw_half(1):
                pass

            for st in (stack5, stack6, stack4, stack3, stack2, stack, stackw):
                st.__exit__(None, None, None)

    nc.compile()
    return nc


_CACHED = {}


def _prep(inputs):
    """Host-side layout prep: sharding, pair tables, weight packing."""
    chars = np.asarray(inputs["chars"]).astype(np.int64).reshape(NTOK, L)
    pairs = chars[:, 0::2] * CHAR_VOCAB + chars[:, 1::2]   # [NTOK, 25]

    emb = np.asarray(inputs["char_emb"], np.float32)

    wc = np.zeros((7, CHAR_DIM, N_FILTERS), np.float32)
    off = 0
    for fi, (w, n) in enumerate(FILTERS):
        cw = np.asarray(inputs[f"conv_w_{fi}"], np.float32)
        wc[:w, :, off:off + n] = cw.transpose(2, 1, 0)
        off += n
    wconv = wc.reshape(KDIM, N_FILTERS).astype(ml_dtypes.bfloat16)
    bconv = np.concatenate([np.asarray(inputs[f"conv_b_{i}"], np.float32)
                            for i in range(7)])
    bconv_dev = bconv.reshape(16, 128).T.copy()

    # highway weights: fp8 W8 packed for DoubleRow streaming.
    whw8 = np.zeros((2, 16, 128, 8, 2, 256), np.float32)
    bhw = np.zeros((2, 128, 16, 2), np.float32)
    for l in range(2):
        W = np.asarray(inputs[f"hw_w_{l}"], np.float32)   # (4096, 2048)
        bb = np.asarray(inputs[f"hw_b_{l}"], np.float32)
        Ws = W * S_W
        W8 = Ws.astype(ml_dtypes.float8_e4m3).astype(np.float32)
        W8T = W8.T  # (2048 ic, 4096 oc)
        for j in range(16):
            for hf in range(2):
                oc0 = 2048 * hf + 128 * j
                for cc in range(8):
                    for g in range(2):
                        cb = 2 * cc + g
                        blk8 = W8T[128 * cb:128 * (cb + 1), oc0:oc0 + 128]
                        whw8[l, j, :, cc, g, 128 * hf:128 * hf + 128] = blk8
            bhw[l, :, j, 0] = bb[128 * j:128 * (j + 1)]
            bhw[l, :, j, 1] = bb[2048 + 128 * j:2048 + 128 * (j + 1)]
    whw8 = whw8.astype(ml_dtypes.float8_e4m3)

    Wp = np.asarray(inputs["proj_w"], np.float32) * S_W  # (512, 2048)
    Wp8 = Wp.astype(ml_dtypes.float8_e4m3).astype(np.float32)
    Wpl = (Wp - Wp8).astype(ml_dtypes.float8_e4m3).astype(np.float32)
    Wp8T = Wp8.T  # (2048, 512)
    WplT = Wpl.T
    wproj8 = np.zeros((128, 8, 2, 512), np.float32)
    wprojc8 = np.zeros((128, 8, 2, 512), np.float32)
    for cc in range(8):
        for g in range(2):
            cb = 2 * cc + g
            wproj8[:, cc, g, :] = Wp8T[128 * cb:128 * (cb + 1), :]
            wprojc8[:, cc, g, :] = WplT[128 * cb:128 * (cb + 1), :]
    wproj8 = wproj8.astype(ml_dtypes.float8_e4m3)
    wprojc8 = wprojc8.astype(ml_dtypes.float8_e4m3)
    bproj = np.zeros((128, 4), np.float32)
    bp = np.asarray(inputs["proj_b"], np.float32)
    for j2 in range(4):
        bproj[:, j2] = bp[128 * j2:128 * (j2 + 1)]

    ident = np.eye(128, dtype=np.float32)

    shared = dict(wconv=wconv, bconv=bconv_dev, whw=whw8,
                  bhw=bhw, wproj=wproj8, wprojc=wprojc8,
                  bproj=bproj, ident=ident)

    in_maps = []
    for core in range(N_CORES):
        cp = pairs[core * TOK:(core + 1) * TOK]            # [512, 25]
        uniq, inv = np.unique(cp, return_inverse=True)
        assert len(uniq) <= TABLE_ROWS, len(uniq)
        tbl = np.zeros((TABLE_ROWS, 128), np.float32)
        tbl[:len(uniq), 0:CHAR_DIM] = emb[uniq // CHAR_VOCAB]
        tbl[:len(uniq), CHAR_DIM:2 * CHAR_DIM] = emb[uniq % CHAR_VOCAB]
        idx_flat = inv.reshape(TOK, NPAIR).T.reshape(-1).astype(np.int16)
        idx16 = idx_flat.reshape(NI // 16, 16).T.copy()
        idx16 = np.tile(idx16, (8, 1))
        m = dict(shared)
        m["table"] = tbl.astype(ml_dtypes.bfloat16)
        m["idx"] = idx16
        in_maps.append(m)
    return in_maps


def kernel(**inputs) -> np.ndarray:
    if "nc" not in _CACHED:
        _CACHED["nc"] = build_module()
    nc = _CACHED["nc"]
    in_maps = _prep(inputs)
    res = run_bass_kernel_spmd(nc, in_maps, core_ids=list(range(N_CORES)))
    full = np.concatenate([r["out"] for r in res.results], axis=0)
    return full.reshape(B, S, PROJ_DIM)


# revision 44
# speedup vs baseline: 1.0686x; 1.0094x over previous
"""CharCNN token embedder (ELMo-style) on 8 Trainium2 NeuronCores.

Data-parallel over the 4096 = 16*256 tokens (512 per core). Weights replicated.

Per-core pipeline (v5):
  1. Char-PAIR gather: host packs each token's 50 chars into 25 pairs and
     builds a per-core table of unique pairs (~12k rows of 256B; cols 0:32
     hold both chars' embeddings) -> 12800 descriptors instead of 28672.
     Four pair-aligned gather chunks; shifted parity-strided SBUF copies
     (fused over contiguous tap pairs) build the K=112 im2col patch matrix;
     conv starts after two chunks.
  2. Tokens processed in two halves of 256: phase1 = conv(A); phase2 =
     conv(B) interleaved with highway+proj(A); phase3 = highway+proj(B).
  3. Conv = bf16 matmuls, K=112, one position per matmul, rounds of 4
     positions x 4 PSUM buffers (phase1) / 6 positions x 2 buffers
     (phase2, sharing PSUM with the highway accumulators). Max-pool
     drain per tile into acc[128, rpos//2 + 4, HALF] bf16:
       ~78% A-led rounds: ACT copies the PSUM round to a tmp stack; DVE
         folds it to half width and merges into acc (emitted one round
         late via a deferred-op queue so no in-order engine parks on a
         cross-engine dependency).
       ~22% D-led rounds: DVE reduce_max writes a spare acc slot directly
         (no merge op). Partial rounds are always D-led.
     Finale: in-place pairwise fold of acc + bias+relu on DVE; fp8 hi/lo
     casts batched per tile pair and deferred two rounds (ACT never waits
     the DVE finale chain). Only ACT and DVE can touch PSUM and the Pool
     engine cannot run TensorTensor on TRN2, so Pool only runs the gather.
  4. Highway layers in fp8 DoubleRow at 2x bf16 throughput: per (layer, j),
     psum = W8(h_hi) + W8(h_lo), scaled e4m3 (S_W=512, S_H=32), descale via
     the ACT sigmoid/relu scale; gating on DVE bf16. PE chains emit
     immediately, ACT/DVE epilogues lag one j. h8 layout [128, hi/lo, j, n]
     batches (j, j+1) casts; the final-layer fp8 buffer aliases the conv
     one (layer-0 chains consume it before layer-1 writes).
  5. Projection fp8 DoubleRow (hi, lo, W-residual chains); PE-transpose;
     ACT bounce to SBUF; DMA out. Projection weight loads are WAW-gated
     behind a late DVE memset so they cannot steal the DMA device from the
     gather at t=0.
"""

import numpy as np
import ml_dtypes

import concourse.bass as bass
import concourse.mybir as mybir
import concourse.tile as tile
from concourse import bacc
from concourse.bass_utils import run_bass_kernel_spmd
from concourse.vector_clock import ScopedClock

# ---------------------------------------------------------------- constants
B, S, L = 16, 256, 50
CHAR_DIM = 16
CHAR_VOCAB = 262
FILTERS = [(1, 32), (2, 32), (3, 64), (4, 128), (5, 256), (6, 512), (7, 1024)]
N_FILTERS = 2048
PROJ_DIM = 512
N_CORES = 8
NTOK = B * S
TOK = NTOK // N_CORES        # 512 tokens per core
NPOS = 50
NPAIR = 25                   # char pairs per token
NPPAD = 28                   # padded pair positions (tap reach 55 -> pair 27)
NI = TOK * NPAIR             # 12800 gather indices per core
TABLE_ROWS = 32768           # fixed per-core unique-pair table allocation
KDIM = 112                   # 7 taps * 16 dims
TSPLIT = 22                  # conv positions < TSPLIT read xsA

S_W = 512.0                  # fp8 storage scale for highway/proj weights
S_H = 32.0                   # fp8 storage scale for highway/proj activations
DESCALE = 1.0 / (S_W * S_H)

# per 128-channel tile: valid positions; tile 0 packs w=1,2,3 with tails
CH_TILES = []
CH_TILES.append({"t_main": 48, "tails": [(0, 32, 50), (32, 64, 49), (64, 128, 48)]})
CH_TILES.append({"t_main": 47, "tails": [(0, 128, 47)]})      # w4
for _ in range(2):
    CH_TILES.append({"t_main": 46, "tails": [(0, 128, 46)]})  # w5
for _ in range(4):
    CH_TILES.append({"t_main": 45, "tails": [(0, 128, 45)]})  # w6
for _ in range(8):
    CH_TILES.append({"t_main": 44, "tails": [(0, 128, 44)]})  # w7

BF16 = mybir.dt.bfloat16
FP32 = mybir.dt.float32
FP8 = mybir.dt.float8e4

# drain schedule fractions: (pA, pPoolMerge, pPoolFinale) per phase
PH1 = (0.78, 0.0, 0.0)
PH2 = (0.75, 0.0, 0.0)

_MAX_WAITS_PER_INST = 1


def _patched_drain_and_barrier(self, tick_clock, wait_clock):
    # The walrus build in this container rejects CTRL instructions carrying
    # more than one sem wait; spread the kernel-tail drain waits over NOPs.
    nc = self.nc
    carrier = nc.sync.nop()
    wait_clock.add_sem_waits(carrier.ins, ScopedClock({None: tick_clock.global_clock}))
    si = carrier.ins.sync_info
    waits = list(si.on_wait) if si is not None and si.on_wait else []
    if len(waits) > _MAX_WAITS_PER_INST:
        carrier.ins.sync_info = mybir.SyncInfo(
            on_wait=waits[:_MAX_WAITS_PER_INST],
            on_update=list(si.on_update) if si.on_update else [])
        for i in range(_MAX_WAITS_PER_INST, len(waits), _MAX_WAITS_PER_INST):
            extra = nc.sync.nop()
            extra.ins.sync_info = mybir.SyncInfo(
                on_wait=waits[i:i + _MAX_WAITS_PER_INST], on_update=[])
    nc.sync.drain()
    nc.all_engine_barrier()
    assert self.sems is not None
    popped = nc._tile_sem_poison_stack.pop()
    assert popped is self._sem_poison
    nc.clear_and_free_semaphores(list(self.sems.allocated().values()))
    nc.all_engine_barrier()


tile.TileContext._drain_and_barrier = _patched_drain_and_barrier


class DrainSched:
    """Debt-based allocator: round kind (A/D), fold + merge engines."""

    def __init__(self, nc, pA, unused0=0.0, unused1=0.0):
        self.nc = nc
        self.pA = pA
        self.da = 0.0

    def kind(self):
        self.da += self.pA
        if self.da >= 1.0:
            self.da -= 1.0
            return "A"
        return "D"

    def note_forced(self, k):
        pass

    def fold_eng(self):
        return self.nc.vector


# ---------------------------------------------------------------- device IR
def build_module():
    nc = bacc.Bacc()
    SIdx = NI // 16

    # 256-byte rows (dma_gather granularity); cols 0:32 hold the pair embs
    table = nc.dram_tensor("table", [TABLE_ROWS, 128], BF16,
                           kind="ExternalInput")
    idx = nc.dram_tensor("idx", [128, SIdx], mybir.dt.int16, kind="ExternalInput")
    wconv = nc.dram_tensor("wconv", [KDIM, N_FILTERS], BF16, kind="ExternalInput")
    bconv = nc.dram_tensor("bconv", [128, 16], FP32, kind="ExternalInput")
    # highway weights fp8, host-packed per (layer, j):
    #   [l, j, p(128), cc(8), g(2), half*128+oc] ; g = DoubleRow group
    whw = nc.dram_tensor("whw", [2, 16, 128, 8, 2, 256], FP8, kind="ExternalInput")
    bhw = nc.dram_tensor("bhw", [2, 128, 16, 2], FP32, kind="ExternalInput")
    wproj = nc.dram_tensor("wproj", [128, 8, 2, 512], FP8, kind="ExternalInput")
    wprojc = nc.dram_tensor("wprojc", [128, 8, 2, 512], FP8, kind="ExternalInput")
    bproj = nc.dram_tensor("bproj", [128, 4], FP32, kind="ExternalInput")
    ident = nc.dram_tensor("ident", [128, 128], FP32, kind="ExternalInput")
    out = nc.dram_tensor("out", [TOK, PROJ_DIM], FP32, kind="ExternalOutput")

    with tile.TileContext(nc) as tc:
        with (
            tc.tile_pool(name="xs", bufs=1) as xspool,
            tc.tile_pool(name="consts", bufs=1) as cpool,
        ):
            # ---- constants in (wconv loads after idx; the rest is deferred
            # so nothing delays the gather + xsA stream on the DMA device)
            wconv_t = cpool.tile([KDIM, N_FILTERS], BF16)
            early_t = cpool.tile([128, 16, NTOK // N_CORES // 2], BF16)
            bconv_t = cpool.tile([128, 16], FP32)
            bhw_t = cpool.tile([128, 2, 16, 2], FP32)
            bproj_t = cpool.tile([128, 4], FP32)
            ident_t = cpool.tile([128, 128], FP32)

            # ---- 1. pair-gather char embeddings + build K=112 patch matrix.
            # xsA/xsB viewed [128, m, parity, TOK] so the strided parity
            # interleave is a plain AP (no step slicing).
            NA = TSPLIT // 2          # 11 position pairs in xsA
            NB = (NPOS - TSPLIT) // 2  # 14 in xsB
            xsA = xspool.tile([KDIM, NA, 2, TOK], BF16, name="xsA")
            xsB = xspool.tile([KDIM, NB, 2, TOK], BF16, name="xsB")
            with tc.tile_pool(name="gather", bufs=1) as gpool:
                idx_t = gpool.tile([128, SIdx], mybir.dt.int16)
                nc.sync.dma_start(out=idx_t[:], in_=idx[:])
                nc.sync.dma_start(out=wconv_t[:], in_=wconv[:])
                xg = gpool.tile([128, 1, TOK * NPPAD], BF16)
                nc.vector.memset(xg[0:32, 0, NI:TOK * NPPAD], 0.0)
                # pair-aligned chunks (idx counts): 5,9,7,4 pairs
                chunks = [(0, 2560), (2560, 4608), (7168, 3584), (10752, 2048)]
                for r, (o, cn) in enumerate(chunks):
                    nc.gpsimd.dma_gather(
                        out_ap=xg[:, :, o:o + cn],
                        in_ap=table[:],
                        idxs_ap=idx_t[:, o // 16:(o + cn) // 16],
                        num_idxs=cn,
                        num_idxs_reg=cn,
                        elem_size=128,
                        transpose=True,
                        single_packet=False,
                    )
                    if r > 1:
                        continue
                    # xsA copies, fused over (k, k+1) pairs with contiguous
                    # source partition blocks; split at the chunk-0 boundary
                    # (pair 4) so the early conv rounds start after chunk 0
                    for s in range(2):
                        k = 0
                        while k < 7:
                            p0 = (s + k) // 2
                            par = (s + k) % 2
                            m_lo = 0 if r == 0 else max(0, 5 - p0)
                            m_hi = min(NA, 5 - p0) if r == 0 else NA
                            fuse = par == 0 and k + 1 < 7
                            if m_hi > m_lo:
                                if fuse:
                                    nc.sync.dma_start(
                                        out=xsA[16 * k:16 * (k + 2),
                                                m_lo:m_hi, s, :],
                                        in_=xg[0:32, 0,
                                               TOK * (p0 + m_lo):
                                               TOK * (p0 + m_hi)],
                                    )
                                else:
                                    nc.sync.dma_start(
                                        out=xsA[16 * k:16 * (k + 1),
                                                m_lo:m_hi, s, :],
                                        in_=xg[16 * par:16 * par + 16, 0,
                                               TOK * (p0 + m_lo):
                                               TOK * (p0 + m_hi)],
                                    )
                            k += 2 if fuse else 1
                for s in range(2):
                    k = 0
                    while k < 7:
                        t0 = TSPLIT + s
                        p0 = (t0 + k) // 2
                        par = (t0 + k) % 2
                        if par == 0 and k + 1 < 7:
                            nc.sync.dma_start(
                                out=xsB[16 * k:16 * (k + 2), :, s, :],
                                in_=xg[0:32, 0, TOK * p0:TOK * (p0 + NB)],
                            )
                            k += 2
                        else:
                            nc.sync.dma_start(
                                out=xsB[16 * k:16 * (k + 1), :, s, :],
                                in_=xg[16 * par:16 * par + 16, 0,
                                       TOK * p0:TOK * (p0 + NB)],
                            )
                            k += 1
            nc.sync.dma_start(out=bconv_t[:], in_=bconv[:])
            nc.sync.dma_start(out=bhw_t[:], in_=bhw[:].rearrange("l p j h -> p l j h"))
            nc.sync.dma_start(out=bproj_t[:], in_=bproj[:])
            nc.sync.dma_start(out=ident_t[:], in_=ident[:])
            stackw = tc.tile_pool(name="wppool", bufs=1)
            wppool = stackw.__enter__()
            wproj_t = wppool.tile([128, 8, 2, 512], FP8)
            wprojc_t = wppool.tile([128, 8, 2, 512], FP8)

            HALF = TOK // 2

            def conv_rhs(t, hlo):
                if t < TSPLIT:
                    return xsA[:, t // 2, t % 2, hlo:hlo + HALF]
                tl = t - TSPLIT
                return xsB[:, tl // 2, tl % 2, hlo:hlo + HALF]

            stack = tc.tile_pool(name="hbuf", bufs=1)
            hpool = stack.__enter__()
            stack2 = tc.tile_pool(name="h8buf", bufs=1)
            h8pool = stack2.__enter__()
            stack3 = tc.tile_pool(name="wstream", bufs=3)
            wpool = stack3.__enter__()
            stack4 = tc.tile_pool(name="small", bufs=2)
            spool = stack4.__enter__()
            stack6 = tc.tile_pool(name="accpool", bufs=6)
            accpool = stack6.__enter__()
            convp = None  # phase-2/3 PSUM pool, opened after phase 1

            # per-half persistent tensors (separate tiles avoid false deps)
            h1 = [hpool.tile([128, 16, HALF], BF16, tag=f"h1{s_}", name=f"h1{s_}")
                  for s_ in range(2)]
            hmid = [hpool.tile([128, 16, HALF], BF16, tag=f"hm{s_}", name=f"hm{s_}")
                    for s_ in range(2)]
            # h8 layout: [128, hi/lo, j, n] so (j, j+1) casts batch into one op
            h8c = [h8pool.tile([128, 2, 16, HALF], FP8, tag=f"h8c{s_}", name=f"h8c{s_}")
                   for s_ in range(2)]
            h8m = [h8pool.tile([128, 2, 16, HALF], FP8, tag=f"h8m{s_}", name=f"h8m{s_}")
                   for s_ in range(2)]
            # h8f aliases h8c: layer-0 chains fully consume h8c before
            # layer-1 writes the final activations (WAR handled by deps)
            h8f = h8c

            def cast_pair(h_bf, j0, nj, h8, lo_eng=None):
                # hi = fp8(h * S_H) on ACT; lo = fp8(h*S_H - hi) on DVE/Pool
                nc.scalar.activation(
                    out=h8[:, 0, j0:j0 + nj, :], in_=h_bf[:, j0:j0 + nj, :],
                    func=mybir.ActivationFunctionType.Copy, scale=S_H)
                (lo_eng or nc.vector).scalar_tensor_tensor(
                    out=h8[:, 1, j0:j0 + nj, :], in0=h_bf[:, j0:j0 + nj, :],
                    scalar=S_H, in1=h8[:, 0, j0:j0 + nj, :],
                    op0=mybir.AluOpType.mult, op1=mybir.AluOpType.subtract)

            def conv_half(hf, rpos, pool, sched, early=False):
                """Generator: conv + max-pool drain for token half hf.

                Touches (PSUM reads) emit immediately; folds/merges/finales
                emit one round late via `pending` so no engine head-of-line
                blocks on a cross-engine dependency that is not ready yet.
                """
                from collections import deque
                hlo = HALF * hf
                hw_ = rpos // 2  # acc width
                pending = deque()
                pcast = deque()   # fp8 casts lag one extra round so the ACT
                                  # hi-cast never parks waiting the DVE finale

                def flush(keep):
                    while len(pending) > keep:
                        pending.popleft()()
                    while len(pcast) > max(keep, 1) + 1:
                        pcast.popleft()()

                if early:
                    # warm-up: positions 0..3 of every tile only need the
                    # first gather chunk -> ACT/DVE start ~20us earlier
                    for i, spec in enumerate(CH_TILES):
                        lhsT = wconv_t[:, 128 * i:128 * (i + 1)]
                        P = pool.tile([128, rpos, HALF], FP32, tag=f"ps{rpos}")
                        for r in range(4):
                            nc.tensor.matmul(
                                out=P[:, r, :], lhsT=lhsT,
                                rhs=conv_rhs(r, hlo),
                                start=True, stop=True)
                        flush(1)
                        tmp = spool.tile([128, rpos, HALF], BF16,
                                         tag="astk", bufs=4)
                        nc.scalar.activation(
                            out=tmp[:, 0:4, :], in_=P[:, 0:4, :],
                            func=mybir.ActivationFunctionType.Copy, scale=1.0)

                        def eop(tmp=tmp, i=i):
                            t2e = spool.tile([128, 2, HALF], BF16, tag="f2e")
                            nc.vector.tensor_tensor(
                                out=t2e[:], in0=tmp[:, 0:2, :],
                                in1=tmp[:, 2:4, :], op=mybir.AluOpType.max)
                            nc.vector.tensor_tensor(
                                out=early_t[:, i, :], in0=t2e[:, 0, :],
                                in1=t2e[:, 1, :], op=mybir.AluOpType.max)
                        pending.append(eop)
                DX = 4  # spare direct-write slots for D-round reduces
                for i, spec in enumerate(CH_TILES):
                    lhsT = wconv_t[:, 128 * i:128 * (i + 1)]
                    t_main = spec["t_main"]
                    acc = accpool.tile([128, hw_ + DX, HALF], BF16, tag="acc")
                    first = True
                    dstate = {"next": hw_}
                    t0 = 4 if early else 0
                    while t0 < t_main:
                        nt = min(rpos, t_main - t0)
                        P = pool.tile([128, rpos, HALF], FP32, tag=f"ps{rpos}")
                        for r in range(nt):
                            nc.tensor.matmul(
                                out=P[:, r, :], lhsT=lhsT,
                                rhs=conv_rhs(t0 + r, hlo),
                                start=True, stop=True)
                        flush(1)
                        if nt == rpos and (first or sched.kind() == "A"):
                            if first:
                                sched.note_forced("A")
                            tmp = spool.tile([128, rpos, HALF], BF16,
                                             tag="astk", bufs=4)
                            nc.scalar.activation(
                                out=tmp[:], in_=P[:],
                                func=mybir.ActivationFunctionType.Copy, scale=1.0)
                            eng = sched.fold_eng()
                            if first:
                                def op(eng=eng, tmp=tmp, acc=acc):
                                    eng.tensor_tensor(
                                        out=acc[:, 0:hw_, :],
                                        in0=tmp[:, 0:hw_, :],
                                        in1=tmp[:, hw_:rpos, :],
                                        op=mybir.AluOpType.max)
                                first = False
                            else:
                                def op(eng=eng, tmp=tmp, acc=acc):
                                    fh = spool.tile([128, hw_, HALF], BF16,
                                                    tag="fh", bufs=3)
                                    eng.tensor_tensor(
                                        out=fh[:], in0=tmp[:, 0:hw_, :],
                                        in1=tmp[:, hw_:rpos, :],
                                        op=mybir.AluOpType.max)
                                    eng.tensor_tensor(
                                        out=acc[:, 0:hw_, :],
                                        in0=acc[:, 0:hw_, :], in1=fh[:],
                                        op=mybir.AluOpType.max)
                            pending.append(op)
                        elif nt == 1:
                            nc.vector.tensor_tensor(
                                out=acc[:, 0, :], in0=acc[:, 0, :],
                                in1=P[:, 0, :], op=mybir.AluOpType.max)
                        elif dstate["next"] < hw_ + DX:
                            # D-led: reduce straight into a spare acc slot
                            nc.vector.reduce_max(
                                out=acc[:, dstate["next"], :],
                                in_=P[:, 0:nt, :].rearrange("p t n -> p n t"),
                                axis=mybir.AxisListType.X)
                            dstate["next"] += 1
                        else:
                            part = spool.tile([128, HALF], BF16, tag="part",
                                              bufs=3)
                            nc.vector.reduce_max(
                                out=part[:],
                                in_=P[:, 0:nt, :].rearrange("p t n -> p n t"),
                                axis=mybir.AxisListType.X)

                            def op(part=part, acc=acc):
                                nc.vector.tensor_tensor(
                                    out=acc[:, 0, :], in0=acc[:, 0, :],
                                    in1=part[:], op=mybir.AluOpType.max)
                            pending.append(op)
                        t0 += nt
                    # ragged tails (tile 0): positions t_main..50 on partition
                    # subranges; reduces touch PSUM now, merges deferred
                    if spec["tails"][0][2] > t_main:
                        nt = spec["tails"][0][2] - t_main
                        P = pool.tile([128, rpos, HALF], FP32, tag=f"ps{rpos}")
                        for r in range(nt):
                            nc.tensor.matmul(
                                out=P[:, r, :], lhsT=lhsT,
                                rhs=conv_rhs(t_main + r, hlo),
                                start=True, stop=True)
                        for (lo, hi, g_cnt) in spec["tails"]:
                            g_nt = g_cnt - t_main
                            if g_nt <= 0:
                                continue
                            if g_nt == 1:
                                nc.vector.tensor_tensor(
                                    out=acc[lo:hi, 0, :], in0=acc[lo:hi, 0, :],
                                    in1=P[lo:hi, 0, :], op=mybir.AluOpType.max)
                            else:
                                part = spool.tile([128, HALF], BF16, tag="part",
                                                  bufs=3)
                                nc.vector.reduce_max(
                                    out=part[lo:hi, :],
                                    in_=P[lo:hi, 0:g_nt, :].rearrange(
                                        "p t n -> p n t"),
                                    axis=mybir.AxisListType.X)

                                def op(part=part, acc=acc, lo=lo, hi=hi):
                                    nc.vector.tensor_tensor(
                                        out=acc[lo:hi, 1, :],
                                        in0=acc[lo:hi, 1, :],
                                        in1=part[lo:hi, :],
                                        op=mybir.AluOpType.max)
                                pending.append(op)

                    used = dstate["next"]

                    def finale(i=i, acc=acc, used=used):
                        pre = spool.tile([128, HALF], BF16, tag="pre")
                        cw = used
                        while cw > 2:
                            if cw % 2:
                                nc.vector.tensor_tensor(
                                    out=acc[:, 0, :], in0=acc[:, 0, :],
                                    in1=acc[:, cw - 1, :],
                                    op=mybir.AluOpType.max)
                                cw -= 1
                            h = cw // 2
                            nc.vector.tensor_tensor(
                                out=acc[:, 0:h, :], in0=acc[:, 0:h, :],
                                in1=acc[:, h:cw, :], op=mybir.AluOpType.max)
                            cw = h
                        nc.vector.tensor_tensor(
                            out=pre[:], in0=acc[:, 0, :], in1=acc[:, 1, :],
                            op=mybir.AluOpType.max)
                        if early:
                            nc.vector.tensor_tensor(
                                out=pre[:], in0=pre[:],
                                in1=early_t[:, i, :], op=mybir.AluOpType.max)
                        nc.vector.tensor_scalar(
                            out=h1[hf][:, i, :], in0=pre[:],
                            scalar1=bconv_t[:, i:i + 1], scalar2=0.0,
                            op0=mybir.AluOpType.add, op1=mybir.AluOpType.max)
                    pending.append(finale)
                    if i % 2 == 1:
                        def cst(i=i):
                            cast_pair(h1[hf], i - 1, 2, h8c[hf])
                        pcast.append(cst)
                    yield
                flush(0)
                while pcast:
                    pcast.popleft()()

            def hw_mm_chain(p_out, wslab, h8, ofs, lo=True):
                # W8 x (h_hi [+ h_lo]); Wl correction skipped for the highway.
                # The sigmoid gate path also skips the h_lo chain (the gate
                # damps the quantization error; verified within tolerance).
                nhl = 2 if lo else 1
                for hl in range(nhl):
                    for cc in range(8):
                        nc.tensor.matmul(
                            out=p_out, lhsT=wslab[:, cc, :, ofs:ofs + 128],
                            rhs=h8[:, hl, 2 * cc:2 * cc + 2, :],
                            start=(hl == 0 and cc == 0),
                            stop=(hl == nhl - 1 and cc == 7),
                            perf_mode=mybir.MatmulPerfMode.DoubleRow)

            def hw_mm_chain_proj(p_out, h8, ofs):
                for hl in range(2):
                    for cc in range(8):
                        nc.tensor.matmul(
                            out=p_out, lhsT=wproj_t[:, cc, :, ofs:ofs + 128],
                            rhs=h8[:, hl, 2 * cc:2 * cc + 2, :],
                            start=(hl == 0 and cc == 0), stop=False,
                            perf_mode=mybir.MatmulPerfMode.DoubleRow)
                for cc in range(8):
                    nc.tensor.matmul(
                        out=p_out, lhsT=wprojc_t[:, cc, :, ofs:ofs + 128],
                        rhs=h8[:, 0, 2 * cc:2 * cc + 2, :],
                        start=False, stop=(cc == 7),
                        perf_mode=mybir.MatmulPerfMode.DoubleRow)

            def hw_half(hf):
                """Generator: highway l0+l1 + proj for token half hf.

                PE chains emit immediately; ACT/DVE epilogues lag one j so
                neither engine parks at its queue head waiting on a chain.
                """
                from collections import deque
                pending = deque()

                def flush(keep):
                    while len(pending) > keep:
                        pending.popleft()()

                state = {}
                for layer in range(2):
                    h_in = h1[hf] if layer == 0 else hmid[hf]
                    h8_in = h8c[hf] if layer == 0 else h8m[hf]
                    h8_out = h8m[hf] if layer == 0 else h8f[hf]
                    for j in range(16):
                        wslab = wpool.tile([128, 8, 2, 256], FP8, tag="wslab")
                        nc.sync.dma_start(out=wslab[:], in_=whw[layer, j])
                        hp = convp.tile([128, 2, HALF], FP32, tag="hwps",
                                        name="hp", bufs=2)
                        p_nl = hp[:, 0, :]
                        p_g = hp[:, 1, :]
                        hw_mm_chain(p_nl, wslab, h8_in, 0)
                        hw_mm_chain(p_g, wslab, h8_in, 128, lo=False)
                        flush(1)

                        def epi(layer=layer, j=j, p_nl=p_nl, p_g=p_g,
                                h_in=h_in, h8_out=h8_out):
                            nl = spool.tile([128, HALF], BF16, tag="nl")
                            gt = spool.tile([128, HALF], BF16, tag="gt")
                            nc.scalar.activation(
                                out=nl[:], in_=p_nl,
                                func=mybir.ActivationFunctionType.Relu,
                                bias=bhw_t[:, layer, j, 0:1], scale=DESCALE)
                            nc.scalar.activation(
                                out=gt[:], in_=p_g,
                                func=mybir.ActivationFunctionType.Sigmoid,
                                bias=bhw_t[:, layer, j, 1:2], scale=DESCALE)
                            d = spool.tile([128, HALF], BF16, tag="d")
                            nc.vector.tensor_tensor(
                                out=d[:], in0=h_in[:, j, :], in1=nl[:],
                                op=mybir.AluOpType.subtract)
                            m = spool.tile([128, HALF], BF16, tag="m")
                            nc.vector.tensor_mul(out=m[:], in0=gt[:], in1=d[:])
                            if layer == 0:
                                nc.vector.tensor_add(
                                    out=hmid[hf][:, j, :], in0=nl[:], in1=m[:])
                                if j % 2 == 1:
                                    cast_pair(hmid[hf], j - 1, 2, h8_out)
                            else:
                                if j % 2 == 0:
                                    state["htp"] = spool.tile(
                                        [128, 2, HALF], BF16, tag="htp",
                                        name="htp")
                                htp = state["htp"]
                                nc.vector.tensor_add(
                                    out=htp[:, j % 2, :], in0=nl[:], in1=m[:])
                                if j % 2 == 1:
                                    nc.scalar.activation(
                                        out=h8_out[:, 0, j - 1:j + 1, :],
                                        in_=htp[:],
                                        func=mybir.ActivationFunctionType.Copy,
                                        scale=S_H)
                                    nc.vector.scalar_tensor_tensor(
                                        out=h8_out[:, 1, j - 1:j + 1, :],
                                        in0=htp[:], scalar=S_H,
                                        in1=h8_out[:, 0, j - 1:j + 1, :],
                                        op0=mybir.AluOpType.mult,
                                        op1=mybir.AluOpType.subtract)
                        pending.append(epi)
                        yield
                    # layer barrier: next layer's chains read every h8 column
                    flush(0)
                # projection + transpose + out for this half
                hlo = HALF * hf
                for j2 in range(4):
                    hp = convp.tile([128, 2, HALF], FP32, tag="hwps",
                                    name="hp", bufs=2)
                    p_o = hp[:, 0, :]
                    hw_mm_chain_proj(p_o, h8f[hf], 128 * j2)
                    flush(1)

                    def proj_epi(j2=j2, hp=hp, p_o=p_o):
                        ot = spool.tile([128, HALF], FP32, tag="ot")
                        nc.scalar.activation(
                            out=ot[:], in_=p_o,
                            func=mybir.ActivationFunctionType.Identity,
                            bias=bproj_t[:, j2:j2 + 1], scale=DESCALE)
                        for m4 in range(2):
                            p_t = hp[:, 1, 128 * m4:128 * (m4 + 1)]
                            nc.tensor.transpose(
                                out=p_t, in_=ot[:, 128 * m4:128 * (m4 + 1)],
                                identity=ident_t[:])
                            ob = spool.tile([128, 128], FP32, tag="ob")
                            nc.scalar.copy(out=ob[:], in_=p_t)
                            row0 = hlo + 128 * m4
                            nc.sync.dma_start(
                                out=out[row0:row0 + 128,
                                        128 * j2:128 * (j2 + 1)],
                                in_=ob[:])
                    pending.append(proj_epi)
                    yield
                flush(0)

            # ---- phase 1: conv half A, 8-position rounds, all 8 PSUM banks
            sched1 = DrainSched(nc, *PH1)
            with tc.tile_pool(name="convp8", bufs=4, space="PSUM") as p8pool:
                for _ in conv_half(0, 4, p8pool, sched1, early=True):
                    pass
            stack5 = tc.tile_pool(name="convp", bufs=2, space="PSUM")
            convp = stack5.__enter__()
            # WAW-gate the projection-weight loads behind a DVE op that sits
            # late in DVE program order, so they cannot steal the DMA device
            # from the gather at t=0 (the sim schedules by readiness)
            nc.vector.memset(wproj_t[0:1, 0:1, 0:1, 0:1], 0.0)
            nc.vector.memset(wprojc_t[0:1, 0:1, 0:1, 0:1], 0.0)
            nc.sync.dma_start(out=wproj_t[:], in_=wproj[:])
            nc.sync.dma_start(out=wprojc_t[:], in_=wprojc[:])
            # ---- phase 2: conv half B interleaved with highway+proj half A
            sched2 = DrainSched(nc, *PH2)
            genB = conv_half(1, 6, convp, sched2)
            genA = hw_half(0)
            unitsB, unitsA = 16, 36
            credit = 0.0
            doneB = doneA = False
            while not (doneB and doneA):
                credit += unitsA / unitsB
                if not doneB:
                    doneB = next(genB, "end") == "end"
                while credit >= 1.0 and not doneA:
                    doneA = next(genA, "end") == "end"
                    credit -= 1.0
                if doneB:
                    while not doneA:
                        doneA = next(genA, "end") == "end"
            # ---- phase 3: highway+proj half B
            for _ in hw_half(1):
                pass

            for st in (stack5, stack6, stack4, stack3, stack2, stack, stackw):
                st.__exit__(None, None, None)

    nc.compile()
    return nc


_CACHED = {}


def _prep(inputs):
    """Host-side layout prep: sharding, pair tables, weight packing."""
    chars = np.asarray(inputs["chars"]).astype(np.int64).reshape(NTOK, L)
    pairs = chars[:, 0::2] * CHAR_VOCAB + chars[:, 1::2]   # [NTOK, 25]

    emb = np.asarray(inputs["char_emb"], np.float32)

    wc = np.zeros((7, CHAR_DIM, N_FILTERS), np.float32)
    off = 0
    for fi, (w, n) in enumerate(FILTERS):
        cw = np.asarray(inputs[f"conv_w_{fi}"], np.float32)
        wc[:w, :, off:off + n] = cw.transpose(2, 1, 0)
        off += n
    wconv = wc.reshape(KDIM, N_FILTERS).astype(ml_dtypes.bfloat16)
    bconv = np.concatenate([np.asarray(inputs[f"conv_b_{i}"], np.float32)
                            for i in range(7)])
    bconv_dev = bconv.reshape(16, 128).T.copy()

    # highway weights: fp8 W8 packed for DoubleRow streaming.
    whw8 = np.zeros((2, 16, 128, 8, 2, 256), np.float32)
    bhw = np.zeros((2, 128, 16, 2), np.float32)
    for l in range(2):
        W = np.asarray(inputs[f"hw_w_{l}"], np.float32)   # (4096, 2048)
        bb = np.asarray(inputs[f"hw_b_{l}"], np.float32)
        Ws = W * S_W
        W8 = Ws.astype(ml_dtypes.float8_e4m3).astype(np.float32)
        W8T = W8.T  # (2048 ic, 4096 oc)
        for j in range(16):
            for hf in range(2):
                oc0 = 2048 * hf + 128 * j
                for cc in range(8):
                    for g in range(2):
                        cb = 2 * cc + g
                        blk8 = W8T[128 * cb:128 * (cb + 1), oc0:oc0 + 128]
                        whw8[l, j, :, cc, g, 128 * hf:128 * hf + 128] = blk8
            bhw[l, :, j, 0] = bb[128 * j:128 * (j + 1)]
            bhw[l, :, j, 1] = bb[2048 + 128 * j:2048 + 128 * (j + 1)]
    whw8 = whw8.astype(ml_dtypes.float8_e4m3)

    Wp = np.asarray(inputs["proj_w"], np.float32) * S_W  # (512, 2048)
    Wp8 = Wp.astype(ml_dtypes.float8_e4m3).astype(np.float32)
    Wpl = (Wp - Wp8).astype(ml_dtypes.float8_e4m3).astype(np.float32)
    Wp8T = Wp8.T  # (2048, 512)
    WplT = Wpl.T
    wproj8 = np.zeros((128, 8, 2, 512), np.float32)
    wprojc8 = np.zeros((128, 8, 2, 512), np.float32)
    for cc in range(8):
        for g in range(2):
            cb = 2 * cc + g
            wproj8[:, cc, g, :] = Wp8T[128 * cb:128 * (cb + 1), :]
            wprojc8[:, cc, g, :] = WplT[128 * cb:128 * (cb + 1), :]
    wproj8 = wproj8.astype(ml_dtypes.float8_e4m3)
    wprojc8 = wprojc8.astype(ml_dtypes.float8_e4m3)
    bproj = np.zeros((128, 4), np.float32)
    bp = np.asarray(inputs["proj_b"], np.float32)
    for j2 in range(4):
        bproj[:, j2] = bp[128 * j2:128 * (j2 + 1)]

    ident = np.eye(128, dtype=np.float32)

    shared = dict(wconv=wconv, bconv=bconv_dev, whw=whw8,
                  bhw=bhw, wproj=wproj8, wprojc=wprojc8,
                  bproj=bproj, ident=ident)

    in_maps = []
    for core in range(N_CORES):
        cp = pairs[core * TOK:(core + 1) * TOK]            # [512, 25]
        uniq, inv = np.unique(cp, return_inverse=True)
        assert len(uniq) <= TABLE_ROWS, len(uniq)
        tbl = np.zeros((TABLE_ROWS, 128), np.float32)
        tbl[:len(uniq), 0:CHAR_DIM] = emb[uniq // CHAR_VOCAB]
        tbl[:len(uniq), CHAR_DIM:2 * CHAR_DIM] = emb[uniq % CHAR_VOCAB]
        idx_flat = inv.reshape(TOK, NPAIR).T.reshape(-1).astype(np.int16)
        idx16 = idx_flat.reshape(NI // 16, 16).T.copy()
        idx16 = np.tile(idx16, (8, 1))
        m = dict(shared)
        m["table"] = tbl.astype(ml_dtypes.bfloat16)
        m["idx"] = idx16
        in_maps.append(m)
    return in_maps


def kernel(**inputs) -> np.ndarray:
    if "nc" not in _CACHED:
        _CACHED["nc"] = build_module()
    nc = _CACHED["nc"]
    in_maps = _prep(inputs)
    res = run_bass_kernel_spmd(nc, in_maps, core_ids=list(range(N_CORES)))
    full = np.concatenate([r["out"] for r in res.results], axis=0)
    return full.reshape(B, S, PROJ_DIM)


# revision 54
# speedup vs baseline: 1.0801x; 1.0108x over previous
"""CharCNN token embedder (ELMo-style) on 8 Trainium2 NeuronCores.

Data-parallel over the 4096 = 16*256 tokens (512 per core). Weights replicated.

Per-core pipeline (v5):
  1. Char-PAIR gather: host packs each token's 50 chars into 25 pairs and
     builds a per-core table of unique pairs (~12k rows of 256B; cols 0:32
     hold both chars' embeddings) -> 12800 descriptors instead of 28672.
     Four pair-aligned gather chunks; shifted parity-strided SBUF copies
     (fused over contiguous tap pairs) build the K=112 im2col patch matrix;
     conv starts after two chunks.
  2. Tokens processed in two halves of 256: phase1 = conv(A); phase2 =
     conv(B) interleaved with highway+proj(A); phase3 = highway+proj(B).
     Phase1 opens with a warm-up pass (positions 0..3 of every tile, which
     need only the first gather chunk) so the drain engines start ~20us
     before the full patch matrix lands; warm-up maxes fold into a small
     per-tile buffer merged at each tile's finale.
  3. Conv = bf16 matmuls, K=112, one position per matmul, rounds of 4
     positions x 4 PSUM buffers (phase1) / 6 positions x 2 buffers
     (phase2, sharing PSUM with the highway accumulators). Max-pool
     drain per tile into acc[128, rpos//2 + 4, HALF] bf16:
       ~78% A-led rounds: ACT copies the PSUM round to a tmp stack; DVE
         folds it to half width and merges into acc (emitted one round
         late via a deferred-op queue so no in-order engine parks on a
         cross-engine dependency).
       ~22% D-led rounds: DVE reduce_max writes a spare acc slot directly
         (no merge op). Partial rounds are always D-led.
     Finale: in-place pairwise fold of acc + bias+relu on DVE; fp8 hi/lo
     casts batched per tile pair and deferred two rounds (ACT never waits
     the DVE finale chain). Only ACT and DVE can touch PSUM and the Pool
     engine cannot run TensorTensor on TRN2, so Pool only runs the gather.
  4. Highway layers in fp8 DoubleRow at 2x bf16 throughput: per (layer, j),
     psum = W8(h_hi) + W8(h_lo), scaled e4m3 (S_W=512, S_H=32), descale via
     the ACT sigmoid/relu scale; gating on DVE bf16. PE chains emit
     immediately, ACT/DVE epilogues lag one j. h8 layout [128, hi/lo, j, n]
     batches (j, j+1) casts; the final-layer fp8 buffer aliases the conv
     one (layer-0 chains consume it before layer-1 writes).
  5. Projection fp8 DoubleRow (hi, lo, W-residual chains); PE-transpose;
     ACT bounce to SBUF; DMA out. Projection weight loads are WAW-gated
     behind a late DVE memset so they cannot steal the DMA device from the
     gather at t=0.
"""

import numpy as np
import ml_dtypes

import concourse.bass as bass
import concourse.mybir as mybir
import concourse.tile as tile
from concourse import bacc
from concourse.bass_utils import run_bass_kernel_spmd
from concourse.vector_clock import ScopedClock

# ---------------------------------------------------------------- constants
B, S, L = 16, 256, 50
CHAR_DIM = 16
CHAR_VOCAB = 262
FILTERS = [(1, 32), (2, 32), (3, 64), (4, 128), (5, 256), (6, 512), (7, 1024)]
N_FILTERS = 2048
PROJ_DIM = 512
N_CORES = 8
NTOK = B * S
TOK = NTOK // N_CORES        # 512 tokens per core
NPOS = 50
NPAIR = 25                   # char pairs per token
NPPAD = 28                   # padded pair positions (tap reach 55 -> pair 27)
NI = TOK * NPAIR             # 12800 gather indices per core
TABLE_ROWS = 32768           # fixed per-core unique-pair table allocation
KDIM = 112                   # 7 taps * 16 dims
TSPLIT = 22                  # conv positions < TSPLIT read xsA

S_W = 512.0                  # fp8 storage scale for highway/proj weights
S_H = 32.0                   # fp8 storage scale for highway/proj activations
DESCALE = 1.0 / (S_W * S_H)

# per 128-channel tile: valid positions; tile 0 packs w=1,2,3 with tails
CH_TILES = []
CH_TILES.append({"t_main": 48, "tails": [(0, 32, 50), (32, 64, 49), (64, 128, 48)]})
CH_TILES.append({"t_main": 47, "tails": [(0, 128, 47)]})      # w4
for _ in range(2):
    CH_TILES.append({"t_main": 46, "tails": [(0, 128, 46)]})  # w5
for _ in range(4):
    CH_TILES.append({"t_main": 45, "tails": [(0, 128, 45)]})  # w6
for _ in range(8):
    CH_TILES.append({"t_main": 44, "tails": [(0, 128, 44)]})  # w7

BF16 = mybir.dt.bfloat16
FP32 = mybir.dt.float32
FP8 = mybir.dt.float8e4

# drain schedule fractions: (pA, pPoolMerge, pPoolFinale) per phase
PH1 = (0.78, 0.0, 0.0)
PH2 = (0.75, 0.0, 0.0)

_MAX_WAITS_PER_INST = 1


def _patched_drain_and_barrier(self, tick_clock, wait_clock):
    # The walrus build in this container rejects CTRL instructions carrying
    # more than one sem wait; spread the kernel-tail drain waits over NOPs.
    nc = self.nc
    carrier = nc.sync.nop()
    wait_clock.add_sem_waits(carrier.ins, ScopedClock({None: tick_clock.global_clock}))
    si = carrier.ins.sync_info
    waits = list(si.on_wait) if si is not None and si.on_wait else []
    if len(waits) > _MAX_WAITS_PER_INST:
        carrier.ins.sync_info = mybir.SyncInfo(
            on_wait=waits[:_MAX_WAITS_PER_INST],
            on_update=list(si.on_update) if si.on_update else [])
        for i in range(_MAX_WAITS_PER_INST, len(waits), _MAX_WAITS_PER_INST):
            extra = nc.sync.nop()
            extra.ins.sync_info = mybir.SyncInfo(
                on_wait=waits[i:i + _MAX_WAITS_PER_INST], on_update=[])
    nc.sync.drain()
    nc.all_engine_barrier()
    assert self.sems is not None
    popped = nc._tile_sem_poison_stack.pop()
    assert popped is self._sem_poison
    nc.clear_and_free_semaphores(list(self.sems.allocated().values()))
    nc.all_engine_barrier()


tile.TileContext._drain_and_barrier = _patched_drain_and_barrier


class DrainSched:
    """Debt-based allocator: round kind (A/D), fold + merge engines."""

    def __init__(self, nc, pA, unused0=0.0, unused1=0.0):
        self.nc = nc
        self.pA = pA
        self.da = 0.0

    def kind(self):
        self.da += self.pA
        if self.da >= 1.0:
            self.da -= 1.0
            return "A"
        return "D"

    def note_forced(self, k):
        pass

    def fold_eng(self):
        return self.nc.vector


# ---------------------------------------------------------------- device IR
def build_module():
    nc = bacc.Bacc()
    SIdx = NI // 16

    # 256-byte rows (dma_gather granularity); cols 0:32 hold the pair embs
    table = nc.dram_tensor("table", [TABLE_ROWS, 128], BF16,
                           kind="ExternalInput")
    idx = nc.dram_tensor("idx", [128, SIdx], mybir.dt.int16, kind="ExternalInput")
    wconv = nc.dram_tensor("wconv", [KDIM, N_FILTERS], BF16, kind="ExternalInput")
    bconv = nc.dram_tensor("bconv", [128, 16], FP32, kind="ExternalInput")
    # highway weights fp8, host-packed per (layer, j):
    #   [l, j, p(128), cc(8), g(2), half*128+oc] ; g = DoubleRow group
    whw = nc.dram_tensor("whw", [2, 16, 128, 8, 2, 256], FP8, kind="ExternalInput")
    bhw = nc.dram_tensor("bhw", [2, 128, 16, 2], FP32, kind="ExternalInput")
    wproj = nc.dram_tensor("wproj", [128, 8, 2, 512], FP8, kind="ExternalInput")
    wprojc = nc.dram_tensor("wprojc", [128, 8, 2, 512], FP8, kind="ExternalInput")
    bproj = nc.dram_tensor("bproj", [128, 4], FP32, kind="ExternalInput")
    ident = nc.dram_tensor("ident", [128, 128], FP32, kind="ExternalInput")
    out = nc.dram_tensor("out", [TOK, PROJ_DIM], FP32, kind="ExternalOutput")

    with tile.TileContext(nc) as tc:
        with (
            tc.tile_pool(name="xs", bufs=1) as xspool,
            tc.tile_pool(name="consts", bufs=1) as cpool,
        ):
            # ---- constants in (wconv loads after idx; the rest is deferred
            # so nothing delays the gather + xsA stream on the DMA device)
            wconv_t = cpool.tile([KDIM, N_FILTERS], BF16)
            early_t = cpool.tile([128, 16, NTOK // N_CORES // 2], BF16)
            bconv_t = cpool.tile([128, 16], FP32)
            bhw_t = cpool.tile([128, 2, 16, 2], FP32)
            bproj_t = cpool.tile([128, 4], FP32)
            ident_t = cpool.tile([128, 128], FP32)

            # ---- 1. pair-gather char embeddings + build K=112 patch matrix.
            # xsA/xsB viewed [128, m, parity, TOK] so the strided parity
            # interleave is a plain AP (no step slicing).
            NA = TSPLIT // 2          # 11 position pairs in xsA
            NB = (NPOS - TSPLIT) // 2  # 14 in xsB
            xsA = xspool.tile([KDIM, NA, 2, TOK], BF16, name="xsA")
            xsB = xspool.tile([KDIM, NB, 2, TOK], BF16, name="xsB")
            with tc.tile_pool(name="gather", bufs=1) as gpool:
                idx_t = gpool.tile([128, SIdx], mybir.dt.int16)
                nc.sync.dma_start(out=idx_t[:], in_=idx[:])
                nc.sync.dma_start(out=wconv_t[:], in_=wconv[:])
                xg = gpool.tile([128, 1, TOK * NPPAD], BF16)
                nc.vector.memset(xg[0:32, 0, NI:TOK * NPPAD], 0.0)
                # pair-aligned chunks (idx counts): 5,9,7,4 pairs
                chunks = [(0, 2560), (2560, 4608), (7168, 3584), (10752, 2048)]
                for r, (o, cn) in enumerate(chunks):
                    nc.gpsimd.dma_gather(
                        out_ap=xg[:, :, o:o + cn],
                        in_ap=table[:],
                        idxs_ap=idx_t[:, o // 16:(o + cn) // 16],
                        num_idxs=cn,
                        num_idxs_reg=cn,
                        elem_size=128,
                        transpose=True,
                        single_packet=False,
                    )
                    if r > 1:
                        continue
                    # xsA copies, fused over (k, k+1) pairs with contiguous
                    # source partition blocks; split at the chunk-0 boundary
                    # (pair 4) so the early conv rounds start after chunk 0
                    for s in range(2):
                        k = 0
                        while k < 7:
                            p0 = (s + k) // 2
                            par = (s + k) % 2
                            m_lo = 0 if r == 0 else max(0, 5 - p0)
                            m_hi = min(NA, 5 - p0) if r == 0 else NA
                            fuse = par == 0 and k + 1 < 7
                            if m_hi > m_lo:
                                if fuse:
                                    nc.sync.dma_start(
                                        out=xsA[16 * k:16 * (k + 2),
                                                m_lo:m_hi, s, :],
                                        in_=xg[0:32, 0,
                                               TOK * (p0 + m_lo):
                                               TOK * (p0 + m_hi)],
                                    )
                                else:
                                    nc.sync.dma_start(
                                        out=xsA[16 * k:16 * (k + 1),
                                                m_lo:m_hi, s, :],
                                        in_=xg[16 * par:16 * par + 16, 0,
                                               TOK * (p0 + m_lo):
                                               TOK * (p0 + m_hi)],
                                    )
                            k += 2 if fuse else 1
                for s in range(2):
                    k = 0
                    while k < 7:
                        t0 = TSPLIT + s
                        p0 = (t0 + k) // 2
                        par = (t0 + k) % 2
                        if par == 0 and k + 1 < 7:
                            nc.sync.dma_start(
                                out=xsB[16 * k:16 * (k + 2), :, s, :],
                                in_=xg[0:32, 0, TOK * p0:TOK * (p0 + NB)],
                            )
                            k += 2
                        else:
                            nc.sync.dma_start(
                                out=xsB[16 * k:16 * (k + 1), :, s, :],
                                in_=xg[16 * par:16 * par + 16, 0,
                                       TOK * p0:TOK * (p0 + NB)],
                            )
                            k += 1
            nc.sync.dma_start(out=bconv_t[:], in_=bconv[:])
            nc.sync.dma_start(out=bhw_t[:], in_=bhw[:].rearrange("l p j h -> p l j h"))
            nc.sync.dma_start(out=bproj_t[:], in_=bproj[:])
            nc.sync.dma_start(out=ident_t[:], in_=ident[:])
            stackw = tc.tile_pool(name="wppool", bufs=1)
            wppool = stackw.__enter__()
            wproj_t = wppool.tile([128, 8, 2, 512], FP8)
            wprojc_t = wppool.tile([128, 8, 2, 512], FP8)

            HALF = TOK // 2

            def conv_rhs(t, hlo):
                if t < TSPLIT:
                    return xsA[:, t // 2, t % 2, hlo:hlo + HALF]
                tl = t - TSPLIT
                return xsB[:, tl // 2, tl % 2, hlo:hlo + HALF]

            stack = tc.tile_pool(name="hbuf", bufs=1)
            hpool = stack.__enter__()
            stack2 = tc.tile_pool(name="h8buf", bufs=1)
            h8pool = stack2.__enter__()
            stack3 = tc.tile_pool(name="wstream", bufs=4)
            wpool = stack3.__enter__()
            stack4 = tc.tile_pool(name="small", bufs=2)
            spool = stack4.__enter__()
            stack6 = tc.tile_pool(name="accpool", bufs=6)
            accpool = stack6.__enter__()
            convp = None  # phase-2/3 PSUM pool, opened after phase 1

            # per-half persistent tensors (separate tiles avoid false deps)
            h1 = [hpool.tile([128, 16, HALF], BF16, tag=f"h1{s_}", name=f"h1{s_}")
                  for s_ in range(2)]
            hmid = [hpool.tile([128, 16, HALF], BF16, tag=f"hm{s_}", name=f"hm{s_}")
                    for s_ in range(2)]
            # h8 layout: [128, hi/lo, j, n] so (j, j+1) casts batch into one op
            h8c = [h8pool.tile([128, 2, 16, HALF], FP8, tag=f"h8c{s_}", name=f"h8c{s_}")
                   for s_ in range(2)]
            h8m = [h8pool.tile([128, 2, 16, HALF], FP8, tag=f"h8m{s_}", name=f"h8m{s_}")
                   for s_ in range(2)]
            # h8f aliases h8c: layer-0 chains fully consume h8c before
            # layer-1 writes the final activations (WAR handled by deps)
            h8f = h8c

            def cast_pair(h_bf, j0, nj, h8, lo_eng=None):
                # hi = fp8(h * S_H) on ACT; lo = fp8(h*S_H - hi) on DVE/Pool
                nc.scalar.activation(
                    out=h8[:, 0, j0:j0 + nj, :], in_=h_bf[:, j0:j0 + nj, :],
                    func=mybir.ActivationFunctionType.Copy, scale=S_H)
                (lo_eng or nc.vector).scalar_tensor_tensor(
                    out=h8[:, 1, j0:j0 + nj, :], in0=h_bf[:, j0:j0 + nj, :],
                    scalar=S_H, in1=h8[:, 0, j0:j0 + nj, :],
                    op0=mybir.AluOpType.mult, op1=mybir.AluOpType.subtract)

            def conv_half(hf, rpos, pool, sched, early=False):
                """Generator: conv + max-pool drain for token half hf.

                Touches (PSUM reads) emit immediately; folds/merges/finales
                emit one round late via `pending` so no engine head-of-line
                blocks on a cross-engine dependency that is not ready yet.
                """
                from collections import deque
                hlo = HALF * hf
                hw_ = rpos // 2  # acc width
                pending = deque()
                pcast = deque()   # fp8 casts lag one extra round so the ACT
                                  # hi-cast never parks waiting the DVE finale

                def flush(keep):
                    while len(pending) > keep:
                        pending.popleft()()
                    while len(pcast) > max(keep, 1) + 1:
                        pcast.popleft()()

                if early:
                    # warm-up: positions 0..3 of every tile only need the
                    # first gather chunk -> ACT/DVE start ~20us earlier
                    for i, spec in enumerate(CH_TILES):
                        lhsT = wconv_t[:, 128 * i:128 * (i + 1)]
                        P = pool.tile([128, rpos, HALF], FP32, tag=f"ps{rpos}")
                        for r in range(4):
                            nc.tensor.matmul(
                                out=P[:, r, :], lhsT=lhsT,
                                rhs=conv_rhs(r, hlo),
                                start=True, stop=True)
                        flush(1)
                        tmp = spool.tile([128, rpos, HALF], BF16,
                                         tag="astk", bufs=4)
                        nc.scalar.activation(
                            out=tmp[:, 0:4, :], in_=P[:, 0:4, :],
                            func=mybir.ActivationFunctionType.Copy, scale=1.0)

                        def eop(tmp=tmp, i=i):
                            t2e = spool.tile([128, 2, HALF], BF16, tag="f2e")
                            nc.vector.tensor_tensor(
                                out=t2e[:], in0=tmp[:, 0:2, :],
                                in1=tmp[:, 2:4, :], op=mybir.AluOpType.max)
                            nc.vector.tensor_tensor(
                                out=early_t[:, i, :], in0=t2e[:, 0, :],
                                in1=t2e[:, 1, :], op=mybir.AluOpType.max)
                        pending.append(eop)
                DX = 3  # spare direct-write slots for D-round reduces
                for i, spec in enumerate(CH_TILES):
                    lhsT = wconv_t[:, 128 * i:128 * (i + 1)]
                    t_main = spec["t_main"]
                    acc = accpool.tile([128, hw_ + DX, HALF], BF16, tag="acc")
                    first = True
                    dstate = {"next": hw_}
                    t0 = 4 if early else 0
                    while t0 < t_main:
                        nt = min(rpos, t_main - t0)
                        P = pool.tile([128, rpos, HALF], FP32, tag=f"ps{rpos}")
                        for r in range(nt):
                            nc.tensor.matmul(
                                out=P[:, r, :], lhsT=lhsT,
                                rhs=conv_rhs(t0 + r, hlo),
                                start=True, stop=True)
                        flush(1)
                        if nt == rpos and (first or sched.kind() == "A"):
                            if first:
                                sched.note_forced("A")
                            tmp = spool.tile([128, rpos, HALF], BF16,
                                             tag="astk", bufs=4)
                            nc.scalar.activation(
                                out=tmp[:], in_=P[:],
                                func=mybir.ActivationFunctionType.Copy, scale=1.0)
                            eng = sched.fold_eng()
                            if first:
                                def op(eng=eng, tmp=tmp, acc=acc):
                                    eng.tensor_tensor(
                                        out=acc[:, 0:hw_, :],
                                        in0=tmp[:, 0:hw_, :],
                                        in1=tmp[:, hw_:rpos, :],
                                        op=mybir.AluOpType.max)
                                first = False
                            else:
                                def op(eng=eng, tmp=tmp, acc=acc):
                                    fh = spool.tile([128, hw_, HALF], BF16,
                                                    tag="fh", bufs=3)
                                    eng.tensor_tensor(
                                        out=fh[:], in0=tmp[:, 0:hw_, :],
                                        in1=tmp[:, hw_:rpos, :],
                                        op=mybir.AluOpType.max)
                                    eng.tensor_tensor(
                                        out=acc[:, 0:hw_, :],
                                        in0=acc[:, 0:hw_, :], in1=fh[:],
                                        op=mybir.AluOpType.max)
                            pending.append(op)
                        elif nt == 1:
                            nc.vector.tensor_tensor(
                                out=acc[:, 0, :], in0=acc[:, 0, :],
                                in1=P[:, 0, :], op=mybir.AluOpType.max)
                        elif dstate["next"] < hw_ + DX:
                            # D-led: reduce straight into a spare acc slot
                            nc.vector.reduce_max(
                                out=acc[:, dstate["next"], :],
                                in_=P[:, 0:nt, :].rearrange("p t n -> p n t"),
                                axis=mybir.AxisListType.X)
                            dstate["next"] += 1
                        else:
                            part = spool.tile([128, HALF], BF16, tag="part",
                                              bufs=3)
                            nc.vector.reduce_max(
                                out=part[:],
                                in_=P[:, 0:nt, :].rearrange("p t n -> p n t"),
                                axis=mybir.AxisListType.X)

                            def op(part=part, acc=acc):
                                nc.vector.tensor_tensor(
                                    out=acc[:, 0, :], in0=acc[:, 0, :],
                                    in1=part[:], op=mybir.AluOpType.max)
                            pending.append(op)
                        t0 += nt
                    # ragged tails (tile 0): positions t_main..50 on partition
                    # subranges; reduces touch PSUM now, merges deferred
                    if spec["tails"][0][2] > t_main:
                        nt = spec["tails"][0][2] - t_main
                        P = pool.tile([128, rpos, HALF], FP32, tag=f"ps{rpos}")
                        for r in range(nt):
                            nc.tensor.matmul(
                                out=P[:, r, :], lhsT=lhsT,
                                rhs=conv_rhs(t_main + r, hlo),
                                start=True, stop=True)
                        for (lo, hi, g_cnt) in spec["tails"]:
                            g_nt = g_cnt - t_main
                            if g_nt <= 0:
                                continue
                            if g_nt == 1:
                                nc.vector.tensor_tensor(
                                    out=acc[lo:hi, 0, :], in0=acc[lo:hi, 0, :],
                                    in1=P[lo:hi, 0, :], op=mybir.AluOpType.max)
                            else:
                                part = spool.tile([128, HALF], BF16, tag="part",
                                                  bufs=3)
                                nc.vector.reduce_max(
                                    out=part[lo:hi, :],
                                    in_=P[lo:hi, 0:g_nt, :].rearrange(
                                        "p t n -> p n t"),
                                    axis=mybir.AxisListType.X)

                                def op(part=part, acc=acc, lo=lo, hi=hi):
                                    nc.vector.tensor_tensor(
                                        out=acc[lo:hi, 1, :],
                                        in0=acc[lo:hi, 1, :],
                                        in1=part[lo:hi, :],
                                        op=mybir.AluOpType.max)
                                pending.append(op)

                    used = dstate["next"]

                    def finale(i=i, acc=acc, used=used):
                        pre = spool.tile([128, HALF], BF16, tag="pre")
                        cw = used
                        while cw > 2:
                            if cw % 2:
                                nc.vector.tensor_tensor(
                                    out=acc[:, 0, :], in0=acc[:, 0, :],
                                    in1=acc[:, cw - 1, :],
                                    op=mybir.AluOpType.max)
                                cw -= 1
                            h = cw // 2
                            nc.vector.tensor_tensor(
                                out=acc[:, 0:h, :], in0=acc[:, 0:h, :],
                                in1=acc[:, h:cw, :], op=mybir.AluOpType.max)
                            cw = h
                        nc.vector.tensor_tensor(
                            out=pre[:], in0=acc[:, 0, :], in1=acc[:, 1, :],
                            op=mybir.AluOpType.max)
                        if early:
                            nc.vector.tensor_tensor(
                                out=pre[:], in0=pre[:],
                                in1=early_t[:, i, :], op=mybir.AluOpType.max)
                        nc.vector.tensor_scalar(
                            out=h1[hf][:, i, :], in0=pre[:],
                            scalar1=bconv_t[:, i:i + 1], scalar2=0.0,
                            op0=mybir.AluOpType.add, op1=mybir.AluOpType.max)
                    pending.append(finale)
                    if i % 2 == 1:
                        def cst(i=i):
                            cast_pair(h1[hf], i - 1, 2, h8c[hf])
                        pcast.append(cst)
                    yield
                flush(0)
                while pcast:
                    pcast.popleft()()

            def hw_mm_chain(p_out, wslab, h8, ofs, lo=True):
                # W8 x (h_hi [+ h_lo]); Wl correction skipped for the highway.
                # The sigmoid gate path also skips the h_lo chain (the gate
                # damps the quantization error; verified within tolerance).
                nhl = 2 if lo else 1
                for hl in range(nhl):
                    for cc in range(8):
                        nc.tensor.matmul(
                            out=p_out, lhsT=wslab[:, cc, :, ofs:ofs + 128],
                            rhs=h8[:, hl, 2 * cc:2 * cc + 2, :],
                            start=(hl == 0 and cc == 0),
                            stop=(hl == nhl - 1 and cc == 7),
                            perf_mode=mybir.MatmulPerfMode.DoubleRow)

            def hw_mm_chain_proj(p_out, h8, ofs):
                for hl in range(2):
                    for cc in range(8):
                        nc.tensor.matmul(
                            out=p_out, lhsT=wproj_t[:, cc, :, ofs:ofs + 128],
                            rhs=h8[:, hl, 2 * cc:2 * cc + 2, :],
                            start=(hl == 0 and cc == 0), stop=False,
                            perf_mode=mybir.MatmulPerfMode.DoubleRow)
                for cc in range(8):
                    nc.tensor.matmul(
                        out=p_out, lhsT=wprojc_t[:, cc, :, ofs:ofs + 128],
                        rhs=h8[:, 0, 2 * cc:2 * cc + 2, :],
                        start=False, stop=(cc == 7),
                        perf_mode=mybir.MatmulPerfMode.DoubleRow)

            def hw_half(hf):
                """Generator: highway l0+l1 + proj for token half hf.

                PE chains emit immediately; ACT/DVE epilogues lag one j so
                neither engine parks at its queue head waiting on a chain.
                """
                from collections import deque
                pending = deque()

                def flush(keep):
                    while len(pending) > keep:
                        pending.popleft()()

                state = {}
                for layer in range(2):
                    h_in = h1[hf] if layer == 0 else hmid[hf]
                    h8_in = h8c[hf] if layer == 0 else h8m[hf]
                    h8_out = h8m[hf] if layer == 0 else h8f[hf]
                    for j in range(16):
                        wslab = wpool.tile([128, 8, 2, 256], FP8, tag="wslab")
                        nc.sync.dma_start(out=wslab[:], in_=whw[layer, j])
                        hp = convp.tile([128, 2, HALF], FP32, tag="hwps",
                                        name="hp", bufs=2)
                        p_nl = hp[:, 0, :]
                        p_g = hp[:, 1, :]
                        hw_mm_chain(p_nl, wslab, h8_in, 0)
                        hw_mm_chain(p_g, wslab, h8_in, 128, lo=False)
                        flush(1)

                        def epi(layer=layer, j=j, p_nl=p_nl, p_g=p_g,
                                h_in=h_in, h8_out=h8_out):
                            nl = spool.tile([128, HALF], BF16, tag="nl")
                            gt = spool.tile([128, HALF], BF16, tag="gt")
                            nc.scalar.activation(
                                out=nl[:], in_=p_nl,
                                func=mybir.ActivationFunctionType.Relu,
                                bias=bhw_t[:, layer, j, 0:1], scale=DESCALE)
                            nc.scalar.activation(
                                out=gt[:], in_=p_g,
                                func=mybir.ActivationFunctionType.Sigmoid,
                                bias=bhw_t[:, layer, j, 1:2], scale=DESCALE)
                            d = spool.tile([128, HALF], BF16, tag="d")
                            nc.vector.tensor_tensor(
                                out=d[:], in0=h_in[:, j, :], in1=nl[:],
                                op=mybir.AluOpType.subtract)
                            m = spool.tile([128, HALF], BF16, tag="m")
                            nc.vector.tensor_mul(out=m[:], in0=gt[:], in1=d[:])
                            if layer == 0:
                                nc.vector.tensor_add(
                                    out=hmid[hf][:, j, :], in0=nl[:], in1=m[:])
                                if j % 2 == 1:
                                    cast_pair(hmid[hf], j - 1, 2, h8_out)
                            else:
                                if j % 2 == 0:
                                    state["htp"] = spool.tile(
                                        [128, 2, HALF], BF16, tag="htp",
                                        name="htp")
                                htp = state["htp"]
                                nc.vector.tensor_add(
                                    out=htp[:, j % 2, :], in0=nl[:], in1=m[:])
                                if j % 2 == 1:
                                    nc.scalar.activation(
                                        out=h8_out[:, 0, j - 1:j + 1, :],
                                        in_=htp[:],
                                        func=mybir.ActivationFunctionType.Copy,
                                        scale=S_H)
                                    nc.vector.scalar_tensor_tensor(
                                        out=h8_out[:, 1, j - 1:j + 1, :],
                                        in0=htp[:], scalar=S_H,
                                        in1=h8_out[:, 0, j - 1:j + 1, :],
                                        op0=mybir.AluOpType.mult,
                                        op1=mybir.AluOpType.subtract)
                        pending.append(epi)
                        yield
                    # layer barrier: next layer's chains read every h8 column
                    flush(0)
                # projection + transpose + out for this half
                hlo = HALF * hf
                for j2 in range(4):
                    hp = convp.tile([128, 2, HALF], FP32, tag="hwps",
                                    name="hp", bufs=2)
                    p_o = hp[:, 0, :]
                    hw_mm_chain_proj(p_o, h8f[hf], 128 * j2)
                    flush(1)

                    def proj_epi(j2=j2, hp=hp, p_o=p_o):
                        ot = spool.tile([128, HALF], FP32, tag="ot")
                        nc.scalar.activation(
                            out=ot[:], in_=p_o,
                            func=mybir.ActivationFunctionType.Identity,
                            bias=bproj_t[:, j2:j2 + 1], scale=DESCALE)
                        for m4 in range(2):
                            p_t = hp[:, 1, 128 * m4:128 * (m4 + 1)]
                            nc.tensor.transpose(
                                out=p_t, in_=ot[:, 128 * m4:128 * (m4 + 1)],
                                identity=ident_t[:])
                            ob = spool.tile([128, 128], FP32, tag="ob")
                            nc.scalar.copy(out=ob[:], in_=p_t)
                            row0 = hlo + 128 * m4
                            nc.sync.dma_start(
                                out=out[row0:row0 + 128,
                                        128 * j2:128 * (j2 + 1)],
                                in_=ob[:])
                    pending.append(proj_epi)
                    yield
                flush(0)

            # ---- phase 1: conv half A, 8-position rounds, all 8 PSUM banks
            sched1 = DrainSched(nc, *PH1)
            with tc.tile_pool(name="convp8", bufs=4, space="PSUM") as p8pool:
                for _ in conv_half(0, 4, p8pool, sched1, early=True):
                    pass
            stack5 = tc.tile_pool(name="convp", bufs=2, space="PSUM")
            convp = stack5.__enter__()
            # WAW-gate the projection-weight loads behind a DVE op that sits
            # late in DVE program order, so they cannot steal the DMA device
            # from the gather at t=0 (the sim schedules by readiness)
            nc.vector.memset(wproj_t[0:1, 0:1, 0:1, 0:1], 0.0)
            nc.vector.memset(wprojc_t[0:1, 0:1, 0:1, 0:1], 0.0)
            nc.sync.dma_start(out=wproj_t[:], in_=wproj[:])
            nc.sync.dma_start(out=wprojc_t[:], in_=wprojc[:])
            # ---- phase 2: conv half B interleaved with highway+proj half A
            sched2 = DrainSched(nc, *PH2)
            genB = conv_half(1, 6, convp, sched2)
            genA = hw_half(0)
            unitsB, unitsA = 16, 36
            credit = 0.0
            doneB = doneA = False
            while not (doneB and doneA):
                credit += unitsA / unitsB
                if not doneB:
                    doneB = next(genB, "end") == "end"
                while credit >= 1.0 and not doneA:
                    doneA = next(genA, "end") == "end"
                    credit -= 1.0
                if doneB:
                    while not doneA:
                        doneA = next(genA, "end") == "end"
            # ---- phase 3: highway+proj half B
            for _ in hw_half(1):
                pass

            for st in (stack5, stack6, stack4, stack3, stack2, stack, stackw):
                st.__exit__(None, None, None)

    nc.compile()
    return nc


_CACHED = {}


def _prep(inputs):
    """Host-side layout prep: sharding, pair tables, weight packing."""
    chars = np.asarray(inputs["chars"]).astype(np.int64).reshape(NTOK, L)
    pairs = chars[:, 0::2] * CHAR_VOCAB + chars[:, 1::2]   # [NTOK, 25]

    emb = np.asarray(inputs["char_emb"], np.float32)

    wc = np.zeros((7, CHAR_DIM, N_FILTERS), np.float32)
    off = 0
    for fi, (w, n) in enumerate(FILTERS):
        cw = np.asarray(inputs[f"conv_w_{fi}"], np.float32)
        wc[:w, :, off:off + n] = cw.transpose(2, 1, 0)
        off += n
    wconv = wc.reshape(KDIM, N_FILTERS).astype(ml_dtypes.bfloat16)
    bconv = np.concatenate([np.asarray(inputs[f"conv_b_{i}"], np.float32)
                            for i in range(7)])
    bconv_dev = bconv.reshape(16, 128).T.copy()

    # highway weights: fp8 W8 packed for DoubleRow streaming.
    whw8 = np.zeros((2, 16, 128, 8, 2, 256), np.float32)
    bhw = np.zeros((2, 128, 16, 2), np.float32)
    for l in range(2):
        W = np.asarray(inputs[f"hw_w_{l}"], np.float32)   # (4096, 2048)
        bb = np.asarray(inputs[f"hw_b_{l}"], np.float32)
        Ws = W * S_W
        W8 = Ws.astype(ml_dtypes.float8_e4m3).astype(np.float32)
        W8T = W8.T  # (2048 ic, 4096 oc)
        for j in range(16):
            for hf in range(2):
                oc0 = 2048 * hf + 128 * j
                for cc in range(8):
                    for g in range(2):
                        cb = 2 * cc + g
                        blk8 = W8T[128 * cb:128 * (cb + 1), oc0:oc0 + 128]
                        whw8[l, j, :, cc, g, 128 * hf:128 * hf + 128] = blk8
            bhw[l, :, j, 0] = bb[128 * j:128 * (j + 1)]
            bhw[l, :, j, 1] = bb[2048 + 128 * j:2048 + 128 * (j + 1)]
    whw8 = whw8.astype(ml_dtypes.float8_e4m3)

    Wp = np.asarray(inputs["proj_w"], np.float32) * S_W  # (512, 2048)
    Wp8 = Wp.astype(ml_dtypes.float8_e4m3).astype(np.float32)
    Wpl = (Wp - Wp8).astype(ml_dtypes.float8_e4m3).astype(np.float32)
    Wp8T = Wp8.T  # (2048, 512)
    WplT = Wpl.T
    wproj8 = np.zeros((128, 8, 2, 512), np.float32)
    wprojc8 = np.zeros((128, 8, 2, 512), np.float32)
    for cc in range(8):
        for g in range(2):
            cb = 2 * cc + g
            wproj8[:, cc, g, :] = Wp8T[128 * cb:128 * (cb + 1), :]
            wprojc8[:, cc, g, :] = WplT[128 * cb:128 * (cb + 1), :]
    wproj8 = wproj8.astype(ml_dtypes.float8_e4m3)
    wprojc8 = wprojc8.astype(ml_dtypes.float8_e4m3)
    bproj = np.zeros((128, 4), np.float32)
    bp = np.asarray(inputs["proj_b"], np.float32)
    for j2 in range(4):
        bproj[:, j2] = bp[128 * j2:128 * (j2 + 1)]

    ident = np.eye(128, dtype=np.float32)

    shared = dict(wconv=wconv, bconv=bconv_dev, whw=whw8,
                  bhw=bhw, wproj=wproj8, wprojc=wprojc8,
                  bproj=bproj, ident=ident)

    in_maps = []
    for core in range(N_CORES):
        cp = pairs[core * TOK:(core + 1) * TOK]            # [512, 25]
        uniq, inv = np.unique(cp, return_inverse=True)
        assert len(uniq) <= TABLE_ROWS, len(uniq)
        tbl = np.zeros((TABLE_ROWS, 128), np.float32)
        tbl[:len(uniq), 0:CHAR_DIM] = emb[uniq // CHAR_VOCAB]
        tbl[:len(uniq), CHAR_DIM:2 * CHAR_DIM] = emb[uniq % CHAR_VOCAB]
        idx_flat = inv.reshape(TOK, NPAIR).T.reshape(-1).astype(np.int16)
        idx16 = idx_flat.reshape(NI // 16, 16).T.copy()
        idx16 = np.tile(idx16, (8, 1))
        m = dict(shared)
        m["table"] = tbl.astype(ml_dtypes.bfloat16)
        m["idx"] = idx16
        in_maps.append(m)
    return in_maps


def kernel(**inputs) -> np.ndarray:
    if "nc" not in _CACHED:
        _CACHED["nc"] = build_module()
    nc = _CACHED["nc"]
    in_maps = _prep(inputs)
    res = run_bass_kernel_spmd(nc, in_maps, core_ids=list(range(N_CORES)))
    full = np.concatenate([r["out"] for r in res.results], axis=0)
    return full.reshape(B, S, PROJ_DIM)
